# revision 1
# baseline (speedup 1.0000x reference)
"""MultiHeadLatentAttention (MLA) Trainium2 Bass kernel.

Problem: B=2, S=2048, D=2048, H=16 heads, d_nope=128, d_rope=64, d_head=128,
q_latent=768, kv_latent=512. Causal attention, rmsnorm'd latents, half-dim RoPE.

Sharding (8 cores): core c handles batch b=c//4 and head group g=c%4 (4 heads).
The small latent down-projections are replicated within each batch group;
W_uq/W_qr/W_uk/W_kr/W_uv are column-sharded by head; W_o row-sharded; the
4 partial outputs per batch are summed on the host.

Device dataflow (everything in "transposed" layout, features on partitions,
sequence on the free dim, so every matmul uses weights as-stored for lhsT and
all moving operands have free dim 512):
  P0: q_latT/kv_latT = W_d*^T @ x^T, rmsnorm via ones-matmul sumsq +
      exp(-0.5*ln(ms)) + gpsimd partition_broadcast; latents round-trip DRAM.
  P1: kT (nope + rope pairs) and v (natural layout) up-projections.
  P2: per 512-wide q-block: q up-proj on demand, scores^T = k^T(tile)^T q^T
      with additive causal mask applied via an identity matmul of a
      precomputed mask; exp batched over [128,1024] two-bank PSUM tiles on
      ACT; softmax denominator accumulated on the DVE and finished with a
      gpsimd partition_all_reduce (broadcast sum), 1/den = exp(-ln(den));
      PV matmuls (staggered one exp-pair behind the scores matmuls) give
      out^T; then y^T += W_o^T out^T, deferred one q-block for overlap.
      Projection chains alternate between two PSUM pools for 4-deep
      chain pipelining within the 8-bank budget.

All matmuls run as float32r (FP22 multiply, fp32 accumulate, 1 cycle/row with
512-wide moving operands) — measured end-to-end relative error vs the fp32
reference is ~4e-4.

MLA_ALLGATHER=1 switches to an S-sharded down-projection with a device
AllGather of the latents within each 4-core batch group (saves ~110us of
replicated down-projection matmuls per core, but the cost model prices the
10.5MB gather at ~250us, so it is off by default).
"""
import math
import os
from contextlib import ExitStack

import numpy as np

import concourse.bass as bass
import concourse.bass_isa as bass_isa
import concourse.bacc as bacc
import concourse.mybir as mybir
import concourse.tile as tile
from concourse.bass_utils import run_bass_kernel_spmd

F32 = mybir.dt.float32
F32R = mybir.dt.float32r
AF = mybir.ActivationFunctionType

B, S_FULL, D = 2, 2048, 2048
H, DN, DR, DH = 16, 128, 64, 128
QL, KVL = 768, 512
EPS = 1e-6
SCALE = 1.0 / math.sqrt(DH)
MASK_NEG = -1e6
NCORES = 8
NKT = D // 128          # 16 contraction tiles over D
NLQ = QL // 128         # 6
NLKV = KVL // 128       # 4
NDT = D // 128          # 16 output D tiles


def _rope_apply(nc, pool, ps, c4s, s4s, out_ap):
    """Half-dim rope on a pair tile [128, 512] (h_even x1|x2 | h_odd x1|x2).

    out = ps * c4 + shuf(ps) * s4,  shuf swaps the 32-blocks within each 64.
    ps is PSUM; out_ap is SBUF.
    """
    shuf = pool.tile([128, 512], F32, tag="rope_shuf")
    nc.vector.tensor_copy(shuf[0:32, :], ps[32:64, :])
    nc.vector.tensor_copy(shuf[32:64, :], ps[0:32, :])
    nc.vector.tensor_copy(shuf[64:96, :], ps[96:128, :])
    nc.vector.tensor_copy(shuf[96:128, :], ps[64:96, :])
    t1 = pool.tile([128, 512], F32, tag="rope_t1")
    nc.vector.tensor_mul(t1[:], ps[:], c4s)
    nc.vector.tensor_mul(out_ap, shuf[:], s4s)
    nc.vector.tensor_add(out_ap, out_ap, t1[:])


PHASE_MARKS = {}


def build_nc(S=S_FULL, allgather=None):
    assert S % 512 == 0
    n_sb = S // 512
    n_st = S // 128
    if allgather is None:
        allgather = bool(int(os.environ.get("MLA_ALLGATHER", "0")))
    PHASE_MARKS.clear()

    nc = bacc.Bacc("TRN2", target_bir_lowering=False, debug=False,
                   num_devices=NCORES)

    x_cols = 512 if allgather else S
    xT_d = nc.dram_tensor("xT", [D, x_cols], F32R, kind="ExternalInput")
    wdq_d = nc.dram_tensor("W_dq", [D, QL], F32R, kind="ExternalInput")
    wdkv_d = nc.dram_tensor("W_dkv", [D, KVL], F32R, kind="ExternalInput")
    wuq_d = nc.dram_tensor("Wuq", [QL, 512], F32R, kind="ExternalInput")
    wqr_d = nc.dram_tensor("Wqr", [QL, 256], F32R, kind="ExternalInput")
    wuk_d = nc.dram_tensor("Wuk", [KVL, 512], F32R, kind="ExternalInput")
    wkr_d = nc.dram_tensor("Wkr", [KVL, 256], F32R, kind="ExternalInput")
    wuv_d = nc.dram_tensor("Wuv", [KVL, 512], F32R, kind="ExternalInput")
    wo_d = nc.dram_tensor("Wo", [512, D], F32R, kind="ExternalInput")
    c4_d = nc.dram_tensor("c4", [128, S], F32, kind="ExternalInput")
    s4_d = nc.dram_tensor("s4", [128, S], F32, kind="ExternalInput")
    mask_d = nc.dram_tensor("mask_big", [128, 896], F32R, kind="ExternalInput")
    id_d = nc.dram_tensor("ident", [128, 128], F32R, kind="ExternalInput")
    ones_d = nc.dram_tensor("ones_col", [128, 1], F32R, kind="ExternalInput")
    yT_d = nc.dram_tensor("yT", [D, S], F32, kind="ExternalOutput")

    def col3(dram_ap, p=128):
        # [R, C] dram slice -> [128, R//128, C] tiled AP
        return dram_ap.rearrange("(t p) c -> p t c", p=p)

    with tile.TileContext(nc) as tc:
        with (
            tc.tile_pool(name="const", bufs=1) as constp,
            tc.tile_pool(name="ps_mm", bufs=2, space="PSUM") as ps_mm,
            tc.tile_pool(name="ps_o", bufs=2, space="PSUM") as ps_op,
            tc.tile_pool(name="dram", bufs=1, space="DRAM") as dramp,
        ):
            if allgather:
                lat_in = dramp.tile([QL + KVL, 512], F32R)
                lat_out = dramp.tile([n_sb * (QL + KVL), 512], F32R)

                def qlat_src(sb):
                    return lat_out[sb * 1280:sb * 1280 + QL, :]

                def kvlat_src(sb):
                    return lat_out[sb * 1280 + QL:(sb + 1) * 1280, :]
            else:
                qlat_ds = [
                    dramp.tile([QL, 512], F32R, tag=f"qlat{i}", name=f"qlat{i}")
                    for i in range(n_sb)
                ]
                kvlat_ds = [
                    dramp.tile([KVL, 512], F32R, tag=f"kvlat{i}",
                               name=f"kvlat{i}")
                    for i in range(n_sb)
                ]

                def qlat_src(sb):
                    return qlat_ds[sb][:]

                def kvlat_src(sb):
                    return kvlat_ds[sb][:]
            def alt_ps(i):
                if i % 2 == 0:
                    return ps_mm.tile([128, 512], F32, tag="mm", name="ps")
                return ps_op.tile([128, 512], F32, tag="pv", name="ps")

            mask_t = constp.tile([128, 896], F32R)
            id_t = constp.tile([128, 128], F32R)
            ones_t = constp.tile([128, 1], F32R)
            eps_t = constp.tile([1, 1], F32)
            # ---------------- P0: down-projections + rmsnorm ----------------
            PHASE_MARKS["P0"] = nc.next_id()
            p1_stack = ExitStack()
            p1lat = p1_stack.enter_context(tc.tile_pool(name="p1lat", bufs=2))
            kvl_tiles = {}
            with (
                tc.tile_pool(name="p0w", bufs=1) as p0w,
                tc.tile_pool(name="p0x", bufs=4) as p0x,
                tc.tile_pool(name="p0latq", bufs=2) as p0latq,
                tc.tile_pool(name="p0latkv", bufs=1) as p0latkv,
                tc.tile_pool(name="p0tmp", bufs=1) as p0tmp,
                tc.tile_pool(name="p0rsb", bufs=2) as p0rsb,
                tc.tile_pool(name="ps_den", bufs=1, space="PSUM") as ps_denp,
                tc.tile_pool(name="ps_p0", bufs=2, space="PSUM") as ps_p0,
            ):
                def alt3_ps(i):
                    if i % 3 == 2:
                        return ps_p0.tile([128, 512], F32, tag="p0", name="ps")
                    return alt_ps(i % 3)
                wdq_t = p0w.tile([128, NKT, QL], F32R)
                wdkv_t = p0w.tile([128, NKT, KVL], F32R)

                def load_wdq(lt):
                    nc.sync.dma_start(
                        wdq_t[:, :, 128 * lt:128 * (lt + 1)],
                        col3(wdq_d[:, 128 * lt:128 * (lt + 1)]),
                    )

                for g4 in range(4):
                    nc.sync.dma_start(
                        wdq_t[:, 4 * g4:4 * (g4 + 1), 0:128],
                        col3(wdq_d[:, 0:128])[:, 4 * g4:4 * (g4 + 1), :],
                    )
                n_local = 1 if allgather else n_sb
                for sb in range(n_local):
                    cs = slice(512 * sb, 512 * (sb + 1))
                    xh = [
                        p0x.tile([128, 8, 512], F32R, tag="xsb", name=f"xsb{i}")
                        for i in range(2)
                    ]
                    for kt in range(NKT):
                        nc.sync.dma_start(
                            xh[kt // 8][:, kt % 8, :],
                            col3(xT_d[:, cs])[:, kt, :],
                        )
                    if sb == 0:
                        nc.sync.dma_start(ones_t[:], ones_d[:])
                        nc.vector.memset(eps_t[:], EPS)
                        for lt in range(1, NLQ):
                            load_wdq(lt)
                        for lt in range(NLKV):
                            nc.sync.dma_start(
                                wdkv_t[:, :, 128 * lt:128 * (lt + 1)],
                                col3(wdkv_d[:, 128 * lt:128 * (lt + 1)]),
                            )
                    for latname, w_t, nl in (("q", wdq_t, NLQ), ("kv", wdkv_t, NLKV)):
                        raw = (p0latq if latname == "q" else p0latkv).tile(
                            [128, nl, 512], F32R, tag=f"raw{latname}",
                            name=f"raw{latname}")
                        ps_ss = ps_denp.tile([1, 512], F32, tag="den")
                        for lt in range(nl):
                            ps = alt3_ps(lt)
                            for kt in range(NKT):
                                nc.tensor.matmul(
                                    ps[:],
                                    w_t[:, kt, 128 * lt:128 * (lt + 1)],
                                    xh[kt // 8][:, kt % 8, :],
                                    start=(kt == 0), stop=(kt == NKT - 1),
                                )
                            nc.scalar.copy(raw[:, lt, :], ps[:])
                            sq = p0tmp.tile([128, 512], F32R, tag="sq")
                            nc.vector.tensor_mul(sq[:], raw[:, lt, :], raw[:, lt, :])
                            nc.tensor.matmul(
                                ps_ss[:], ones_t[:], sq[:],
                                start=(lt == 0), stop=(lt == nl - 1),
                            )
                        lrow = p0tmp.tile([1, 512], F32, tag="lrow")
                        nc.scalar.activation(
                            lrow[:], ps_ss[:], AF.Ln, scale=1.0 / (128 * nl),
                            bias=eps_t[:],
                        )
                        rrow = p0tmp.tile([1, 512], F32R, tag="rrow")
                        nc.scalar.activation(rrow[:], lrow[:], AF.Exp, scale=-0.5)
                        rsb = p0rsb.tile([128, 512], F32R, tag="rsb")
                        nc.gpsimd.partition_broadcast(rsb[:], rrow[:])
                        for lt in range(nl):
                            nc.vector.tensor_mul(raw[:, lt, :], raw[:, lt, :], rsb[:])
                        if allgather:
                            lat_ap = (lat_in[0:QL, :] if latname == "q"
                                      else lat_in[QL:QL + KVL, :])
                        else:
                            lat_ap = (qlat_ds if latname == "q"
                                      else kvlat_ds)[sb][:]
                        nc.sync.dma_start(col3(lat_ap), raw[:])
                    if sb == 0 and not allgather:
                        kvl0 = p1lat.tile([128, NLKV, 512], F32R, tag="kvl",
                                          name="kvl0")
                        nc.sync.dma_start(kvl0[:], col3(kvlat_src(0)))
                        kvl_tiles[0] = kvl0

            nc.sync.dma_start(mask_t[:], mask_d[:])
            nc.sync.dma_start(id_t[:], id_d[:])
            if allgather:
                nc.gpsimd.collective_compute(
                    "AllGather",
                    mybir.AluOpType.bypass,
                    replica_groups=[[0, 1, 2, 3], [4, 5, 6, 7]],
                    ins=[lat_in[:]],
                    outs=[lat_out[:]],
                )

            # ---------------- P1: k/v up-projections ----------------
            PHASE_MARKS["P1"] = nc.next_id()
            persist_stack = ExitStack()
            persist = persist_stack.enter_context(
                tc.tile_pool(name="persist", bufs=1)
            )
            kTn_t = persist.tile([128, 4, S], F32R)    # nope k^T per head
            kTr_t = persist.tile([128, 2, S], F32R)    # rope k^T per pair
            v_t = persist.tile([128, n_st, 512], F32R)  # v natural
            p2w_stack = ExitStack()
            p2w = p2w_stack.enter_context(tc.tile_pool(name="p2w", bufs=1))
            p2lat = p2w_stack.enter_context(tc.tile_pool(name="p2lat", bufs=1))
            wuq_t = p2w.tile([128, NLQ, 512], F32R)
            wqr_t = p2w.tile([128, NLQ, 256], F32R)
            qlat0 = p2lat.tile([128, NLQ, 512], F32R, tag="qlat", name="qlat0")
            with (
                tc.tile_pool(name="p1w", bufs=1) as p1w,
                tc.tile_pool(name="p1tmp", bufs=2) as p1tmp,
            ):
                wuk_t = p1w.tile([128, NLKV, 512], F32R)
                wkr_t = p1w.tile([128, NLKV, 256], F32R)
                wuv_t = p1w.tile([128, NLKV, 512], F32R)
                nc.sync.dma_start(wuk_t[:], col3(wuk_d[:]))
                for sb in range(n_sb):
                    cs = slice(512 * sb, 512 * (sb + 1))
                    if sb in kvl_tiles:
                        kvl = kvl_tiles[sb]
                    else:
                        kvl = p1lat.tile([128, NLKV, 512], F32R, tag="kvl",
                                         name=f"kvl{sb}")
                        nc.sync.dma_start(kvl[:], col3(kvlat_src(sb)))
                    if sb == 0:
                        nc.sync.dma_start(wkr_t[:], col3(wkr_d[:]))
                        nc.sync.dma_start(wuv_t[:], col3(wuv_d[:]))
                    c4s = p1tmp.tile([128, 512], F32, tag="c4")
                    s4s = p1tmp.tile([128, 512], F32, tag="s4")
                    nc.sync.dma_start(c4s[:], c4_d[:, cs])
                    nc.sync.dma_start(s4s[:], s4_d[:, cs])
                    for h in range(4):
                        ps = alt_ps(h)
                        for kl in range(NLKV):
                            nc.tensor.matmul(
                                ps[:], wuk_t[:, kl, 128 * h:128 * (h + 1)],
                                kvl[:, kl, :],
                                start=(kl == 0), stop=(kl == NLKV - 1),
                            )
                        nc.scalar.copy(kTn_t[:, h, cs], ps[:])
                    for pr in range(2):
                        ps = alt_ps(pr)
                        for kl in range(NLKV):
                            nc.tensor.matmul(
                                ps[:], wkr_t[:, kl, 128 * pr:128 * (pr + 1)],
                                kvl[:, kl, :],
                                start=(kl == 0), stop=(kl == NLKV - 1),
                            )
                        _rope_apply(nc, p1tmp, ps, c4s[:], s4s[:], kTr_t[:, pr, cs])
                    for stl in range(4):
                        st = 4 * sb + stl
                        ps = alt_ps(stl)
                        for kl in range(NLKV):
                            nc.tensor.matmul(
                                ps[:],
                                kvl[:, kl, 128 * stl:128 * (stl + 1)],
                                wuv_t[:, kl, :],
                                start=(kl == 0), stop=(kl == NLKV - 1),
                            )
                        nc.scalar.copy(v_t[:, st, :], ps[:])
                    if sb == min(1, n_sb - 1):
                        nc.sync.dma_start(wuq_t[:], col3(wuq_d[:]))
                        nc.sync.dma_start(wqr_t[:], col3(wqr_d[:]))
                        nc.sync.dma_start(qlat0[:], col3(qlat_src(0)))

            # ---------------- P2: attention + W_o ----------------
            PHASE_MARKS["P2"] = nc.next_id()
            with (
                tc.tile_pool(name="p2wo", bufs=3) as p2wo,
                tc.tile_pool(name="p2q", bufs=4) as p2q,
                tc.tile_pool(name="p2qr", bufs=2) as p2qr,
                tc.tile_pool(name="p2exp", bufs=2) as p2exp,
                tc.tile_pool(name="ps_s", bufs=2, space="PSUM") as ps_sp,
                tc.tile_pool(name="p2acc", bufs=2) as p2acc,
                tc.tile_pool(name="p2acc1", bufs=1) as p2acc1,
                tc.tile_pool(name="p2out", bufs=9) as p2out,
                tc.tile_pool(name="p2tmp", bufs=2) as p2tmp,
                tc.tile_pool(name="p2y", bufs=2) as p2y,
            ):
                def emit_wo(out_tiles, cs):
                    for dt in range(NDT):
                        woc = p2wo.tile([128, 4, 128], F32R, tag="wo", name="woc")
                        nc.sync.dma_start(
                            woc[:], col3(wo_d[:, 128 * dt:128 * (dt + 1)])
                        )
                        ps_y = ps_mm.tile([128, 512], F32, tag="mm", name="ps")
                        for h in range(4):
                            nc.tensor.matmul(
                                ps_y[:], woc[:, h, :],
                                out_tiles[h][:],
                                start=(h == 0), stop=(h == 3),
                            )
                        ystage = p2y.tile([128, 512], F32, tag="y")
                        nc.vector.tensor_copy(ystage[:], ps_y[:])
                        nc.sync.dma_start(
                            yT_d[128 * dt:128 * (dt + 1), cs], ystage[:]
                        )

                prev_out = None
                prev_cs = None
                for qb in range(n_sb):
                    cs = slice(512 * qb, 512 * (qb + 1))
                    if qb == 0:
                        qlat = qlat0
                    else:
                        qlat = p2lat.tile([128, NLQ, 512], F32R, tag="qlat",
                                          name=f"qlat{qb}")
                        nc.sync.dma_start(qlat[:], col3(qlat_src(qb)))
                    c4s = p2tmp.tile([128, 512], F32, tag="c4")
                    s4s = p2tmp.tile([128, 512], F32, tag="s4")
                    nc.sync.dma_start(c4s[:], c4_d[:, cs])
                    nc.sync.dma_start(s4s[:], s4_d[:, cs])
                    qr_tiles = []
                    for pr in range(2):
                        ps = alt_ps(pr)
                        for ql in range(NLQ):
                            nc.tensor.matmul(
                                ps[:], wqr_t[:, ql, 128 * pr:128 * (pr + 1)],
                                qlat[:, ql, :],
                                start=(ql == 0), stop=(ql == NLQ - 1),
                            )
                        qr = p2qr.tile([128, 512], F32R, tag="qr")
                        _rope_apply(nc, p2tmp, ps, c4s[:], s4s[:], qr[:])
                        qr_tiles.append(qr)
                    qn_tiles = []
                    for h in range(4):
                        ps = alt_ps(h)
                        for ql in range(NLQ):
                            nc.tensor.matmul(
                                ps[:], wuq_t[:, ql, 128 * h:128 * (h + 1)],
                                qlat[:, ql, :],
                                start=(ql == 0), stop=(ql == NLQ - 1),
                            )
                        qn = p2q.tile([128, 512], F32R, tag="qn", name=f"qn{h}")
                        nc.vector.tensor_copy(qn[:], ps[:])
                        qn_tiles.append(qn)
                    out_tiles = []
                    for h in range(4):
                        qn = qn_tiles[h]
                        qr = qr_tiles[h // 2]
                        pb = 64 * (h % 2)
                        nkt = 4 * (qb + 1)
                        ps_o = ps_op.tile([128, 512], F32, tag="pv")
                        # softmax denominator: DVE accumulates exp half 0,
                        # GPSIMD half 1; exp batches two score tiles per ACT op.
                        dacc = p2acc.tile([128, 512], F32R, tag="dacc")

                        def emit_pv(exp_pair, pk, npair, ps_o=ps_o, h=h):
                            for j in (0, 1):
                                kt = 2 * pk + j
                                nc.tensor.matmul(
                                    ps_o[:],
                                    v_t[:, kt, 128 * h:128 * (h + 1)],
                                    exp_pair[:, 512 * j:512 * (j + 1)],
                                    start=(kt == 0), stop=(kt == 2 * npair - 1),
                                )

                        npair = nkt // 2
                        pend = []   # (exp pair tile, pk) one pair behind
                        for pk in range(npair):
                            ps_s = ps_sp.tile([128, 1024], F32, tag="scores")
                            for j in (0, 1):
                                kt = 2 * pk + j
                                ks = slice(128 * kt, 128 * (kt + 1))
                                delta = 128 * kt - 512 * qb
                                diag = delta >= 0
                                half = ps_s[:, 512 * j:512 * (j + 1)]
                                nc.tensor.matmul(
                                    half, kTn_t[:, h, ks], qn[:],
                                    start=True, stop=False,
                                )
                                nc.tensor.matmul(
                                    half,
                                    kTr_t[pb:pb + 64, h // 2, ks],
                                    qr[pb:pb + 64, :],
                                    start=False, stop=not diag,
                                )
                                if diag:
                                    nc.tensor.matmul(
                                        half, id_t[:],
                                        mask_t[:, 384 - delta:896 - delta],
                                        start=False, stop=True,
                                    )
                            exp_t = p2exp.tile([128, 1024], F32R, tag="exp")
                            nc.scalar.activation(
                                exp_t[:], ps_s[:], AF.Exp, scale=SCALE
                            )
                            if pk == 0:
                                nc.vector.tensor_copy(dacc[:], exp_t[:, 0:512])
                            else:
                                nc.vector.tensor_add(
                                    dacc[:], dacc[:], exp_t[:, 0:512])
                            nc.vector.tensor_add(
                                dacc[:], dacc[:], exp_t[:, 512:1024])
                            pend.append((exp_t, pk))
                            if len(pend) > 1:
                                emit_pv(*pend.pop(0), npair)
                        for e in pend:
                            emit_pv(*e, npair)
                        red = p2acc1.tile([128, 512], F32R, tag="dred")
                        nc.gpsimd.partition_all_reduce(
                            red[:], dacc[:], 128, bass_isa.ReduceOp.add
                        )
                        nc.scalar.activation(red[:], red[:], AF.Ln)
                        rsb = p2tmp.tile([128, 512], F32R, tag="rsb")
                        nc.scalar.activation(rsb[:], red[:], AF.Exp, scale=-1.0)
                        out_t = p2out.tile([128, 512], F32R, tag="outT")
                        nc.vector.tensor_mul(out_t[:], ps_o[:], rsb[:])
                        out_tiles.append(out_t)
                    if prev_out is not None:
                        emit_wo(prev_out, prev_cs)
                    prev_out, prev_cs = out_tiles, cs
                emit_wo(prev_out, prev_cs)
            p2w_stack.close()
            persist_stack.close()
            p1_stack.close()

    nc.compile()
    return nc


def host_prep(inputs, S=S_FULL):
    """Build the 8 per-core input maps from the full problem inputs."""
    x = np.ascontiguousarray(np.asarray(inputs["x"], np.float32))
    cosT = np.ascontiguousarray(np.asarray(inputs["rope_cos"], np.float32).T)
    sinT = np.ascontiguousarray(np.asarray(inputs["rope_sin"], np.float32).T)
    c4 = np.ascontiguousarray(np.concatenate([cosT, cosT, cosT, cosT], 0))
    s4 = np.ascontiguousarray(np.concatenate([-sinT, sinT, -sinT, sinT], 0))
    qw = np.asarray(inputs["q_norm_w"], np.float32)
    kvw = np.asarray(inputs["kv_norm_w"], np.float32)
    W_uq = np.asarray(inputs["W_uq"], np.float32) * qw[:, None]
    W_qr = np.asarray(inputs["W_qr"], np.float32) * qw[:, None]
    W_uk = np.asarray(inputs["W_uk"], np.float32) * kvw[:, None]
    W_kr = np.asarray(inputs["W_kr"], np.float32) * kvw[:, None]
    W_uv = np.asarray(inputs["W_uv"], np.float32) * kvw[:, None]
    W_o = np.asarray(inputs["W_o"], np.float32)
    W_dq = np.ascontiguousarray(np.asarray(inputs["W_dq"], np.float32))
    W_dkv = np.ascontiguousarray(np.asarray(inputs["W_dkv"], np.float32))

    cgrid = np.arange(896)[None, :] - 384
    igrid = np.arange(128)[:, None]
    mask_big = np.where(cgrid >= igrid, 0.0, MASK_NEG).astype(np.float32)
    ident = np.eye(128, dtype=np.float32)

    allgather = bool(int(os.environ.get("MLA_ALLGATHER", "0")))
    in_maps = []
    for c in range(NCORES):
        b, g = c // 4, c % 4
        hs = slice(4 * g * DN, 4 * (g + 1) * DN)
        hr = slice(4 * g * DR, 4 * (g + 1) * DR)
        xT_c = x[b].T[:, 512 * g:512 * (g + 1)] if allgather else x[b].T
        in_maps.append(dict(
            xT=np.ascontiguousarray(xT_c),
            W_dq=W_dq, W_dkv=W_dkv,
            Wuq=np.ascontiguousarray(W_uq[:, hs]),
            Wqr=np.ascontiguousarray(W_qr[:, hr]),
            Wuk=np.ascontiguousarray(W_uk[:, hs]),
            Wkr=np.ascontiguousarray(W_kr[:, hr]),
            Wuv=np.ascontiguousarray(W_uv[:, hs]),
            Wo=np.ascontiguousarray(W_o[512 * g:512 * (g + 1), :]),
            c4=c4, s4=s4, mask_big=mask_big, ident=ident,
            ones_col=np.ones((128, 1), np.float32),
        ))
    return in_maps


_NC_CACHE = {}


def kernel(**inputs) -> np.ndarray:
    S = np.asarray(inputs["x"]).shape[1]
    if S not in _NC_CACHE:
        _NC_CACHE[S] = build_nc(S)
    nc = _NC_CACHE[S]
    in_maps = host_prep(inputs, S)
    trace = bool(os.environ.get("MLA_TRACE"))
    res = run_bass_kernel_spmd(
        nc, in_maps, core_ids=list(range(NCORES)), trace=trace
    )
    if trace:
        print(f"HW exec time: {res.exec_time_ns} ns")
        print(f"trace: {res.instructions_and_trace[1] if res.instructions_and_trace else None}")
    y = np.empty((B, S, D), np.float32)
    for b in range(B):
        acc = res.results[4 * b]["yT"].astype(np.float32).copy()
        for g in range(1, 4):
            acc += res.results[4 * b + g]["yT"]
        y[b] = acc.T
    return y



# revision 13
# speedup vs baseline: 1.5869x; 1.5869x over previous
"""MultiHeadLatentAttention (MLA) Trainium2 Bass kernel, v2.

Problem: B=2, S=2048, D=2048, H=16 heads, d_nope=128, d_rope=64, d_head=128,
q_latent=768, kv_latent=512. Causal attention, rmsnorm'd latents, half-dim RoPE.

Sharding (8 cores): core c handles batch b=c//4 and head group g=c%4 (4 heads).
The small latent down-projections are replicated within each batch group;
W_uq/W_qr/W_uk/W_kr/W_uv are column-sharded by head; W_o row-sharded; the
4 partial outputs per batch are summed on the host.

v2 precision/engine plan (metric = InstructionCostModel timeline):
  - scores matmul in fp8e4m3 with MatmulPerfMode.DoubleRow: the nope(128) and
    zero-padded rope(64) contractions are packed as the two DoubleRow k-tiles,
    so each 128x512 score tile costs 256 PE cycles instead of 1024.
    End-to-end error from quantizing qn/kn/qr/kr to fp8 measured 1.33e-2.
  - causal mask added in the same PSUM group by a fp8 DoubleRow matmul of
    60*I against a {0,-240} mask table (-14400 pre-scale -> exp()=2e-9).
  - down-projections run as a 3-term fp8 DoubleRow residual split
    (W8@x8 + W8@x8r + W8r@x8, all host-prepared, scaled so residuals stay
    in fp8 normal range) -- fp8 speed at ~0.1% error.
  - everything else (up-projections, PV, W_o) in fp16: same 1 cycle/row as
    f32r but half the DMA/SBUF and 2x DVE elementwise.
  - softmax: exp on ACT (fp16 out, scale=SCALE/(sq*sk)), denominator
    accumulated on DVE in fp16 pairs, partition_all_reduce on Pool,
    reciprocal on DVE (InstReciprocal) -- no Ln, so no act-table thrashing.
  - rmsnorm rsqrt via DVE tensor_scalar pow(-0.5) (fallback Ln/Exp).
  - W_o and latents stay resident in SBUF; output yT written fp16.
"""
import math
import os
from contextlib import ExitStack

import numpy as np
import ml_dtypes

import concourse.bass as bass
import concourse.bass_isa as bass_isa
import concourse.bacc as bacc
import concourse.mybir as mybir
import concourse.tile as tile
from concourse.bass_utils import run_bass_kernel_spmd

F32 = mybir.dt.float32
F32R = mybir.dt.float32r
F16 = mybir.dt.float16
F8 = mybir.dt.float8e4
AF = mybir.ActivationFunctionType
DR_MODE = mybir.MatmulPerfMode.DoubleRow

B, S_FULL, D = 2, 2048, 2048
H, DN, DRR, DH = 16, 128, 64, 128
QL, KVL = 768, 512
EPS = 1e-6
SCALE = 1.0 / math.sqrt(DH)
NCORES = 8
NKT = D // 128          # 16 contraction tiles over D
NKP = NKT // 2          # 8 DoubleRow pairs
NLQ = QL // 128         # 6
NLKV = KVL // 128       # 4
NDT = D // 128          # 16 output D tiles

# fp8 scaling for the residual-split down-projection: x' = x*AX, W' = W*BW
# so both the quantized tensors and their residuals stay in fp8 normal range.
AX = 32.0
BW = 256.0
PSUM_UNSCALE = 1.0 / (AX * BW)
# score operand quantization scale (qn8 = 8*qn etc.)
SQ8 = 8.0
EXP_SCALE = SCALE / (SQ8 * SQ8)
MASK_ID = 60.0          # mask matmul: 60 * (-240) * 1 plane = -14400 pre-scale

PHASE_MARKS = {}


def build_nc(S=S_FULL):
    assert S % 512 == 0
    n_sb = S // 512
    n_st = S // 128
    PHASE_MARKS.clear()

    nc = bacc.Bacc("TRN2", target_bir_lowering=False, debug=False,
                   num_devices=NCORES)

    x8_d = nc.dram_tensor("x8", [D, S], F8, kind="ExternalInput")
    x8r_d = nc.dram_tensor("x8r", [D, S], F8, kind="ExternalInput")
    wdq_d = nc.dram_tensor("W_dq8", [D, QL], F8, kind="ExternalInput")
    wdqr_d = nc.dram_tensor("W_dq8r", [D, QL], F8, kind="ExternalInput")
    wdkv_d = nc.dram_tensor("W_dkv8", [D, KVL], F8, kind="ExternalInput")
    wdkvr_d = nc.dram_tensor("W_dkv8r", [D, KVL], F8, kind="ExternalInput")
    wuq_d = nc.dram_tensor("Wuq", [QL, 512], F16, kind="ExternalInput")
    wqr_d = nc.dram_tensor("Wqr", [QL, 256], F16, kind="ExternalInput")
    wuk_d = nc.dram_tensor("Wuk", [KVL, 512], F16, kind="ExternalInput")
    wkr_d = nc.dram_tensor("Wkr", [KVL, 256], F16, kind="ExternalInput")
    wuv_d = nc.dram_tensor("Wuv", [KVL, 512], F16, kind="ExternalInput")
    wo_d = nc.dram_tensor("Wo", [512, D], F16, kind="ExternalInput")
    c4_d = nc.dram_tensor("c4", [128, S], F16, kind="ExternalInput")
    s4_d = nc.dram_tensor("s4", [128, S], F16, kind="ExternalInput")
    mask_d = nc.dram_tensor("mask8", [128, 2, 896], F8, kind="ExternalInput")
    id_d = nc.dram_tensor("id8", [128, 2, 128], F8, kind="ExternalInput")
    ones_d = nc.dram_tensor("ones8", [128, 1], F8, kind="ExternalInput")
    yT_d = nc.dram_tensor("yT", [D, S], F16, kind="ExternalOutput")
    debug = bool(int(os.environ.get("MLA_DEBUG", "0")))
    if debug:
        dbg_qlat_d = nc.dram_tensor("dbg_qlat", [128, NLQ, S], F16,
                                    kind="ExternalOutput")
        dbg_kvlat_d = nc.dram_tensor("dbg_kvlat", [128, NLKV, S], F16,
                                     kind="ExternalOutput")
        dbg_kT8_d = nc.dram_tensor("dbg_kT8", [128, 4, 2, S], F16,
                                   kind="ExternalOutput")
        dbg_v_d = nc.dram_tensor("dbg_v", [128, n_st, 512], F16,
                                 kind="ExternalOutput")

    def col3(dram_ap, p=128):
        # [R, C] dram slice -> [128, R//128, C] tiled AP
        return dram_ap.rearrange("(t p) c -> p t c", p=p)

    def rope_pair(nc, pool, outs, ps, c4s, s4s):
        """Half-dim rope on a 2-head pair tile [128, 512] in PSUM.

        out = ps * c4 + shuf(ps) * s4, shuf swaps 32-blocks within each 64.
        Stages through fp16 SBUF so the DVE muls run in 2x mode. `outs` is a
        list of (out_ap, pslice) fp8 destinations.
        """
        rs = pool.tile([128, 512], F16, tag="rope_rs")
        nc.scalar.copy(rs[:], ps[:])
        shuf = pool.tile([128, 512], F16, tag="rope_shuf")
        nc.vector.tensor_copy(shuf[0:32, :], rs[32:64, :])
        nc.vector.tensor_copy(shuf[32:64, :], rs[0:32, :])
        nc.vector.tensor_copy(shuf[64:96, :], rs[96:128, :])
        nc.vector.tensor_copy(shuf[96:128, :], rs[64:96, :])
        t1 = pool.tile([128, 512], F16, tag="rope_t1")
        nc.vector.tensor_mul(t1[:], rs[:], c4s)
        nc.vector.tensor_mul(shuf[:], shuf[:], s4s)
        for out_ap, psl in outs:
            nc.vector.tensor_add(out_ap, t1[psl, :], shuf[psl, :])

    with tile.TileContext(nc) as tc:
        with (
            tc.tile_pool(name="const", bufs=1) as constp,
            tc.tile_pool(name="ps_mm", bufs=2, space="PSUM") as ps_mm,
            tc.tile_pool(name="ps_o", bufs=2, space="PSUM") as ps_op,
        ):
            def alt_ps(i):
                if i % 2 == 0:
                    return ps_mm.tile([128, 512], F32, tag="mm", name="ps")
                return ps_op.tile([128, 512], F32, tag="pv", name="ps")

            mask_t = constp.tile([128, 2, 896], F8)
            id_t = constp.tile([128, 2, 128], F8)
            ones_t = constp.tile([128, 1], F8)

            # persistent SBUF state
            persist_stack = ExitStack()
            persist = persist_stack.enter_context(
                tc.tile_pool(name="persist", bufs=1))
            # kT8: per head (rope_padded, nope) planes, fp8 stationary
            kT8_t = persist.tile([128, 4, 2, S], F8)
            v_t = persist.tile([128, n_st, 512], F16)
            wo_t = persist.tile([128, 4, D], F16)
            qlat_t = persist.tile([128, NLQ, S], F16)
            kvlat_t = persist.tile([128, NLKV, S], F16)

            # ---------------- P0: down-projections + rmsnorm ----------------
            PHASE_MARKS["P0"] = nc.next_id()
            p0_stack = ExitStack()
            p0w = p0_stack.enter_context(tc.tile_pool(name="p0w", bufs=1))
            wdq_t = p0w.tile([128, NKT, QL], F8)
            wdqr_t = p0w.tile([128, NKT, QL], F8)
            wdkv_t = p0w.tile([128, NKT, KVL], F8)
            wdkvr_t = p0w.tile([128, NKT, KVL], F8)
            with (
                tc.tile_pool(name="p0x", bufs=2) as p0x,
                tc.tile_pool(name="p0raw", bufs=2) as p0raw,
                tc.tile_pool(name="p0sq", bufs=2) as p0sq,
                tc.tile_pool(name="p0tmp", bufs=2) as p0tmp,
                tc.tile_pool(name="ps_den", bufs=1, space="PSUM") as ps_denp,
                tc.tile_pool(name="ps_p0", bufs=2, space="PSUM") as ps_p0,
            ):
                def alt3_ps(i):
                    if i % 3 == 2:
                        return ps_p0.tile([128, 512], F32, tag="p0", name="ps")
                    return alt_ps(i % 3)

                # zero the pad halves of the rope planes once (rope data for
                # even heads lives at partitions 0:64, odd heads at 64:128)
                for h in range(4):
                    lo = 0 if h % 2 else 64
                    nc.vector.memset(kT8_t[lo:lo + 64, h, 0, :], 0.0)

                for sb in range(n_sb):
                    cs = slice(512 * sb, 512 * (sb + 1))
                    xh = p0x.tile([128, NKT, 512], F8, tag="x8", name=f"x8_{sb}")
                    xrh = p0x.tile([128, NKT, 512], F8, tag="x8r",
                                   name=f"x8r_{sb}")
                    for kt in range(0, NKT, 4):
                        nc.sync.dma_start(
                            xh[:, kt:kt + 4, :],
                            col3(x8_d[:, cs])[:, kt:kt + 4, :])
                    for kt in range(0, NKT, 4):
                        nc.sync.dma_start(
                            xrh[:, kt:kt + 4, :],
                            col3(x8r_d[:, cs])[:, kt:kt + 4, :])
                    if sb == 0:
                        nc.sync.dma_start(ones_t[:], ones_d[:])
                        nc.sync.dma_start(mask_t[:], mask_d[:])
                        nc.sync.dma_start(id_t[:], id_d[:])
                        for lt in range(NLQ):
                            lsl = slice(128 * lt, 128 * (lt + 1))
                            nc.sync.dma_start(wdq_t[:, :, lsl],
                                              col3(wdq_d[:, lsl]))
                            nc.sync.dma_start(wdqr_t[:, :, lsl],
                                              col3(wdqr_d[:, lsl]))
                        for lt in range(NLKV):
                            lsl = slice(128 * lt, 128 * (lt + 1))
                            nc.sync.dma_start(wdkv_t[:, :, lsl],
                                              col3(wdkv_d[:, lsl]))
                            nc.sync.dma_start(wdkvr_t[:, :, lsl],
                                              col3(wdkvr_d[:, lsl]))
                        # W_o resident load (overlaps with P0 compute)
                        for dt4 in range(0, NDT, 4):
                            nc.sync.dma_start(
                                wo_t[:, :, 128 * dt4:128 * (dt4 + 4)],
                                col3(wo_d[:, 128 * dt4:128 * (dt4 + 4)]))
                    for latname, w_t, wr_t, nl, lat_t in (
                        ("q", wdq_t, wdqr_t, NLQ, qlat_t),
                        ("kv", wdkv_t, wdkvr_t, NLKV, kvlat_t),
                    ):
                        raw = p0raw.tile([128, nl, 512], F16, tag=f"raw{latname}",
                                         name=f"raw{latname}")
                        sq = p0sq.tile([128, nl, 512], F8, tag=f"sq{latname}",
                                       name=f"sq{latname}")
                        ps_ss = ps_denp.tile([1, 512], F32, tag="den")
                        for lt in range(nl):
                            ps = alt3_ps(lt)
                            lsl = slice(128 * lt, 128 * (lt + 1))
                            for term_w, term_x in (
                                (w_t, xh), (w_t, xrh), (wr_t, xh),
                            ):
                                first = term_w is w_t and term_x is xh
                                last = term_w is wr_t
                                for kp in range(NKP):
                                    nc.tensor.matmul(
                                        ps[:],
                                        term_w[:, 2 * kp:2 * kp + 2, lsl],
                                        term_x[:, 2 * kp:2 * kp + 2, :],
                                        start=(first and kp == 0),
                                        stop=(last and kp == NKP - 1),
                                        perf_mode=DR_MODE,
                                    )
                            nc.scalar.activation(
                                raw[:, lt, :], ps[:], AF.Copy,
                                scale=PSUM_UNSCALE)
                            nc.scalar.activation(
                                sq[:, lt, :], ps[:], AF.Square,
                                scale=PSUM_UNSCALE)
                            nc.tensor.matmul(
                                ps_ss[:], ones_t[:], sq[:, lt, :],
                                start=(lt == 0), stop=(lt == nl - 1),
                            )
                        # rsqrt of mean square: sqrt(1/m) via DVE reciprocal
                        # + ACT Sqrt (Copy/Square live in the sqrt act table,
                        # so P0 needs no act-table reloads)
                        mrow = p0tmp.tile([1, 512], F32, tag="mrow")
                        nc.vector.tensor_scalar(
                            mrow[:], ps_ss[:], 1.0 / (128 * nl), EPS,
                            mybir.AluOpType.mult, mybir.AluOpType.add)
                        rrec = p0tmp.tile([1, 512], F32, tag="rrec")
                        nc.vector.reciprocal(rrec[:], mrow[:])
                        rrow = p0tmp.tile([1, 512], F16, tag="rrow")
                        nc.scalar.activation(rrow[:], rrec[:], AF.Sqrt)
                        rsb = p0tmp.tile([128, 512], F16, tag="rsb")
                        nc.gpsimd.partition_broadcast(rsb[:], rrow[:])
                        for lt in range(nl):
                            nc.vector.tensor_mul(
                                lat_t[:, lt, cs], raw[:, lt, :], rsb[:])
            p0_stack.close()

            # ---------------- P1: k/v up-projections ----------------
            PHASE_MARKS["P1"] = nc.next_id()
            p2w_stack = ExitStack()
            p2w = p2w_stack.enter_context(tc.tile_pool(name="p2w", bufs=1))
            wuq_t = p2w.tile([128, NLQ, 512], F16)
            wqr_t = p2w.tile([128, NLQ, 256], F16)
            with (
                tc.tile_pool(name="p1w", bufs=1) as p1w,
                tc.tile_pool(name="p1tmp", bufs=2) as p1tmp,
            ):
                wuk_t = p1w.tile([128, NLKV, 512], F16)
                wkr_t = p1w.tile([128, NLKV, 256], F16)
                wuv_t = p1w.tile([128, NLKV, 512], F16)
                nc.sync.dma_start(wuk_t[:], col3(wuk_d[:]))
                for sb in range(n_sb):
                    cs = slice(512 * sb, 512 * (sb + 1))
                    if sb == 0:
                        nc.sync.dma_start(wkr_t[:], col3(wkr_d[:]))
                        nc.sync.dma_start(wuv_t[:], col3(wuv_d[:]))
                        nc.sync.dma_start(wuq_t[:], col3(wuq_d[:]))
                        nc.sync.dma_start(wqr_t[:], col3(wqr_d[:]))
                    c4s = p1tmp.tile([128, 512], F16, tag="c4")
                    s4s = p1tmp.tile([128, 512], F16, tag="s4")
                    nc.sync.dma_start(c4s[:], c4_d[:, cs])
                    nc.sync.dma_start(s4s[:], s4_d[:, cs])
                    for h in range(4):
                        ps = alt_ps(h)
                        for kl in range(NLKV):
                            nc.tensor.matmul(
                                ps[:], wuk_t[:, kl, 128 * h:128 * (h + 1)],
                                kvlat_t[:, kl, cs],
                                start=(kl == 0), stop=(kl == NLKV - 1),
                            )
                        nc.scalar.activation(
                            kT8_t[:, h, 1, cs], ps[:], AF.Copy, scale=SQ8)
                    for pr in range(2):
                        ps = alt_ps(pr)
                        for kl in range(NLKV):
                            nc.tensor.matmul(
                                ps[:], wkr_t[:, kl, 128 * pr:128 * (pr + 1)],
                                kvlat_t[:, kl, cs],
                                start=(kl == 0), stop=(kl == NLKV - 1),
                            )
                        he, ho = 2 * pr, 2 * pr + 1
                        rope_pair(
                            nc, p1tmp, [
                                (kT8_t[0:64, he, 0, cs], slice(0, 64)),
                                (kT8_t[64:128, ho, 0, cs], slice(64, 128)),
                            ], ps, c4s[:], s4s[:])
                    for stl in range(4):
                        st = 4 * sb + stl
                        ps = alt_ps(stl)
                        for kl in range(NLKV):
                            nc.tensor.matmul(
                                ps[:],
                                kvlat_t[:, kl, 512 * sb + 128 * stl:
                                        512 * sb + 128 * (stl + 1)],
                                wuv_t[:, kl, :],
                                start=(kl == 0), stop=(kl == NLKV - 1),
                            )
                        nc.scalar.copy(v_t[:, st, :], ps[:])

            if debug:
                nc.sync.dma_start(dbg_qlat_d[:], qlat_t[:])
                nc.sync.dma_start(dbg_kvlat_d[:], kvlat_t[:])
                nc.sync.dma_start(dbg_v_d[:], v_t[:])
                nc.gpsimd.dma_start(dbg_kT8_d[:], kT8_t[:])

            # ---------------- P2: attention + W_o ----------------
            PHASE_MARKS["P2"] = nc.next_id()
            with (
                tc.tile_pool(name="p2q", bufs=2) as p2q,
                tc.tile_pool(name="p2exp", bufs=2) as p2exp,
                tc.tile_pool(name="ps_s", bufs=2, space="PSUM") as ps_sp,
                tc.tile_pool(name="p2acc", bufs=2) as p2acc,
                tc.tile_pool(name="p2acc1", bufs=1) as p2acc1,
                tc.tile_pool(name="p2out", bufs=9) as p2out,
                tc.tile_pool(name="p2tmp", bufs=2) as p2tmp,
                tc.tile_pool(name="p2y", bufs=2) as p2y,
            ):
                def emit_wo(out_tiles, cs):
                    for dt in range(NDT):
                        ps_y = ps_mm.tile([128, 512], F32, tag="mm", name="ps")
                        for h in range(4):
                            nc.tensor.matmul(
                                ps_y[:], wo_t[:, h, 128 * dt:128 * (dt + 1)],
                                out_tiles[h][:],
                                start=(h == 0), stop=(h == 3),
                            )
                        ystage = p2y.tile([128, 512], F16, tag="y")
                        nc.vector.tensor_copy(ystage[:], ps_y[:])
                        nc.sync.dma_start(
                            yT_d[128 * dt:128 * (dt + 1), cs], ystage[:])

                prev_out = None
                prev_cs = None
                for qb in range(n_sb):
                    cs = slice(512 * qb, 512 * (qb + 1))
                    c4s = p2tmp.tile([128, 512], F16, tag="c4")
                    s4s = p2tmp.tile([128, 512], F16, tag="s4")
                    nc.sync.dma_start(c4s[:], c4_d[:, cs])
                    nc.sync.dma_start(s4s[:], s4_d[:, cs])
                    # q8 moving layout: slots (qr01, qn0, qn1, qr23, qn2, qn3)
                    q8 = p2q.tile([128, 6, 512], F8, tag="q8", name=f"q8_{qb}")
                    for pr in range(2):
                        ps = alt_ps(pr)
                        for ql in range(NLQ):
                            nc.tensor.matmul(
                                ps[:], wqr_t[:, ql, 128 * pr:128 * (pr + 1)],
                                qlat_t[:, ql, cs],
                                start=(ql == 0), stop=(ql == NLQ - 1),
                            )
                        rope_pair(nc, p2tmp,
                                  [(q8[:, 3 * pr, :], slice(0, 128))],
                                  ps, c4s[:], s4s[:])
                    for h in range(4):
                        ps = alt_ps(h)
                        for ql in range(NLQ):
                            nc.tensor.matmul(
                                ps[:], wuq_t[:, ql, 128 * h:128 * (h + 1)],
                                qlat_t[:, ql, cs],
                                start=(ql == 0), stop=(ql == NLQ - 1),
                            )
                        slot = (1, 2, 4, 5)[h]
                        nc.scalar.activation(
                            q8[:, slot, :], ps[:], AF.Copy, scale=SQ8)

                    def q8_mov(h):
                        base = 3 * (h // 2)
                        if h % 2 == 0:
                            return q8[:, base:base + 2, :]
                        return q8[:, base:base + 3:2, :]

                    out_tiles = []
                    for h in range(4):
                        nkt = 4 * (qb + 1)
                        npair = nkt // 2
                        ps_o = ps_op.tile([128, 512], F32, tag="pv")
                        dacc = p2acc.tile([128, 1024], F16, tag="dacc")

                        def emit_pv(exp_pair, pk, npair, ps_o=ps_o, h=h):
                            for j in (0, 1):
                                kt = 2 * pk + j
                                nc.tensor.matmul(
                                    ps_o[:],
                                    v_t[:, kt, 128 * h:128 * (h + 1)],
                                    exp_pair[:, 512 * j:512 * (j + 1)],
                                    start=(kt == 0), stop=(kt == 2 * npair - 1),
                                )

                        pend = []   # (exp pair tile, pk) one pair behind
                        for pk in range(npair):
                            ps_s = ps_sp.tile([128, 1024], F32, tag="scores")
                            for j in (0, 1):
                                kt = 2 * pk + j
                                ks = slice(128 * kt, 128 * (kt + 1))
                                delta = 128 * kt - 512 * qb
                                diag = delta >= 0
                                half = ps_s[:, 512 * j:512 * (j + 1)]
                                nc.tensor.matmul(
                                    half, kT8_t[:, h, :, ks], q8_mov(h),
                                    start=True, stop=not diag,
                                    perf_mode=DR_MODE,
                                )
                                if diag:
                                    nc.tensor.matmul(
                                        half, id_t[:],
                                        mask_t[:, :, 384 - delta:896 - delta],
                                        start=False, stop=True,
                                        perf_mode=DR_MODE,
                                    )
                            exp_t = p2exp.tile([128, 1024], F16, tag="exp")
                            nc.scalar.activation(
                                exp_t[:], ps_s[:], AF.Exp, scale=EXP_SCALE)
                            if pk == 0:
                                nc.vector.tensor_copy(dacc[:], exp_t[:])
                            else:
                                nc.vector.tensor_add(dacc[:], dacc[:], exp_t[:])
                            pend.append((exp_t, pk))
                            if len(pend) > 1:
                                emit_pv(*pend.pop(0), npair)
                        for e in pend:
                            emit_pv(*e, npair)
                        dfold = p2acc1.tile([128, 512], F16, tag="dfold")
                        nc.vector.tensor_add(
                            dfold[:], dacc[:, 0:512], dacc[:, 512:1024])
                        dred = p2acc1.tile([128, 512], F32, tag="dred")
                        nc.gpsimd.partition_all_reduce(
                            dred[:], dfold[:], 128, bass_isa.ReduceOp.add)
                        rsb = p2tmp.tile([128, 512], F32, tag="rsbd")
                        nc.vector.reciprocal(rsb[:], dred[:])
                        out_t = p2out.tile([128, 512], F16, tag="outT")
                        nc.vector.tensor_mul(out_t[:], ps_o[:], rsb[:])
                        out_tiles.append(out_t)
                    if prev_out is not None:
                        emit_wo(prev_out, prev_cs)
                    prev_out, prev_cs = out_tiles, cs
                emit_wo(prev_out, prev_cs)
            p2w_stack.close()
            persist_stack.close()

    nc.compile()
    return nc


def host_prep(inputs, S=S_FULL):
    """Build the 8 per-core input maps from the full problem inputs."""
    FP8 = ml_dtypes.float8_e4m3

    def to8(a):
        return np.ascontiguousarray(a).astype(FP8)

    def split8(a, scale):
        hi = (a * scale).astype(FP8)
        lo = (a * scale - hi.astype(np.float32)).astype(FP8)
        return hi, lo

    x = np.asarray(inputs["x"], np.float32)
    cosT = np.asarray(inputs["rope_cos"], np.float32).T
    sinT = np.asarray(inputs["rope_sin"], np.float32).T
    c4 = np.concatenate([cosT, cosT, cosT, cosT], 0) * SQ8
    s4 = np.concatenate([-sinT, sinT, -sinT, sinT], 0) * SQ8
    c4 = np.ascontiguousarray(c4).astype(np.float16)
    s4 = np.ascontiguousarray(s4).astype(np.float16)
    qw = np.asarray(inputs["q_norm_w"], np.float32)
    kvw = np.asarray(inputs["kv_norm_w"], np.float32)
    W_uq = np.asarray(inputs["W_uq"], np.float32) * qw[:, None]
    W_qr = np.asarray(inputs["W_qr"], np.float32) * qw[:, None]
    W_uk = np.asarray(inputs["W_uk"], np.float32) * kvw[:, None]
    W_kr = np.asarray(inputs["W_kr"], np.float32) * kvw[:, None]
    W_uv = np.asarray(inputs["W_uv"], np.float32) * kvw[:, None]
    W_o = np.asarray(inputs["W_o"], np.float32)
    W_dq = np.asarray(inputs["W_dq"], np.float32)
    W_dkv = np.asarray(inputs["W_dkv"], np.float32)

    wdq8, wdq8r = split8(W_dq, BW)
    wdkv8, wdkv8r = split8(W_dkv, BW)

    # mask table: plane 0 = {0, -240} causal pattern, plane 1 = 0
    cgrid = np.arange(896)[None, :] - 384
    igrid = np.arange(128)[:, None]
    mask8 = np.zeros((128, 2, 896), np.float32)
    mask8[:, 0, :] = np.where(cgrid >= igrid, 0.0, -240.0)
    mask8 = mask8.astype(FP8)
    id8 = np.zeros((128, 2, 128), np.float32)
    id8[:, 0, :] = MASK_ID * np.eye(128, dtype=np.float32)
    id8 = id8.astype(FP8)
    ones8 = np.ones((128, 1), np.float32).astype(FP8)

    in_maps = []
    for c in range(NCORES):
        b, g = c // 4, c % 4
        hs = slice(4 * g * DN, 4 * (g + 1) * DN)
        hr = slice(4 * g * DRR, 4 * (g + 1) * DRR)
        xT = np.ascontiguousarray(x[b].T)
        x8, x8r = split8(xT, AX)
        in_maps.append(dict(
            x8=x8, x8r=x8r,
            W_dq8=wdq8, W_dq8r=wdq8r,
            W_dkv8=wdkv8, W_dkv8r=wdkv8r,
            Wuq=np.ascontiguousarray(W_uq[:, hs]).astype(np.float16),
            Wqr=np.ascontiguousarray(W_qr[:, hr]).astype(np.float16),
            Wuk=np.ascontiguousarray(W_uk[:, hs]).astype(np.float16),
            Wkr=np.ascontiguousarray(W_kr[:, hr]).astype(np.float16),
            Wuv=np.ascontiguousarray(W_uv[:, hs]).astype(np.float16),
            Wo=np.ascontiguousarray(W_o[512 * g:512 * (g + 1), :]).astype(
                np.float16),
            c4=c4, s4=s4, mask8=mask8, id8=id8, ones8=ones8,
        ))
    return in_maps


_NC_CACHE = {}


def kernel(**inputs) -> np.ndarray:
    S = np.asarray(inputs["x"]).shape[1]
    if S not in _NC_CACHE:
        _NC_CACHE[S] = build_nc(S)
    nc = _NC_CACHE[S]
    in_maps = host_prep(inputs, S)
    res = run_bass_kernel_spmd(nc, in_maps, core_ids=list(range(NCORES)))
    y = np.empty((B, S, D), np.float32)
    for b in range(B):
        acc = res.results[4 * b]["yT"].astype(np.float32)
        for g in range(1, 4):
            acc = acc + res.results[4 * b + g]["yT"].astype(np.float32)
        y[b] = acc.T
    return y


# revision 47
# speedup vs baseline: 1.8428x; 1.1613x over previous
"""MultiHeadLatentAttention (MLA) Trainium2 Bass kernel, v2.

Problem: B=2, S=2048, D=2048, H=16 heads, d_nope=128, d_rope=64, d_head=128,
q_latent=768, kv_latent=512. Causal attention, rmsnorm'd latents, half-dim RoPE.

Sharding (8 cores): core c handles batch b=c//4 and head group g=c%4 (4 heads).
The small latent down-projections are replicated within each batch group;
W_uq/W_qr/W_uk/W_kr/W_uv are column-sharded by head; W_o row-sharded; the
4 partial outputs per batch are summed on the host.

v2 precision/engine plan (metric = InstructionCostModel timeline):
  - scores matmul in fp8e4m3 with MatmulPerfMode.DoubleRow: the nope(128) and
    zero-padded rope(64) contractions are packed as the two DoubleRow k-tiles,
    so each 128x512 score tile costs 256 PE cycles instead of 1024.
    End-to-end error from quantizing qn/kn/qr/kr to fp8 measured 1.33e-2.
  - causal mask added in the same PSUM group by a fp8 DoubleRow matmul of
    60*I against a {0,-240} mask table (-14400 pre-scale -> exp()=2e-9).
  - down-projections run as a 3-term fp8 DoubleRow residual split
    (W8@x8 + W8@x8r + W8r@x8, all host-prepared, scaled so residuals stay
    in fp8 normal range) -- fp8 speed at ~0.1% error.
  - everything else (up-projections, PV, W_o) in fp16: same 1 cycle/row as
    f32r but half the DMA/SBUF and 2x DVE elementwise.
  - softmax: exp on ACT (fp16 out, scale=SCALE/(sq*sk)), denominator
    accumulated on DVE in fp16 pairs, partition_all_reduce on Pool,
    reciprocal on DVE (InstReciprocal) -- no Ln, so no act-table thrashing.
  - rmsnorm rsqrt via DVE tensor_scalar pow(-0.5) (fallback Ln/Exp).
  - W_o and latents stay resident in SBUF; output yT written fp16.
"""
import math
import os
from contextlib import ExitStack

import numpy as np
import ml_dtypes

import concourse.bass as bass
import concourse.bass_isa as bass_isa
import concourse.bacc as bacc
import concourse.mybir as mybir
import concourse.tile as tile
from concourse.bass_utils import run_bass_kernel_spmd

F32 = mybir.dt.float32
F32R = mybir.dt.float32r
F16 = mybir.dt.float16
F8 = mybir.dt.float8e4
AF = mybir.ActivationFunctionType
DR_MODE = mybir.MatmulPerfMode.DoubleRow

B, S_FULL, D = 2, 2048, 2048
H, DN, DRR, DH = 16, 128, 64, 128
QL, KVL = 768, 512
EPS = 1e-6
SCALE = 1.0 / math.sqrt(DH)
NCORES = 8
NKT = D // 128          # 16 contraction tiles over D
NKP = NKT // 2          # 8 DoubleRow pairs
NLQ = QL // 128         # 6
NLKV = KVL // 128       # 4
NDT = D // 128          # 16 output D tiles

# fp8 scaling for the residual-split down-projection: x' = x*AX, W' = W*BW
# so both the quantized tensors and their residuals stay in fp8 normal range.
AX = 32.0
BW = 256.0
PSUM_UNSCALE = 1.0 / (AX * BW)
# score operand quantization scale (qn8 = 8*qn etc.)
SQ8 = 8.0
EXP_SCALE = SCALE / (SQ8 * SQ8)
MASK_ID = 60.0          # mask matmul: 60 * (-240) * 1 plane = -14400 pre-scale
# W_o fp8 residual split: out tiles scaled x32 (folded into v), W_o x1024
SO = 32.0
BWO = 1024.0
Y_UNSCALE = 1.0 / (SO * BWO)

PHASE_MARKS = {}


def build_nc(S=S_FULL):
    assert S % 512 == 0
    n_sb = S // 512
    n_st = S // 128
    PHASE_MARKS.clear()

    nc = bacc.Bacc("TRN2", target_bir_lowering=False, debug=False,
                   num_devices=NCORES)

    x8_d = nc.dram_tensor("x8", [D, S], F8, kind="ExternalInput")
    x8r_d = nc.dram_tensor("x8r", [D, S], F8, kind="ExternalInput")
    # per-core own-block column slice of x, for the S-sharded kv down-proj
    xkv8_d = nc.dram_tensor("xkv8", [D, 512], F8, kind="ExternalInput")
    xkv8r_d = nc.dram_tensor("xkv8r", [D, 512], F8, kind="ExternalInput")
    wdq_d = nc.dram_tensor("W_dq8", [D, QL], F8, kind="ExternalInput")
    wdqr_d = nc.dram_tensor("W_dq8r", [D, QL], F8, kind="ExternalInput")
    wdkv_d = nc.dram_tensor("W_dkv8", [D, KVL], F8, kind="ExternalInput")
    wdkvr_d = nc.dram_tensor("W_dkv8r", [D, KVL], F8, kind="ExternalInput")
    wuq_d = nc.dram_tensor("Wuq", [QL, 512], F16, kind="ExternalInput")
    wqr_d = nc.dram_tensor("Wqr", [QL, 256], F16, kind="ExternalInput")
    wuk_d = nc.dram_tensor("Wuk", [KVL, 512], F16, kind="ExternalInput")
    wkr_d = nc.dram_tensor("Wkr", [KVL, 256], F16, kind="ExternalInput")
    wuv_d = nc.dram_tensor("Wuv", [KVL, 512], F16, kind="ExternalInput")
    wo_d = nc.dram_tensor("Wo8", [512, D], F8, kind="ExternalInput")
    wor_d = nc.dram_tensor("Wo8r", [512, D], F8, kind="ExternalInput")
    c4_d = nc.dram_tensor("c4", [128, S], F16, kind="ExternalInput")
    s4_d = nc.dram_tensor("s4", [128, S], F16, kind="ExternalInput")
    mask_d = nc.dram_tensor("mask8", [128, 2, 896], F8, kind="ExternalInput")
    id_d = nc.dram_tensor("id8", [128, 2, 128], F8, kind="ExternalInput")
    ones_d = nc.dram_tensor("ones8", [128, 1], F8, kind="ExternalInput")
    yT_d = nc.dram_tensor("yT", [D, S], F16, kind="ExternalOutput")
    debug = bool(int(os.environ.get("MLA_DEBUG", "0")))
    if debug:
        dbg_qlat_d = nc.dram_tensor("dbg_qlat", [128, NLQ, S], F16,
                                    kind="ExternalOutput")
        dbg_kvlat_d = nc.dram_tensor("dbg_kvlat", [128, NLKV, S], F16,
                                     kind="ExternalOutput")
        dbg_kT8_d = nc.dram_tensor("dbg_kT8", [128, 4, 2, S], F16,
                                   kind="ExternalOutput")
        dbg_v_d = nc.dram_tensor("dbg_v", [128, n_st, 512], F16,
                                 kind="ExternalOutput")

    def col3(dram_ap, p=128):
        # [R, C] dram slice -> [128, R//128, C] tiled AP
        return dram_ap.rearrange("(t p) c -> p t c", p=p)

    def rope_pair(nc, pool, outs, ps, c4s, s4s):
        """Half-dim rope on a 2-head pair tile [128, 512] in PSUM.

        out = ps * c4 + shuf(ps) * s4, shuf swaps 32-blocks within each 64.
        Stages through fp16 SBUF so the DVE muls run in 2x mode. `outs` is a
        list of (out_ap, pslice) fp8 destinations.
        """
        rs = pool.tile([128, 512], F16, tag="rope_rs")
        nc.scalar.copy(rs[:], ps[:])
        shuf = pool.tile([128, 512], F16, tag="rope_shuf")
        nc.vector.tensor_copy(shuf[0:32, :], rs[32:64, :])
        nc.vector.tensor_copy(shuf[32:64, :], rs[0:32, :])
        nc.vector.tensor_copy(shuf[64:96, :], rs[96:128, :])
        nc.vector.tensor_copy(shuf[96:128, :], rs[64:96, :])
        t1 = pool.tile([128, 512], F16, tag="rope_t1")
        nc.vector.tensor_mul(t1[:], rs[:], c4s)
        nc.vector.tensor_mul(shuf[:], shuf[:], s4s)
        for out_ap, psl in outs:
            nc.vector.tensor_add(out_ap, t1[psl, :], shuf[psl, :])

    with tile.TileContext(nc) as tc:
        with (
            tc.tile_pool(name="const", bufs=1) as constp,
            tc.tile_pool(name="ps_mm", bufs=2, space="PSUM") as ps_mm,
            tc.tile_pool(name="ps_o", bufs=2, space="PSUM") as ps_op,
        ):
            def alt_ps(i):
                if i % 2 == 0:
                    return ps_mm.tile([128, 512], F32, tag="mm", name="ps")
                return ps_op.tile([128, 512], F32, tag="pv", name="ps")

            mask_t = constp.tile([128, 2, 896], F8)
            id_t = constp.tile([128, 2, 128], F8)
            ones_t = constp.tile([128, 1], F8)
            ones_row = constp.tile([1, 128], F16)
            nc.vector.memset(ones_row[:], 1.0)

            # persistent SBUF state
            persist_stack = ExitStack()
            persist = persist_stack.enter_context(
                tc.tile_pool(name="persist", bufs=1))
            # kT8: per head (rope_padded, nope) planes, fp8 stationary
            kT8_t = persist.tile([128, 4, 2, S], F8)
            v_t = persist.tile([128, n_st, 512], F16)
            wo_t = persist.tile([128, 4, D], F8)
            wor_t = persist.tile([128, 4, D], F8)
            qlat_t = persist.tile([128, NLQ, S], F16)
            kvlat_t = persist.tile([128, NLKV, S], F16)

            # ---------------- P0: down-projections + rmsnorm ----------------
            PHASE_MARKS["P0"] = nc.next_id()
            p0_stack = ExitStack()
            p0w = p0_stack.enter_context(tc.tile_pool(name="p0w", bufs=1))
            wdq_t = p0w.tile([128, NKT, QL], F8)
            wdqr_t = p0w.tile([128, NKT, QL], F8)
            wdkv_t = p0w.tile([128, NKT, KVL], F8)
            wdkvr_t = p0w.tile([128, NKT, KVL], F8)
            with (
                tc.tile_pool(name="p0x", bufs=2) as p0x,
                tc.tile_pool(name="p0raw", bufs=2) as p0raw,
                tc.tile_pool(name="p0sq", bufs=2) as p0sq,
                tc.tile_pool(name="p0own", bufs=1) as p0own,
                tc.tile_pool(name="p0tmp", bufs=2) as p0tmp,
                tc.tile_pool(name="p0dram", bufs=1, space="DRAM") as p0dram,
                tc.tile_pool(name="ps_den", bufs=1, space="PSUM") as ps_denp,
                tc.tile_pool(name="ps_p0", bufs=2, space="PSUM") as ps_p0,
            ):
                def alt3_ps(i):
                    if i % 3 == 2:
                        return ps_p0.tile([128, 512], F32, tag="p0", name="ps")
                    return alt_ps(i % 3)

                # zero the pad halves of the rope planes once (rope data for
                # even heads lives at partitions 0:64, odd heads at 64:128)
                for h in range(4):
                    lo = 0 if h % 2 else 64
                    nc.vector.memset(kT8_t[lo:lo + 64, h, 0, :], 0.0)

                def down_proj(latname, w_t, wr_t, nl, xh, xrh, dest,
                              rawp=None, sqp=None):
                    raw = (rawp or p0raw).tile(
                        [128, nl, 512], F16, tag=f"raw{latname}",
                        name=f"raw{latname}")
                    sq = (sqp or p0sq).tile(
                        [128, nl, 512], F8, tag=f"sq{latname}",
                        name=f"sq{latname}")
                    ps_ss = ps_denp.tile([1, 512], F32, tag="den")
                    for lt in range(nl):
                        ps = alt3_ps(lt)
                        lsl = slice(128 * lt, 128 * (lt + 1))
                        for term_w, term_x in (
                            (w_t, xh), (wr_t, xh), (w_t, xrh),
                        ):
                            first = term_w is w_t and term_x is xh
                            last = term_x is xrh
                            for kp in range(NKP):
                                nc.tensor.matmul(
                                    ps[:],
                                    term_w[:, 2 * kp:2 * kp + 2, lsl],
                                    term_x[:, 2 * kp:2 * kp + 2, :],
                                    start=(first and kp == 0),
                                    stop=(last and kp == NKP - 1),
                                    perf_mode=DR_MODE,
                                )
                        nc.scalar.activation(
                            raw[:, lt, :], ps[:], AF.Copy, scale=PSUM_UNSCALE)
                        nc.scalar.activation(
                            sq[:, lt, :], ps[:], AF.Square, scale=PSUM_UNSCALE)
                        nc.tensor.matmul(
                            ps_ss[:], ones_t[:], sq[:, lt, :],
                            start=(lt == 0), stop=(lt == nl - 1),
                        )
                    # rsqrt of mean square: sqrt(1/m) via DVE reciprocal
                    # + ACT Sqrt (Copy/Square live in the sqrt act table,
                    # so P0 needs no act-table reloads)
                    mrow = p0tmp.tile([1, 512], F32, tag="mrow")
                    nc.vector.tensor_scalar(
                        mrow[:], ps_ss[:], 1.0 / (128 * nl), EPS,
                        mybir.AluOpType.mult, mybir.AluOpType.add)
                    rrec = p0tmp.tile([1, 512], F32, tag="rrec")
                    nc.vector.reciprocal(rrec[:], mrow[:])
                    rrow = p0tmp.tile([1, 512], F16, tag="rrow")
                    nc.scalar.activation(rrow[:], rrec[:], AF.Sqrt)
                    # broadcast across partitions via a PE outer product so P0
                    # keeps the Pool queue empty (the AllGather blocks it)
                    ps_bc = ps_denp.tile([128, 512], F32, tag="bc")
                    nc.tensor.matmul(ps_bc[:], ones_row[:], rrow[:],
                                     start=True, stop=True)
                    rsb = p0tmp.tile([128, 512], F16, tag="rsb")
                    nc.scalar.copy(rsb[:], ps_bc[:])
                    for lt in range(nl):
                        nc.vector.tensor_mul(dest(lt), raw[:, lt, :], rsb[:])

                # --- kv down-proj for this core's own block only; the other
                # blocks arrive via an AllGather of the fp16 latents that
                # overlaps with the (replicated) q down-projection.
                xkvh = p0x.tile([128, NKT, 512], F8, tag="x8", name="xkv8")
                xkvrh = p0x.tile([128, NKT, 512], F8, tag="x8r", name="xkv8r")
                nc.sync.dma_start(xkvh[:], col3(xkv8_d))
                nc.sync.dma_start(ones_t[:], ones_d[:])
                nc.sync.dma_start(wdkv_t[:], col3(wdkv_d[:]))
                nc.sync.dma_start(wdkvr_t[:], col3(wdkvr_d[:]))
                nc.sync.dma_start(xkvrh[:], col3(xkv8r_d))
                kvlat_own = p0own.tile([128, NLKV, 512], F16, tag="kvown",
                                       name="kvlat_own")
                down_proj("kv", wdkv_t, wdkvr_t, NLKV, xkvh, xkvrh,
                          lambda lt: kvlat_own[:, lt, :],
                          rawp=p0own, sqp=p0own)
                # the whole collective path lives on the (otherwise idle)
                # Pool queue: its in-order waits must not block the SP/ACT
                # DMA queues or the ACT compute stream
                kv_own_d = p0dram.tile([KVL, 512], F16, name="kv_own")
                kv_all_d = p0dram.tile([4 * KVL, 512], F16, name="kv_all")
                nc.gpsimd.dma_start(col3(kv_own_d[:]), kvlat_own[:])
                nc.gpsimd.collective_compute(
                    "AllGather",
                    mybir.AluOpType.bypass,
                    replica_groups=[[0, 1, 2, 3], [4, 5, 6, 7]],
                    ins=[kv_own_d[:]],
                    outs=[kv_all_d[:]],
                )
                for c in range(4):
                    nc.gpsimd.dma_start(
                        kvlat_t[:, :, 512 * c:512 * (c + 1)],
                        col3(kv_all_d[c * KVL:(c + 1) * KVL, :]))

                # --- replicated q down-projection over all blocks
                for sb in range(n_sb):
                    cs = slice(512 * sb, 512 * (sb + 1))
                    xh = p0x.tile([128, NKT, 512], F8, tag="x8", name=f"x8_{sb}")
                    xrh = p0x.tile([128, NKT, 512], F8, tag="x8r",
                                   name=f"x8r_{sb}")
                    nc.sync.dma_start(xh[:], col3(x8_d[:, cs]))
                    if sb == 0:
                        nc.sync.dma_start(wdq_t[:], col3(wdq_d[:]))
                        nc.sync.dma_start(wdqr_t[:], col3(wdqr_d[:]))
                    nc.sync.dma_start(xrh[:], col3(x8r_d[:, cs]))
                    if sb == 0:
                        nc.sync.dma_start(mask_t[:], mask_d[:])
                        nc.sync.dma_start(id_t[:], id_d[:])
                    if sb == 2:
                        # W_o resident load (needed only from P2)
                        nc.sync.dma_start(wo_t[:], col3(wo_d[:]))
                        nc.sync.dma_start(wor_t[:], col3(wor_d[:]))
                    down_proj("q", wdq_t, wdqr_t, NLQ, xh, xrh,
                              lambda lt, cs=cs: qlat_t[:, lt, cs])
            p0_stack.close()

            # ---------------- P1: k/v up-projections ----------------
            PHASE_MARKS["P1"] = nc.next_id()
            p2w_stack = ExitStack()
            p2w = p2w_stack.enter_context(tc.tile_pool(name="p2w", bufs=1))
            p2q = p2w_stack.enter_context(tc.tile_pool(name="p2q", bufs=2))
            p2tmp = p2w_stack.enter_context(tc.tile_pool(name="p2tmp", bufs=2))
            wuq_t = p2w.tile([128, NLQ, 512], F16)
            wqr_t = p2w.tile([128, NLQ, 256], F16)

            def compute_q8(qb):
                """q up-projection + rope for one q-block into a fp8 moving
                tile with slots (qr01, qn0, qn1, qr23, qn2, qn3)."""
                cs = slice(512 * qb, 512 * (qb + 1))
                c4s = p2tmp.tile([128, 512], F16, tag="c4")
                s4s = p2tmp.tile([128, 512], F16, tag="s4")
                nc.sync.dma_start(c4s[:], c4_d[:, cs])
                nc.sync.dma_start(s4s[:], s4_d[:, cs])
                q8 = p2q.tile([128, 6, 512], F8, tag="q8", name=f"q8_{qb}")
                for pr in range(2):
                    ps = alt_ps(pr)
                    for ql in range(NLQ):
                        nc.tensor.matmul(
                            ps[:], wqr_t[:, ql, 128 * pr:128 * (pr + 1)],
                            qlat_t[:, ql, cs],
                            start=(ql == 0), stop=(ql == NLQ - 1),
                        )
                    rope_pair(nc, p2tmp,
                              [(q8[:, 3 * pr, :], slice(0, 128))],
                              ps, c4s[:], s4s[:])
                for h in range(4):
                    ps = alt_ps(h)
                    for ql in range(NLQ):
                        nc.tensor.matmul(
                            ps[:], wuq_t[:, ql, 128 * h:128 * (h + 1)],
                            qlat_t[:, ql, cs],
                            start=(ql == 0), stop=(ql == NLQ - 1),
                        )
                    slot = (1, 2, 4, 5)[h]
                    nc.scalar.activation(
                        q8[:, slot, :], ps[:], AF.Copy, scale=SQ8)
                return q8

            with (
                tc.tile_pool(name="p1w", bufs=1) as p1w,
                tc.tile_pool(name="p1tmp", bufs=2) as p1tmp,
            ):
                wuk_t = p1w.tile([128, NLKV, 512], F16)
                wkr_t = p1w.tile([128, NLKV, 256], F16)
                wuv_t = p1w.tile([128, NLKV, 512], F16)
                nc.sync.dma_start(wuk_t[:], col3(wuk_d[:]))
                nc.sync.dma_start(wuq_t[:], col3(wuq_d[:]))
                nc.sync.dma_start(wqr_t[:], col3(wqr_d[:]))
                # q8 for block 0 first: its inputs are ready before the
                # AllGathered kv latents land, filling the P1 entry stall
                q8_0 = compute_q8(0)
                for sb in range(n_sb):
                    cs = slice(512 * sb, 512 * (sb + 1))
                    if sb == 0:
                        nc.sync.dma_start(wkr_t[:], col3(wkr_d[:]))
                        nc.sync.dma_start(wuv_t[:], col3(wuv_d[:]))
                    c4s = p1tmp.tile([128, 512], F16, tag="c4")
                    s4s = p1tmp.tile([128, 512], F16, tag="s4")
                    nc.sync.dma_start(c4s[:], c4_d[:, cs])
                    nc.sync.dma_start(s4s[:], s4_d[:, cs])
                    for h in range(4):
                        ps = alt_ps(h)
                        for kl in range(NLKV):
                            nc.tensor.matmul(
                                ps[:], wuk_t[:, kl, 128 * h:128 * (h + 1)],
                                kvlat_t[:, kl, cs],
                                start=(kl == 0), stop=(kl == NLKV - 1),
                            )
                        nc.scalar.activation(
                            kT8_t[:, h, 1, cs], ps[:], AF.Copy, scale=SQ8)
                    for pr in range(2):
                        ps = alt_ps(pr)
                        for kl in range(NLKV):
                            nc.tensor.matmul(
                                ps[:], wkr_t[:, kl, 128 * pr:128 * (pr + 1)],
                                kvlat_t[:, kl, cs],
                                start=(kl == 0), stop=(kl == NLKV - 1),
                            )
                        he, ho = 2 * pr, 2 * pr + 1
                        rope_pair(
                            nc, p1tmp, [
                                (kT8_t[0:64, he, 0, cs], slice(0, 64)),
                                (kT8_t[64:128, ho, 0, cs], slice(64, 128)),
                            ], ps, c4s[:], s4s[:])
                    for stl in range(4):
                        st = 4 * sb + stl
                        ps = alt_ps(stl)
                        for kl in range(NLKV):
                            nc.tensor.matmul(
                                ps[:],
                                kvlat_t[:, kl, 512 * sb + 128 * stl:
                                        512 * sb + 128 * (stl + 1)],
                                wuv_t[:, kl, :],
                                start=(kl == 0), stop=(kl == NLKV - 1),
                            )
                        # x SO so the fp8 split of attention outputs uses
                        # fp8 normal range (unscaled at the yT stage)
                        nc.scalar.activation(v_t[:, st, :], ps[:], AF.Copy,
                                             scale=SO)

            if debug:
                nc.sync.dma_start(dbg_qlat_d[:], qlat_t[:])
                nc.sync.dma_start(dbg_kvlat_d[:], kvlat_t[:])
                nc.sync.dma_start(dbg_v_d[:], v_t[:])
                nc.gpsimd.dma_start(dbg_kT8_d[:], kT8_t[:])

            # ---------------- P2: attention + W_o ----------------
            PHASE_MARKS["P2"] = nc.next_id()
            with (
                tc.tile_pool(name="p2exp", bufs=3) as p2exp,
                tc.tile_pool(name="ps_s", bufs=2, space="PSUM") as ps_sp,
                tc.tile_pool(name="p2acc", bufs=2) as p2acc,
                tc.tile_pool(name="p2acc1", bufs=2) as p2acc1,
                tc.tile_pool(name="p2out", bufs=2) as p2out,
                tc.tile_pool(name="p2y", bufs=4) as p2y,
            ):
                def emit_wo(outs, cs, dts=range(NDT), alt=False):
                    o8, o8r = outs
                    for dt in dts:
                        dsl = slice(128 * dt, 128 * (dt + 1))
                        # the final (non-interleaved) call alternates PSUM
                        # pools for 4-bank pipelining against the ystage drain
                        ps_y = alt_ps(dt if alt else 0)
                        for j in (0, 1):
                            hp = slice(2 * j, 2 * j + 2)
                            for ti, (w_s, o_s) in enumerate(
                                ((wo_t, o8), (wor_t, o8), (wo_t, o8r))
                            ):
                                nc.tensor.matmul(
                                    ps_y[:], w_s[:, hp, dsl], o_s[:, hp, :],
                                    start=(j == 0 and ti == 0),
                                    stop=(j == 1 and ti == 2),
                                    perf_mode=DR_MODE,
                                )
                        ystage = p2y.tile([128, 512], F16, tag="y")
                        nc.vector.tensor_scalar(
                            ystage[:], ps_y[:], Y_UNSCALE, None,
                            mybir.AluOpType.mult)
                        nc.sync.dma_start(yT_d[dsl, cs], ystage[:])

                prev_out = None
                prev_cs = None
                for qb in range(n_sb):
                    cs = slice(512 * qb, 512 * (qb + 1))
                    q8 = q8_0 if qb == 0 else compute_q8(qb)

                    def q8_mov(h):
                        base = 3 * (h // 2)
                        if h % 2 == 0:
                            return q8[:, base:base + 2, :]
                        return q8[:, base:base + 3:2, :]

                    o8 = p2out.tile([128, 4, 512], F8, tag="o8",
                                    name=f"o8_{qb}")
                    o8r = p2out.tile([128, 4, 512], F8, tag="o8r",
                                     name=f"o8r_{qb}")
                    for h in range(4):
                        nkt = 4 * (qb + 1)
                        npair = nkt // 2
                        ps_o = ps_op.tile([128, 512], F32, tag="pv")
                        dacc = p2acc.tile([128, 1024], F16, tag="dacc")

                        def emit_pv(exp_pair, pk, npair, ps_o=ps_o, h=h):
                            for j in (0, 1):
                                kt = 2 * pk + j
                                nc.tensor.matmul(
                                    ps_o[:],
                                    v_t[:, kt, 128 * h:128 * (h + 1)],
                                    exp_pair[:, 512 * j:512 * (j + 1)],
                                    start=(kt == 0), stop=(kt == 2 * npair - 1),
                                )

                        pend = []   # (exp pair tile, pk) one pair behind
                        for pk in range(npair):
                            ps_s = ps_sp.tile([128, 1024], F32, tag="scores")
                            for j in (0, 1):
                                kt = 2 * pk + j
                                ks = slice(128 * kt, 128 * (kt + 1))
                                delta = 128 * kt - 512 * qb
                                diag = delta >= 0
                                half = ps_s[:, 512 * j:512 * (j + 1)]
                                nc.tensor.matmul(
                                    half, kT8_t[:, h, :, ks], q8_mov(h),
                                    start=True, stop=not diag,
                                    perf_mode=DR_MODE,
                                )
                                if diag:
                                    nc.tensor.matmul(
                                        half, id_t[:],
                                        mask_t[:, :, 384 - delta:896 - delta],
                                        start=False, stop=True,
                                        perf_mode=DR_MODE,
                                    )
                            exp_t = p2exp.tile([128, 1024], F16, tag="exp")
                            nc.scalar.activation(
                                exp_t[:], ps_s[:], AF.Exp, scale=EXP_SCALE)
                            if pk == 0:
                                nc.vector.tensor_copy(dacc[:], exp_t[:])
                            else:
                                nc.vector.tensor_add(dacc[:], dacc[:], exp_t[:])
                            pend.append((exp_t, pk))
                            if len(pend) > 1:
                                emit_pv(*pend.pop(0), npair)
                        for e in pend:
                            emit_pv(*e, npair)
                        dfold = p2acc1.tile([128, 512], F16, tag="dfold")
                        nc.vector.tensor_add(
                            dfold[:], dacc[:, 0:512], dacc[:, 512:1024])
                        dred = p2acc1.tile([128, 512], F32, tag="dred")
                        nc.gpsimd.partition_all_reduce(
                            dred[:], dfold[:], 128, bass_isa.ReduceOp.add)
                        rsb = p2tmp.tile([128, 512], F32, tag="rsbd")
                        nc.vector.reciprocal(rsb[:], dred[:])
                        o16 = p2tmp.tile([128, 512], F16, tag="o16")
                        nc.vector.tensor_mul(o16[:], ps_o[:], rsb[:])
                        nc.vector.tensor_copy(o8[:, h, :], o16[:])
                        nc.vector.tensor_sub(o8r[:, h, :], o16[:], o8[:, h, :])
                        # interleave W_o d-tiles of the previous q-block so
                        # the in-order PE stream has fill work during this
                        # block's exp-latency stalls
                        if prev_out is not None:
                            emit_wo(prev_out, prev_cs,
                                    range(4 * h, 4 * (h + 1)))
                    prev_out, prev_cs = (o8, o8r), cs
                emit_wo(prev_out, prev_cs, alt=True)
            p2w_stack.close()
            persist_stack.close()

    nc.compile()
    return nc


def host_prep(inputs, S=S_FULL):
    """Build the 8 per-core input maps from the full problem inputs."""
    FP8 = ml_dtypes.float8_e4m3

    def to8(a):
        return np.ascontiguousarray(a).astype(FP8)

    def split8(a, scale):
        hi = (a * scale).astype(FP8)
        lo = (a * scale - hi.astype(np.float32)).astype(FP8)
        return hi, lo

    x = np.asarray(inputs["x"], np.float32)
    cosT = np.asarray(inputs["rope_cos"], np.float32).T
    sinT = np.asarray(inputs["rope_sin"], np.float32).T
    c4 = np.concatenate([cosT, cosT, cosT, cosT], 0) * SQ8
    s4 = np.concatenate([-sinT, sinT, -sinT, sinT], 0) * SQ8
    c4 = np.ascontiguousarray(c4).astype(np.float16)
    s4 = np.ascontiguousarray(s4).astype(np.float16)
    qw = np.asarray(inputs["q_norm_w"], np.float32)
    kvw = np.asarray(inputs["kv_norm_w"], np.float32)
    W_uq = np.asarray(inputs["W_uq"], np.float32) * qw[:, None]
    W_qr = np.asarray(inputs["W_qr"], np.float32) * qw[:, None]
    W_uk = np.asarray(inputs["W_uk"], np.float32) * kvw[:, None]
    W_kr = np.asarray(inputs["W_kr"], np.float32) * kvw[:, None]
    W_uv = np.asarray(inputs["W_uv"], np.float32) * kvw[:, None]
    W_o = np.asarray(inputs["W_o"], np.float32)
    W_dq = np.asarray(inputs["W_dq"], np.float32)
    W_dkv = np.asarray(inputs["W_dkv"], np.float32)

    wdq8, wdq8r = split8(W_dq, BW)
    wdkv8, wdkv8r = split8(W_dkv, BW)
    wo8_full, wo8r_full = split8(W_o, BWO)

    # mask table: plane 0 = {0, -240} causal pattern, plane 1 = 0
    cgrid = np.arange(896)[None, :] - 384
    igrid = np.arange(128)[:, None]
    mask8 = np.zeros((128, 2, 896), np.float32)
    mask8[:, 0, :] = np.where(cgrid >= igrid, 0.0, -240.0)
    mask8 = mask8.astype(FP8)
    id8 = np.zeros((128, 2, 128), np.float32)
    id8[:, 0, :] = MASK_ID * np.eye(128, dtype=np.float32)
    id8 = id8.astype(FP8)
    ones8 = np.ones((128, 1), np.float32).astype(FP8)

    in_maps = []
    for c in range(NCORES):
        b, g = c // 4, c % 4
        hs = slice(4 * g * DN, 4 * (g + 1) * DN)
        hr = slice(4 * g * DRR, 4 * (g + 1) * DRR)
        xT = np.ascontiguousarray(x[b].T)
        x8, x8r = split8(xT, AX)
        in_maps.append(dict(
            x8=x8, x8r=x8r,
            xkv8=np.ascontiguousarray(x8[:, 512 * g:512 * (g + 1)]),
            xkv8r=np.ascontiguousarray(x8r[:, 512 * g:512 * (g + 1)]),
            W_dq8=wdq8, W_dq8r=wdq8r,
            W_dkv8=wdkv8, W_dkv8r=wdkv8r,
            Wuq=np.ascontiguousarray(W_uq[:, hs]).astype(np.float16),
            Wqr=np.ascontiguousarray(W_qr[:, hr]).astype(np.float16),
            Wuk=np.ascontiguousarray(W_uk[:, hs]).astype(np.float16),
            Wkr=np.ascontiguousarray(W_kr[:, hr]).astype(np.float16),
            Wuv=np.ascontiguousarray(W_uv[:, hs]).astype(np.float16),
            Wo8=np.ascontiguousarray(wo8_full[512 * g:512 * (g + 1), :]),
            Wo8r=np.ascontiguousarray(wo8r_full[512 * g:512 * (g + 1), :]),
            c4=c4, s4=s4, mask8=mask8, id8=id8, ones8=ones8,
        ))
    return in_maps


_NC_CACHE = {}


def kernel(**inputs) -> np.ndarray:
    S = np.asarray(inputs["x"]).shape[1]
    if S not in _NC_CACHE:
        _NC_CACHE[S] = build_nc(S)
    nc = _NC_CACHE[S]
    in_maps = host_prep(inputs, S)
    res = run_bass_kernel_spmd(nc, in_maps, core_ids=list(range(NCORES)))
    y = np.empty((B, S, D), np.float32)
    for b in range(B):
        acc = res.results[4 * b]["yT"].astype(np.float32)
        for g in range(1, 4):
            acc = acc + res.results[4 * b + g]["yT"].astype(np.float32)
        y[b] = acc.T
    return y


# revision 59
# speedup vs baseline: 1.9637x; 1.0656x over previous
"""MultiHeadLatentAttention (MLA) Trainium2 Bass kernel, v2.

Problem: B=2, S=2048, D=2048, H=16 heads, d_nope=128, d_rope=64, d_head=128,
q_latent=768, kv_latent=512. Causal attention, rmsnorm'd latents, half-dim RoPE.

Sharding (8 cores): core c handles batch b=c//4 and head group g=c%4 (4 heads).
The small latent down-projections are replicated within each batch group;
W_uq/W_qr/W_uk/W_kr/W_uv are column-sharded by head; W_o row-sharded; the
4 partial outputs per batch are summed on the host.

v2 precision/engine plan (metric = InstructionCostModel timeline):
  - scores matmul in fp8e4m3 with MatmulPerfMode.DoubleRow: the nope(128) and
    zero-padded rope(64) contractions are packed as the two DoubleRow k-tiles,
    so each 128x512 score tile costs 256 PE cycles instead of 1024.
    End-to-end error from quantizing qn/kn/qr/kr to fp8 measured 1.33e-2.
  - causal mask added in the same PSUM group by a fp8 DoubleRow matmul of
    60*I against a {0,-240} mask table (-14400 pre-scale -> exp()=2e-9).
  - down-projections run as a 3-term fp8 DoubleRow residual split
    (W8@x8 + W8@x8r + W8r@x8, all host-prepared, scaled so residuals stay
    in fp8 normal range) -- fp8 speed at ~0.1% error.
  - everything else (up-projections, PV, W_o) in fp16: same 1 cycle/row as
    f32r but half the DMA/SBUF and 2x DVE elementwise.
  - softmax: exp on ACT (fp16 out, scale=SCALE/(sq*sk)), denominator
    accumulated on DVE in fp16 pairs, partition_all_reduce on Pool,
    reciprocal on DVE (InstReciprocal) -- no Ln, so no act-table thrashing.
  - rmsnorm rsqrt via DVE tensor_scalar pow(-0.5) (fallback Ln/Exp).
  - W_o and latents stay resident in SBUF; output yT written fp16.
"""
import math
import os
from contextlib import ExitStack

import numpy as np
import ml_dtypes

import concourse.bass as bass
import concourse.bass_isa as bass_isa
import concourse.bacc as bacc
import concourse.mybir as mybir
import concourse.tile as tile
from concourse.bass_utils import run_bass_kernel_spmd

F32 = mybir.dt.float32
F32R = mybir.dt.float32r
F16 = mybir.dt.float16
F8 = mybir.dt.float8e4
AF = mybir.ActivationFunctionType
DR_MODE = mybir.MatmulPerfMode.DoubleRow

B, S_FULL, D = 2, 2048, 2048
H, DN, DRR, DH = 16, 128, 64, 128
QL, KVL = 768, 512
EPS = 1e-6
SCALE = 1.0 / math.sqrt(DH)
NCORES = 8
NKT = D // 128          # 16 contraction tiles over D
NKP = NKT // 2          # 8 DoubleRow pairs
NLQ = QL // 128         # 6
NLKV = KVL // 128       # 4
NDT = D // 128          # 16 output D tiles

# fp8 scaling for the residual-split down-projection: x' = x*AX, W' = W*BW
# so both the quantized tensors and their residuals stay in fp8 normal range.
AX = 32.0
BW = 256.0
PSUM_UNSCALE = 1.0 / (AX * BW)
# residual-split up-projections: latents x SL (folded into the rsqrt), and
# the up-projection weights x BW
SL = 16.0
UP_UNSCALE = 1.0 / (SL * BW)
# score operand quantization scale (qn8 = 8*qn etc.)
SQ8 = 8.0
EXP_SCALE = SCALE / (SQ8 * SQ8)
MASK_ID = 60.0          # mask matmul: 60 * (-240) * 1 plane = -14400 pre-scale
# W_o fp8 residual split: out tiles scaled x32 (folded into v), W_o x1024
SO = 32.0
BWO = 1024.0
Y_UNSCALE = 1.0 / (SO * BWO)

PHASE_MARKS = {}


def build_nc(S=S_FULL):
    assert S % 512 == 0
    n_sb = S // 512
    n_st = S // 128
    PHASE_MARKS.clear()

    nc = bacc.Bacc("TRN2", target_bir_lowering=False, debug=False,
                   num_devices=NCORES)

    x8_d = nc.dram_tensor("x8", [D, S], F8, kind="ExternalInput")
    x8r_d = nc.dram_tensor("x8r", [D, S], F8, kind="ExternalInput")
    # per-core own-block column slice of x, for the S-sharded kv down-proj
    xkv8_d = nc.dram_tensor("xkv8", [D, 512], F8, kind="ExternalInput")
    xkv8r_d = nc.dram_tensor("xkv8r", [D, 512], F8, kind="ExternalInput")
    wdq_d = nc.dram_tensor("W_dq8", [D, QL], F8, kind="ExternalInput")
    wdqr_d = nc.dram_tensor("W_dq8r", [D, QL], F8, kind="ExternalInput")
    wdkv_d = nc.dram_tensor("W_dkv8", [D, KVL], F8, kind="ExternalInput")
    wdkvr_d = nc.dram_tensor("W_dkv8r", [D, KVL], F8, kind="ExternalInput")
    wuq_d = nc.dram_tensor("Wuq8", [QL, 512], F8, kind="ExternalInput")
    wuqr_d = nc.dram_tensor("Wuq8r", [QL, 512], F8, kind="ExternalInput")
    wqr_d = nc.dram_tensor("Wqr8", [QL, 256], F8, kind="ExternalInput")
    wqrr_d = nc.dram_tensor("Wqr8r", [QL, 256], F8, kind="ExternalInput")
    wuk_d = nc.dram_tensor("Wuk8", [KVL, 512], F8, kind="ExternalInput")
    wukr_d = nc.dram_tensor("Wuk8r", [KVL, 512], F8, kind="ExternalInput")
    wkr_d = nc.dram_tensor("Wkr8", [KVL, 256], F8, kind="ExternalInput")
    wkrr_d = nc.dram_tensor("Wkr8r", [KVL, 256], F8, kind="ExternalInput")
    wuv_d = nc.dram_tensor("Wuv8", [KVL, 512], F8, kind="ExternalInput")
    wuvr_d = nc.dram_tensor("Wuv8r", [KVL, 512], F8, kind="ExternalInput")
    wo_d = nc.dram_tensor("Wo8", [512, D], F8, kind="ExternalInput")
    wor_d = nc.dram_tensor("Wo8r", [512, D], F8, kind="ExternalInput")
    c4_d = nc.dram_tensor("c4", [128, S], F16, kind="ExternalInput")
    s4_d = nc.dram_tensor("s4", [128, S], F16, kind="ExternalInput")
    mask_d = nc.dram_tensor("mask8", [128, 2, 896], F8, kind="ExternalInput")
    id_d = nc.dram_tensor("id8", [128, 2, 128], F8, kind="ExternalInput")
    ones_d = nc.dram_tensor("ones8", [128, 1], F8, kind="ExternalInput")
    yT_d = nc.dram_tensor("yT", [D, S], F16, kind="ExternalOutput")
    debug = bool(int(os.environ.get("MLA_DEBUG", "0")))
    if debug:
        dbg_qlat_d = nc.dram_tensor("dbg_qlat", [128, NLQ, S], F16,
                                    kind="ExternalOutput")
        dbg_kvlat_d = nc.dram_tensor("dbg_kvlat", [128, NLKV, S], F16,
                                     kind="ExternalOutput")
        dbg_kT8_d = nc.dram_tensor("dbg_kT8", [128, 4, 2, S], F16,
                                   kind="ExternalOutput")
        dbg_v_d = nc.dram_tensor("dbg_v", [128, n_st, 512], F16,
                                 kind="ExternalOutput")

    def col3(dram_ap, p=128):
        # [R, C] dram slice -> [128, R//128, C] tiled AP
        return dram_ap.rearrange("(t p) c -> p t c", p=p)

    def rope_pair(nc, pool, outs, ps, c4s, s4s):
        """Half-dim rope on a 2-head pair tile [128, 512] in PSUM.

        out = ps * c4 + shuf(ps) * s4, shuf swaps 32-blocks within each 64.
        Stages through fp16 SBUF so the DVE muls run in 2x mode. `outs` is a
        list of (out_ap, pslice) fp8 destinations.
        """
        rs = pool.tile([128, 512], F16, tag="rope_rs")
        nc.scalar.activation(rs[:], ps[:], AF.Copy, scale=UP_UNSCALE)
        shuf = pool.tile([128, 512], F16, tag="rope_shuf")
        nc.vector.tensor_copy(shuf[0:32, :], rs[32:64, :])
        nc.vector.tensor_copy(shuf[32:64, :], rs[0:32, :])
        nc.vector.tensor_copy(shuf[64:96, :], rs[96:128, :])
        nc.vector.tensor_copy(shuf[96:128, :], rs[64:96, :])
        t1 = pool.tile([128, 512], F16, tag="rope_t1")
        nc.vector.tensor_mul(t1[:], rs[:], c4s)
        nc.vector.tensor_mul(shuf[:], shuf[:], s4s)
        for out_ap, psl in outs:
            nc.vector.tensor_add(out_ap, t1[psl, :], shuf[psl, :])

    with tile.TileContext(nc) as tc:
        with (
            tc.tile_pool(name="const", bufs=1) as constp,
            tc.tile_pool(name="ps_mm", bufs=2, space="PSUM") as ps_mm,
            tc.tile_pool(name="ps_o", bufs=2, space="PSUM") as ps_op,
        ):
            def alt_ps(i):
                if i % 2 == 0:
                    return ps_mm.tile([128, 512], F32, tag="mm", name="ps")
                return ps_op.tile([128, 512], F32, tag="pv", name="ps")

            mask_t = constp.tile([128, 2, 896], F8)
            id_t = constp.tile([128, 2, 128], F8)
            ones_t = constp.tile([128, 1], F8)
            ones_row = constp.tile([1, 128], F16)
            nc.vector.memset(ones_row[:], 1.0)

            # persistent SBUF state
            persist_stack = ExitStack()
            persist = persist_stack.enter_context(
                tc.tile_pool(name="persist", bufs=1))
            # kT8: per head (rope_padded, nope) planes, fp8 stationary
            kT8_t = persist.tile([128, 4, 2, S], F8)
            v_t = persist.tile([128, n_st, 512], F16)
            wo_t = persist.tile([128, 4, D], F8)
            wor_t = persist.tile([128, 4, D], F8)
            qlat_t = persist.tile([128, NLQ, S], F8)
            qlatr_t = persist.tile([128, NLQ, S], F8)
            kvlat_t = persist.tile([128, NLKV, S], F8)
            kvlatr_t = persist.tile([128, NLKV, S], F8)

            # ---------------- P0: down-projections + rmsnorm ----------------
            PHASE_MARKS["P0"] = nc.next_id()
            p0_stack = ExitStack()
            p0w = p0_stack.enter_context(tc.tile_pool(name="p0w", bufs=1))
            wdq_t = p0w.tile([128, NKT, QL], F8)
            wdqr_t = p0w.tile([128, NKT, QL], F8)
            wdkv_t = p0w.tile([128, NKT, KVL], F8)
            wdkvr_t = p0w.tile([128, NKT, KVL], F8)
            with (
                tc.tile_pool(name="p0x", bufs=2) as p0x,
                tc.tile_pool(name="p0raw", bufs=2) as p0raw,
                tc.tile_pool(name="p0sq", bufs=2) as p0sq,
                tc.tile_pool(name="p0own", bufs=1) as p0own,
                tc.tile_pool(name="p0tmp", bufs=2) as p0tmp,
                tc.tile_pool(name="p0dram", bufs=1, space="DRAM") as p0dram,
                tc.tile_pool(name="ps_den", bufs=1, space="PSUM") as ps_denp,
                tc.tile_pool(name="ps_p0", bufs=2, space="PSUM") as ps_p0,
            ):
                def alt3_ps(i):
                    if i % 3 == 2:
                        return ps_p0.tile([128, 512], F32, tag="p0", name="ps")
                    return alt_ps(i % 3)

                # zero the pad halves of the rope planes once (rope data for
                # even heads lives at partitions 0:64, odd heads at 64:128)
                for h in range(4):
                    lo = 0 if h % 2 else 64
                    nc.vector.memset(kT8_t[lo:lo + 64, h, 0, :], 0.0)

                def down_proj(latname, w_t, wr_t, nl, xh, xrh, dest8,
                              dest8r, rawp=None, sqp=None):
                    raw = (rawp or p0raw).tile(
                        [128, nl, 512], F16, tag=f"raw{latname}",
                        name=f"raw{latname}")
                    sq = (sqp or p0sq).tile(
                        [128, nl, 512], F8, tag=f"sq{latname}",
                        name=f"sq{latname}")
                    ps_ss = ps_denp.tile([1, 512], F32, tag="den")
                    for lt in range(nl):
                        ps = alt3_ps(lt)
                        lsl = slice(128 * lt, 128 * (lt + 1))
                        for term_w, term_x in (
                            (w_t, xh), (wr_t, xh), (w_t, xrh),
                        ):
                            first = term_w is w_t and term_x is xh
                            last = term_x is xrh
                            for kp in range(NKP):
                                nc.tensor.matmul(
                                    ps[:],
                                    term_w[:, 2 * kp:2 * kp + 2, lsl],
                                    term_x[:, 2 * kp:2 * kp + 2, :],
                                    start=(first and kp == 0),
                                    stop=(last and kp == NKP - 1),
                                    perf_mode=DR_MODE,
                                )
                        nc.scalar.activation(
                            raw[:, lt, :], ps[:], AF.Copy, scale=PSUM_UNSCALE)
                        nc.scalar.activation(
                            sq[:, lt, :], ps[:], AF.Square, scale=PSUM_UNSCALE)
                        nc.tensor.matmul(
                            ps_ss[:], ones_t[:], sq[:, lt, :],
                            start=(lt == 0), stop=(lt == nl - 1),
                        )
                    # rsqrt of mean square: sqrt(1/m) via DVE reciprocal
                    # + ACT Sqrt (Copy/Square live in the sqrt act table,
                    # so P0 needs no act-table reloads)
                    mrow = p0tmp.tile([1, 512], F32, tag="mrow")
                    nc.vector.tensor_scalar(
                        mrow[:], ps_ss[:], 1.0 / (128 * nl), EPS,
                        mybir.AluOpType.mult, mybir.AluOpType.add)
                    rrec = p0tmp.tile([1, 512], F32, tag="rrec")
                    nc.vector.reciprocal(rrec[:], mrow[:])
                    rrow = p0tmp.tile([1, 512], F16, tag="rrow")
                    # scale=SL^2 folds the latent fp8 scale into the rsqrt
                    nc.scalar.activation(rrow[:], rrec[:], AF.Sqrt,
                                         scale=SL * SL)
                    # broadcast across partitions via a PE outer product so P0
                    # keeps the Pool queue empty (the AllGather blocks it)
                    ps_bc = ps_denp.tile([128, 512], F32, tag="bc")
                    nc.tensor.matmul(ps_bc[:], ones_row[:], rrow[:],
                                     start=True, stop=True)
                    rsb = p0tmp.tile([128, 512], F16, tag="rsb")
                    nc.scalar.copy(rsb[:], ps_bc[:])
                    for lt in range(nl):
                        tmp = p0tmp.tile([128, 512], F16, tag="ntmp")
                        nc.vector.tensor_mul(tmp[:], raw[:, lt, :], rsb[:])
                        nc.vector.tensor_copy(dest8(lt), tmp[:])
                        nc.vector.tensor_sub(dest8r(lt), tmp[:], dest8(lt))

                # --- kv down-proj for this core's own block only; the other
                # blocks arrive via an AllGather of the fp16 latents that
                # overlaps with the (replicated) q down-projection.
                xkvh = p0x.tile([128, NKT, 512], F8, tag="x8", name="xkv8")
                xkvrh = p0x.tile([128, NKT, 512], F8, tag="x8r", name="xkv8r")
                nc.sync.dma_start(xkvh[:], col3(xkv8_d))
                nc.sync.dma_start(ones_t[:], ones_d[:])
                nc.sync.dma_start(wdkv_t[:], col3(wdkv_d[:]))
                nc.sync.dma_start(wdkvr_t[:], col3(wdkvr_d[:]))
                nc.sync.dma_start(xkvrh[:], col3(xkv8r_d))
                kvlat_own = p0own.tile([128, NLKV, 512], F8, tag="kvown",
                                       name="kvlat_own")
                kvlatr_own = p0own.tile([128, NLKV, 512], F8, tag="kvownr",
                                        name="kvlatr_own")
                down_proj("kv", wdkv_t, wdkvr_t, NLKV, xkvh, xkvrh,
                          lambda lt: kvlat_own[:, lt, :],
                          lambda lt: kvlatr_own[:, lt, :],
                          rawp=p0own, sqp=p0own)
                # the whole collective path lives on the (otherwise idle)
                # Pool queue: its in-order waits must not block the SP/ACT
                # DMA queues or the ACT compute stream
                kv_own_d = p0dram.tile([2 * KVL, 512], F8, name="kv_own")
                kv_all_d = p0dram.tile([8 * KVL, 512], F8, name="kv_all")
                nc.gpsimd.dma_start(col3(kv_own_d[0:KVL, :]), kvlat_own[:])
                nc.gpsimd.dma_start(col3(kv_own_d[KVL:2 * KVL, :]),
                                    kvlatr_own[:])
                nc.gpsimd.collective_compute(
                    "AllGather",
                    mybir.AluOpType.bypass,
                    replica_groups=[[0, 1, 2, 3], [4, 5, 6, 7]],
                    ins=[kv_own_d[:]],
                    outs=[kv_all_d[:]],
                )
                for c in range(4):
                    base = c * 2 * KVL
                    nc.gpsimd.dma_start(
                        kvlat_t[:, :, 512 * c:512 * (c + 1)],
                        col3(kv_all_d[base:base + KVL, :]))
                    nc.gpsimd.dma_start(
                        kvlatr_t[:, :, 512 * c:512 * (c + 1)],
                        col3(kv_all_d[base + KVL:base + 2 * KVL, :]))

                # --- replicated q down-projection over all blocks
                for sb in range(n_sb):
                    cs = slice(512 * sb, 512 * (sb + 1))
                    xh = p0x.tile([128, NKT, 512], F8, tag="x8", name=f"x8_{sb}")
                    xrh = p0x.tile([128, NKT, 512], F8, tag="x8r",
                                   name=f"x8r_{sb}")
                    nc.sync.dma_start(xh[:], col3(x8_d[:, cs]))
                    if sb == 0:
                        nc.sync.dma_start(wdq_t[:], col3(wdq_d[:]))
                        nc.sync.dma_start(wdqr_t[:], col3(wdqr_d[:]))
                    nc.sync.dma_start(xrh[:], col3(x8r_d[:, cs]))
                    if sb == 0:
                        nc.sync.dma_start(mask_t[:], mask_d[:])
                        nc.sync.dma_start(id_t[:], id_d[:])
                    if sb == 3:
                        # W_o resident load (needed only from P2, and after
                        # the last x chunks so it never delays them)
                        nc.sync.dma_start(wo_t[:], col3(wo_d[:]))
                        nc.sync.dma_start(wor_t[:], col3(wor_d[:]))
                    down_proj("q", wdq_t, wdqr_t, NLQ, xh, xrh,
                              lambda lt, cs=cs: qlat_t[:, lt, cs],
                              lambda lt, cs=cs: qlatr_t[:, lt, cs])
            p0_stack.close()

            # ---------------- P1: k/v up-projections ----------------
            PHASE_MARKS["P1"] = nc.next_id()
            p2w_stack = ExitStack()
            p2w = p2w_stack.enter_context(tc.tile_pool(name="p2w", bufs=1))
            p2q = p2w_stack.enter_context(tc.tile_pool(name="p2q", bufs=2))
            p2tmp = p2w_stack.enter_context(tc.tile_pool(name="p2tmp", bufs=2))
            wuq_t = p2w.tile([128, NLQ, 512], F8)
            wuqr_t = p2w.tile([128, NLQ, 512], F8)
            wqr_t = p2w.tile([128, NLQ, 256], F8)
            wqrr_t = p2w.tile([128, NLQ, 256], F8)

            def compute_q8(qb):
                """q up-projection + rope for one q-block into a fp8 moving
                tile with slots (qr01, qn0, qn1, qr23, qn2, qn3)."""
                cs = slice(512 * qb, 512 * (qb + 1))
                c4s = p2tmp.tile([128, 512], F16, tag="c4")
                s4s = p2tmp.tile([128, 512], F16, tag="s4")
                nc.sync.dma_start(c4s[:], c4_d[:, cs])
                nc.sync.dma_start(s4s[:], s4_d[:, cs])
                q8 = p2q.tile([128, 6, 512], F8, tag="q8", name=f"q8_{qb}")

                def up_chain(ps, w_t, wr_t, ccols, np_, lat=qlat_t,
                             latr=qlatr_t):
                    terms = ((w_t, lat), (wr_t, lat), (w_t, latr))
                    for ti, (tw, tl) in enumerate(terms):
                        for qp in range(np_):
                            nc.tensor.matmul(
                                ps[:], tw[:, 2 * qp:2 * qp + 2, ccols],
                                tl[:, 2 * qp:2 * qp + 2, cs],
                                start=(ti == 0 and qp == 0),
                                stop=(ti == 2 and qp == np_ - 1),
                                perf_mode=DR_MODE,
                            )

                for pr in range(2):
                    ps = alt_ps(pr)
                    up_chain(ps, wqr_t, wqrr_t,
                             slice(128 * pr, 128 * (pr + 1)), NLQ // 2)
                    rope_pair(nc, p2tmp,
                              [(q8[:, 3 * pr, :], slice(0, 128))],
                              ps, c4s[:], s4s[:])
                for h in range(4):
                    ps = alt_ps(h)
                    up_chain(ps, wuq_t, wuqr_t,
                             slice(128 * h, 128 * (h + 1)), NLQ // 2)
                    slot = (1, 2, 4, 5)[h]
                    nc.scalar.activation(
                        q8[:, slot, :], ps[:], AF.Copy,
                        scale=SQ8 * UP_UNSCALE)
                return q8

            with (
                tc.tile_pool(name="p1w", bufs=1) as p1w,
                tc.tile_pool(name="p1tmp", bufs=2) as p1tmp,
            ):
                wuk_t = p1w.tile([128, NLKV, 512], F8)
                wukr_t = p1w.tile([128, NLKV, 512], F8)
                wkr_t = p1w.tile([128, NLKV, 256], F8)
                wkrr_t = p1w.tile([128, NLKV, 256], F8)
                wuv_t = p1w.tile([128, NLKV, 512], F8)
                wuvr_t = p1w.tile([128, NLKV, 512], F8)
                nc.sync.dma_start(wuk_t[:], col3(wuk_d[:]))
                nc.sync.dma_start(wukr_t[:], col3(wukr_d[:]))
                nc.sync.dma_start(wuq_t[:], col3(wuq_d[:]))
                nc.sync.dma_start(wuqr_t[:], col3(wuqr_d[:]))
                nc.sync.dma_start(wqr_t[:], col3(wqr_d[:]))
                nc.sync.dma_start(wqrr_t[:], col3(wqrr_d[:]))
                # q8 for block 0 first: its inputs are ready before the
                # AllGathered kv latents land, filling the P1 entry stall
                q8_0 = compute_q8(0)
                for sb in range(n_sb):
                    cs = slice(512 * sb, 512 * (sb + 1))
                    if sb == 0:
                        nc.sync.dma_start(wkr_t[:], col3(wkr_d[:]))
                        nc.sync.dma_start(wkrr_t[:], col3(wkrr_d[:]))
                        nc.sync.dma_start(wuv_t[:], col3(wuv_d[:]))
                        nc.sync.dma_start(wuvr_t[:], col3(wuvr_d[:]))
                    c4s = p1tmp.tile([128, 512], F16, tag="c4")
                    s4s = p1tmp.tile([128, 512], F16, tag="s4")
                    nc.sync.dma_start(c4s[:], c4_d[:, cs])
                    nc.sync.dma_start(s4s[:], s4_d[:, cs])
                    def kv_chain(ps, w_t, wr_t, ccols):
                        terms = ((w_t, kvlat_t), (wr_t, kvlat_t),
                                 (w_t, kvlatr_t))
                        for ti, (tw, tl) in enumerate(terms):
                            for kp in range(NLKV // 2):
                                nc.tensor.matmul(
                                    ps[:], tw[:, 2 * kp:2 * kp + 2, ccols],
                                    tl[:, 2 * kp:2 * kp + 2, cs],
                                    start=(ti == 0 and kp == 0),
                                    stop=(ti == 2 and kp == NLKV // 2 - 1),
                                    perf_mode=DR_MODE,
                                )

                    for h in range(4):
                        ps = alt_ps(h)
                        kv_chain(ps, wuk_t, wukr_t,
                                 slice(128 * h, 128 * (h + 1)))
                        nc.scalar.activation(
                            kT8_t[:, h, 1, cs], ps[:], AF.Copy,
                            scale=SQ8 * UP_UNSCALE)
                    for pr in range(2):
                        ps = alt_ps(pr)
                        kv_chain(ps, wkr_t, wkrr_t,
                                 slice(128 * pr, 128 * (pr + 1)))
                        he, ho = 2 * pr, 2 * pr + 1
                        rope_pair(
                            nc, p1tmp, [
                                (kT8_t[0:64, he, 0, cs], slice(0, 64)),
                                (kT8_t[64:128, ho, 0, cs], slice(64, 128)),
                            ], ps, c4s[:], s4s[:])
                    for stl in range(4):
                        st = 4 * sb + stl
                        stc = slice(512 * sb + 128 * stl,
                                    512 * sb + 128 * (stl + 1))
                        ps = alt_ps(stl)
                        terms = ((kvlat_t, wuv_t), (kvlatr_t, wuv_t),
                                 (kvlat_t, wuvr_t))
                        for ti, (tl, tw) in enumerate(terms):
                            for kp in range(NLKV // 2):
                                nc.tensor.matmul(
                                    ps[:], tl[:, 2 * kp:2 * kp + 2, stc],
                                    tw[:, 2 * kp:2 * kp + 2, :],
                                    start=(ti == 0 and kp == 0),
                                    stop=(ti == 2 and kp == NLKV // 2 - 1),
                                    perf_mode=DR_MODE,
                                )
                        # x SO so the fp8 split of attention outputs uses
                        # fp8 normal range (unscaled at the yT stage)
                        nc.scalar.activation(v_t[:, st, :], ps[:], AF.Copy,
                                             scale=SO * UP_UNSCALE)

            if debug:
                nc.sync.dma_start(dbg_qlat_d[:], qlat_t[:])
                nc.sync.dma_start(dbg_kvlat_d[:], kvlat_t[:])
                nc.sync.dma_start(dbg_v_d[:], v_t[:])
                nc.gpsimd.dma_start(dbg_kT8_d[:], kT8_t[:])

            # ---------------- P2: attention + W_o ----------------
            PHASE_MARKS["P2"] = nc.next_id()
            with (
                tc.tile_pool(name="p2exp", bufs=4) as p2exp,
                tc.tile_pool(name="ps_s", bufs=2, space="PSUM") as ps_sp,
                tc.tile_pool(name="p2acc", bufs=2) as p2acc,
                tc.tile_pool(name="p2acc1", bufs=2) as p2acc1,
                tc.tile_pool(name="p2out", bufs=2) as p2out,
                tc.tile_pool(name="p2y", bufs=4) as p2y,
            ):
                def emit_wo(outs, cs, dts=range(NDT), alt=False):
                    o8, o8r = outs
                    for dt in dts:
                        dsl = slice(128 * dt, 128 * (dt + 1))
                        # the final (non-interleaved) call alternates PSUM
                        # pools for 4-bank pipelining against the ystage drain
                        ps_y = alt_ps(dt if alt else 0)
                        for j in (0, 1):
                            hp = slice(2 * j, 2 * j + 2)
                            for ti, (w_s, o_s) in enumerate(
                                ((wo_t, o8), (wor_t, o8), (wo_t, o8r))
                            ):
                                nc.tensor.matmul(
                                    ps_y[:], w_s[:, hp, dsl], o_s[:, hp, :],
                                    start=(j == 0 and ti == 0),
                                    stop=(j == 1 and ti == 2),
                                    perf_mode=DR_MODE,
                                )
                        ystage = p2y.tile([128, 512], F16, tag="y")
                        if dt % 2 == 0:
                            nc.vector.tensor_scalar(
                                ystage[:], ps_y[:], Y_UNSCALE, None,
                                mybir.AluOpType.mult)
                        else:
                            nc.scalar.activation(
                                ystage[:], ps_y[:], AF.Copy, scale=Y_UNSCALE)
                        nc.sync.dma_start(yT_d[dsl, cs], ystage[:])

                prev_out = None
                prev_cs = None
                q8_next = q8_0
                for qb in range(n_sb):
                    cs = slice(512 * qb, 512 * (qb + 1))
                    q8 = q8_next

                    def q8_mov(h):
                        base = 3 * (h // 2)
                        if h % 2 == 0:
                            return q8[:, base:base + 2, :]
                        return q8[:, base:base + 3:2, :]

                    o8 = p2out.tile([128, 4, 512], F8, tag="o8",
                                    name=f"o8_{qb}")
                    o8r = p2out.tile([128, 4, 512], F8, tag="o8r",
                                     name=f"o8r_{qb}")
                    for h in range(4):
                        nkt = 4 * (qb + 1)
                        npair = nkt // 2
                        ps_o = ps_op.tile([128, 512], F32, tag="pv")
                        dacc = p2acc.tile([128, 1024], F16, tag="dacc")

                        def emit_pv(exp_pair, pk, npair, ps_o=ps_o, h=h):
                            for j in (0, 1):
                                kt = 2 * pk + j
                                nc.tensor.matmul(
                                    ps_o[:],
                                    v_t[:, kt, 128 * h:128 * (h + 1)],
                                    exp_pair[:, 512 * j:512 * (j + 1)],
                                    start=(kt == 0), stop=(kt == 2 * npair - 1),
                                )

                        pend = []   # (exp pair tile, pk) one pair behind
                        for pk in range(npair):
                            ps_s = ps_sp.tile([128, 1024], F32, tag="scores")
                            for j in (0, 1):
                                kt = 2 * pk + j
                                ks = slice(128 * kt, 128 * (kt + 1))
                                delta = 128 * kt - 512 * qb
                                diag = delta >= 0
                                half = ps_s[:, 512 * j:512 * (j + 1)]
                                nc.tensor.matmul(
                                    half, kT8_t[:, h, :, ks], q8_mov(h),
                                    start=True, stop=not diag,
                                    perf_mode=DR_MODE,
                                )
                                if diag:
                                    nc.tensor.matmul(
                                        half, id_t[:],
                                        mask_t[:, :, 384 - delta:896 - delta],
                                        start=False, stop=True,
                                        perf_mode=DR_MODE,
                                    )
                            exp_t = p2exp.tile([128, 1024], F16, tag="exp")
                            nc.scalar.activation(
                                exp_t[:], ps_s[:], AF.Exp, scale=EXP_SCALE)
                            if pk == 0:
                                nc.vector.tensor_copy(dacc[:], exp_t[:])
                            else:
                                nc.vector.tensor_add(dacc[:], dacc[:], exp_t[:])
                            pend.append((exp_t, pk))
                            if len(pend) > 1:
                                emit_pv(*pend.pop(0), npair)
                        for e in pend:
                            emit_pv(*e, npair)
                        dfold = p2acc1.tile([128, 512], F16, tag="dfold")
                        nc.vector.tensor_add(
                            dfold[:], dacc[:, 0:512], dacc[:, 512:1024])
                        dred = p2acc1.tile([128, 512], F32, tag="dred")
                        nc.gpsimd.partition_all_reduce(
                            dred[:], dfold[:], 128, bass_isa.ReduceOp.add)
                        rsb = p2tmp.tile([128, 512], F32, tag="rsbd")
                        nc.vector.reciprocal(rsb[:], dred[:])
                        o16 = p2tmp.tile([128, 512], F16, tag="o16")
                        nc.vector.tensor_mul(o16[:], ps_o[:], rsb[:])
                        nc.vector.tensor_copy(o8[:, h, :], o16[:])
                        nc.vector.tensor_sub(o8r[:, h, :], o16[:], o8[:, h, :])
                        # interleave W_o d-tiles of the previous q-block so
                        # the in-order PE stream has fill work during this
                        # block's exp-latency stalls
                        if prev_out is not None:
                            emit_wo(prev_out, prev_cs,
                                    range(4 * h, 4 * (h + 1)))
                        # interleave the next block's q projections mid-stream
                        # instead of serializing them at the block boundary
                        if h == 1 and qb + 1 < n_sb:
                            q8_next = compute_q8(qb + 1)
                    prev_out, prev_cs = (o8, o8r), cs
                emit_wo(prev_out, prev_cs, alt=True)
            p2w_stack.close()
            persist_stack.close()

    nc.compile()
    return nc


def host_prep(inputs, S=S_FULL):
    """Build the 8 per-core input maps from the full problem inputs."""
    FP8 = ml_dtypes.float8_e4m3

    def to8(a):
        return np.ascontiguousarray(a).astype(FP8)

    def split8(a, scale):
        hi = (a * scale).astype(FP8)
        lo = (a * scale - hi.astype(np.float32)).astype(FP8)
        return hi, lo

    x = np.asarray(inputs["x"], np.float32)
    cosT = np.asarray(inputs["rope_cos"], np.float32).T
    sinT = np.asarray(inputs["rope_sin"], np.float32).T
    c4 = np.concatenate([cosT, cosT, cosT, cosT], 0) * SQ8
    s4 = np.concatenate([-sinT, sinT, -sinT, sinT], 0) * SQ8
    c4 = np.ascontiguousarray(c4).astype(np.float16)
    s4 = np.ascontiguousarray(s4).astype(np.float16)
    qw = np.asarray(inputs["q_norm_w"], np.float32)
    kvw = np.asarray(inputs["kv_norm_w"], np.float32)
    W_uq = np.asarray(inputs["W_uq"], np.float32) * qw[:, None]
    W_qr = np.asarray(inputs["W_qr"], np.float32) * qw[:, None]
    W_uk = np.asarray(inputs["W_uk"], np.float32) * kvw[:, None]
    W_kr = np.asarray(inputs["W_kr"], np.float32) * kvw[:, None]
    W_uv = np.asarray(inputs["W_uv"], np.float32) * kvw[:, None]
    W_o = np.asarray(inputs["W_o"], np.float32)
    W_dq = np.asarray(inputs["W_dq"], np.float32)
    W_dkv = np.asarray(inputs["W_dkv"], np.float32)

    wdq8, wdq8r = split8(W_dq, BW)
    wdkv8, wdkv8r = split8(W_dkv, BW)
    wo8_full, wo8r_full = split8(W_o, BWO)
    wuq8, wuq8r = split8(W_uq, BW)
    wqr8, wqr8r = split8(W_qr, BW)
    wuk8, wuk8r = split8(W_uk, BW)
    wkr8, wkr8r = split8(W_kr, BW)
    wuv8, wuv8r = split8(W_uv, BW)

    # mask table: plane 0 = {0, -240} causal pattern, plane 1 = 0
    cgrid = np.arange(896)[None, :] - 384
    igrid = np.arange(128)[:, None]
    mask8 = np.zeros((128, 2, 896), np.float32)
    mask8[:, 0, :] = np.where(cgrid >= igrid, 0.0, -240.0)
    mask8 = mask8.astype(FP8)
    id8 = np.zeros((128, 2, 128), np.float32)
    id8[:, 0, :] = MASK_ID * np.eye(128, dtype=np.float32)
    id8 = id8.astype(FP8)
    ones8 = np.ones((128, 1), np.float32).astype(FP8)

    in_maps = []
    for c in range(NCORES):
        b, g = c // 4, c % 4
        hs = slice(4 * g * DN, 4 * (g + 1) * DN)
        hr = slice(4 * g * DRR, 4 * (g + 1) * DRR)
        xT = np.ascontiguousarray(x[b].T)
        x8, x8r = split8(xT, AX)
        in_maps.append(dict(
            x8=x8, x8r=x8r,
            xkv8=np.ascontiguousarray(x8[:, 512 * g:512 * (g + 1)]),
            xkv8r=np.ascontiguousarray(x8r[:, 512 * g:512 * (g + 1)]),
            W_dq8=wdq8, W_dq8r=wdq8r,
            W_dkv8=wdkv8, W_dkv8r=wdkv8r,
            Wuq8=np.ascontiguousarray(wuq8[:, hs]),
            Wuq8r=np.ascontiguousarray(wuq8r[:, hs]),
            Wqr8=np.ascontiguousarray(wqr8[:, hr]),
            Wqr8r=np.ascontiguousarray(wqr8r[:, hr]),
            Wuk8=np.ascontiguousarray(wuk8[:, hs]),
            Wuk8r=np.ascontiguousarray(wuk8r[:, hs]),
            Wkr8=np.ascontiguousarray(wkr8[:, hr]),
            Wkr8r=np.ascontiguousarray(wkr8r[:, hr]),
            Wuv8=np.ascontiguousarray(wuv8[:, hs]),
            Wuv8r=np.ascontiguousarray(wuv8r[:, hs]),
            Wo8=np.ascontiguousarray(wo8_full[512 * g:512 * (g + 1), :]),
            Wo8r=np.ascontiguousarray(wo8r_full[512 * g:512 * (g + 1), :]),
            c4=c4, s4=s4, mask8=mask8, id8=id8, ones8=ones8,
        ))
    return in_maps


_NC_CACHE = {}


def kernel(**inputs) -> np.ndarray:
    S = np.asarray(inputs["x"]).shape[1]
    if S not in _NC_CACHE:
        _NC_CACHE[S] = build_nc(S)
    nc = _NC_CACHE[S]
    in_maps = host_prep(inputs, S)
    res = run_bass_kernel_spmd(nc, in_maps, core_ids=list(range(NCORES)))
    y = np.empty((B, S, D), np.float32)
    for b in range(B):
        acc = res.results[4 * b]["yT"].astype(np.float32)
        for g in range(1, 4):
            acc = acc + res.results[4 * b + g]["yT"].astype(np.float32)
        y[b] = acc.T
    return y


# revision 67
# speedup vs baseline: 2.0003x; 1.0186x over previous
"""MultiHeadLatentAttention (MLA) Trainium2 Bass kernel, v2.

Problem: B=2, S=2048, D=2048, H=16 heads, d_nope=128, d_rope=64, d_head=128,
q_latent=768, kv_latent=512. Causal attention, rmsnorm'd latents, half-dim RoPE.

Sharding (8 cores): core c handles batch b=c//4 and head group g=c%4 (4 heads).
The small latent down-projections are replicated within each batch group;
W_uq/W_qr/W_uk/W_kr/W_uv are column-sharded by head; W_o row-sharded; the
4 partial outputs per batch are summed on the host.

Precision/engine plan (metric = InstructionCostModel timeline; baseline
559461 ns -> 279693 ns, rel err 1.32e-2 < 2e-2):
  - scores matmul in fp8e4m3 with MatmulPerfMode.DoubleRow: the nope(128) and
    zero-padded rope(64) contractions are packed as the two DoubleRow k-tiles,
    so each 128x512 score tile costs 256 PE cycles instead of 1024.
    End-to-end error from quantizing qn/kn/qr/kr to fp8 measured 1.33e-2;
    every other fp8 stage below is a lossless-ish residual split adding <1e-3.
  - causal mask added in the same PSUM group by a fp8 DoubleRow matmul of
    60*I against a {0,-240} mask table (-14400 pre-scale -> exp()=2e-9).
  - all projections (down, up, W_o) run as 3-term fp8 DoubleRow residual
    splits (W8@x8 + W8r@x8 + W8@x8r, dropping the second-order W8r@x8r):
    fp8 PE speed (0.5 cycles/row) at fp16-like accuracy. Weights are split
    on the host; x arrives pre-split; latents are split once at the rmsnorm
    multiply; attention outputs are split on DVE before W_o.
  - kv down-projection is S-sharded: each core computes only its own 512-col
    block and the fp8 hi+lo latent pair is AllGathered (2.1MB) on the
    collective cores, overlapped with the replicated q down-projection.
    The collective's SWDGE upload/downloads live on the otherwise-empty Pool
    queue so its in-order waits block nothing.
  - PV in fp16 (exp quantization to fp8 would cost ~3% output error).
  - softmax: exp on ACT (fp16 out), denominator via two alternating fp16
    accumulators on DVE + partition_all_reduce on Pool + DVE reciprocal --
    no Ln anywhere, so a single act-table load (was 49 reloads/63us).
  - rmsnorm rsqrt = ACT Sqrt(DVE reciprocal(mean sq)); the partition
    broadcast of the per-token scale is a PE outer product (ones x row) so
    P0 keeps the Pool queue empty for the collective.
  - W_o, latents, k^T, v stay resident in SBUF; W_o(prev block) d-tiles and
    q8(next block) are interleaved into the in-order PE stream as fill work
    during exp-latency stalls; yT stores pair two d-tiles per DMA.
"""
import math
import os
from contextlib import ExitStack

import numpy as np
import ml_dtypes

import concourse.bass as bass
import concourse.bass_isa as bass_isa
import concourse.bacc as bacc
import concourse.mybir as mybir
import concourse.tile as tile
from concourse.bass_utils import run_bass_kernel_spmd

F32 = mybir.dt.float32
F32R = mybir.dt.float32r
F16 = mybir.dt.float16
F8 = mybir.dt.float8e4
AF = mybir.ActivationFunctionType
DR_MODE = mybir.MatmulPerfMode.DoubleRow

B, S_FULL, D = 2, 2048, 2048
H, DN, DRR, DH = 16, 128, 64, 128
QL, KVL = 768, 512
EPS = 1e-6
SCALE = 1.0 / math.sqrt(DH)
NCORES = 8
NKT = D // 128          # 16 contraction tiles over D
NKP = NKT // 2          # 8 DoubleRow pairs
NLQ = QL // 128         # 6
NLKV = KVL // 128       # 4
NDT = D // 128          # 16 output D tiles

# fp8 scaling for the residual-split down-projection: x' = x*AX, W' = W*BW
# so both the quantized tensors and their residuals stay in fp8 normal range.
AX = 32.0
BW = 256.0
PSUM_UNSCALE = 1.0 / (AX * BW)
# residual-split up-projections: latents x SL (folded into the rsqrt), and
# the up-projection weights x BW
SL = 16.0
UP_UNSCALE = 1.0 / (SL * BW)
# score operand quantization scale (qn8 = 8*qn etc.)
SQ8 = 8.0
EXP_SCALE = SCALE / (SQ8 * SQ8)
MASK_ID = 60.0          # mask matmul: 60 * (-240) * 1 plane = -14400 pre-scale
# W_o fp8 residual split: out tiles scaled x32 (folded into v), W_o x1024
SO = 32.0
BWO = 1024.0
Y_UNSCALE = 1.0 / (SO * BWO)

PHASE_MARKS = {}


def build_nc(S=S_FULL):
    assert S % 512 == 0
    n_sb = S // 512
    n_st = S // 128
    PHASE_MARKS.clear()

    nc = bacc.Bacc("TRN2", target_bir_lowering=False, debug=False,
                   num_devices=NCORES)

    x8_d = nc.dram_tensor("x8", [D, S], F8, kind="ExternalInput")
    x8r_d = nc.dram_tensor("x8r", [D, S], F8, kind="ExternalInput")
    # per-core own-block column slice of x, for the S-sharded kv down-proj
    xkv8_d = nc.dram_tensor("xkv8", [D, 512], F8, kind="ExternalInput")
    xkv8r_d = nc.dram_tensor("xkv8r", [D, 512], F8, kind="ExternalInput")
    wdq_d = nc.dram_tensor("W_dq8", [D, QL], F8, kind="ExternalInput")
    wdqr_d = nc.dram_tensor("W_dq8r", [D, QL], F8, kind="ExternalInput")
    wdkv_d = nc.dram_tensor("W_dkv8", [D, KVL], F8, kind="ExternalInput")
    wdkvr_d = nc.dram_tensor("W_dkv8r", [D, KVL], F8, kind="ExternalInput")
    wuq_d = nc.dram_tensor("Wuq8", [QL, 512], F8, kind="ExternalInput")
    wuqr_d = nc.dram_tensor("Wuq8r", [QL, 512], F8, kind="ExternalInput")
    wqr_d = nc.dram_tensor("Wqr8", [QL, 256], F8, kind="ExternalInput")
    wqrr_d = nc.dram_tensor("Wqr8r", [QL, 256], F8, kind="ExternalInput")
    wuk_d = nc.dram_tensor("Wuk8", [KVL, 512], F8, kind="ExternalInput")
    wukr_d = nc.dram_tensor("Wuk8r", [KVL, 512], F8, kind="ExternalInput")
    wkr_d = nc.dram_tensor("Wkr8", [KVL, 256], F8, kind="ExternalInput")
    wkrr_d = nc.dram_tensor("Wkr8r", [KVL, 256], F8, kind="ExternalInput")
    wuv_d = nc.dram_tensor("Wuv8", [KVL, 512], F8, kind="ExternalInput")
    wuvr_d = nc.dram_tensor("Wuv8r", [KVL, 512], F8, kind="ExternalInput")
    wo_d = nc.dram_tensor("Wo8", [512, D], F8, kind="ExternalInput")
    wor_d = nc.dram_tensor("Wo8r", [512, D], F8, kind="ExternalInput")
    c4_d = nc.dram_tensor("c4", [128, S], F16, kind="ExternalInput")
    s4_d = nc.dram_tensor("s4", [128, S], F16, kind="ExternalInput")
    mask_d = nc.dram_tensor("mask8", [128, 2, 896], F8, kind="ExternalInput")
    id_d = nc.dram_tensor("id8", [128, 2, 128], F8, kind="ExternalInput")
    ones_d = nc.dram_tensor("ones8", [128, 1], F8, kind="ExternalInput")
    yT_d = nc.dram_tensor("yT", [D, S], F16, kind="ExternalOutput")
    debug = bool(int(os.environ.get("MLA_DEBUG", "0")))
    if debug:
        dbg_qlat_d = nc.dram_tensor("dbg_qlat", [128, NLQ, S], F16,
                                    kind="ExternalOutput")
        dbg_kvlat_d = nc.dram_tensor("dbg_kvlat", [128, NLKV, S], F16,
                                     kind="ExternalOutput")
        dbg_kT8_d = nc.dram_tensor("dbg_kT8", [128, 4, 2, S], F16,
                                   kind="ExternalOutput")
        dbg_v_d = nc.dram_tensor("dbg_v", [128, n_st, 512], F16,
                                 kind="ExternalOutput")

    def col3(dram_ap, p=128):
        # [R, C] dram slice -> [128, R//128, C] tiled AP
        return dram_ap.rearrange("(t p) c -> p t c", p=p)

    def rope_pair(nc, pool, outs, ps, c4s, s4s):
        """Half-dim rope on a 2-head pair tile [128, 512] in PSUM.

        out = ps * c4 + shuf(ps) * s4, shuf swaps 32-blocks within each 64.
        Stages through fp16 SBUF so the DVE muls run in 2x mode. `outs` is a
        list of (out_ap, pslice) fp8 destinations.
        """
        rs = pool.tile([128, 512], F16, tag="rope_rs")
        nc.scalar.activation(rs[:], ps[:], AF.Copy, scale=UP_UNSCALE)
        shuf = pool.tile([128, 512], F16, tag="rope_shuf")
        nc.vector.tensor_copy(shuf[0:32, :], rs[32:64, :])
        nc.vector.tensor_copy(shuf[32:64, :], rs[0:32, :])
        nc.vector.tensor_copy(shuf[64:96, :], rs[96:128, :])
        nc.vector.tensor_copy(shuf[96:128, :], rs[64:96, :])
        t1 = pool.tile([128, 512], F16, tag="rope_t1")
        nc.vector.tensor_mul(t1[:], rs[:], c4s)
        nc.vector.tensor_mul(shuf[:], shuf[:], s4s)
        for out_ap, psl in outs:
            nc.vector.tensor_add(out_ap, t1[psl, :], shuf[psl, :])

    with tile.TileContext(nc) as tc:
        with (
            tc.tile_pool(name="const", bufs=1) as constp,
            tc.tile_pool(name="ps_mm", bufs=2, space="PSUM") as ps_mm,
            tc.tile_pool(name="ps_o", bufs=2, space="PSUM") as ps_op,
        ):
            def alt_ps(i):
                if i % 2 == 0:
                    return ps_mm.tile([128, 512], F32, tag="mm", name="ps")
                return ps_op.tile([128, 512], F32, tag="pv", name="ps")

            mask_t = constp.tile([128, 2, 896], F8)
            id_t = constp.tile([128, 2, 128], F8)
            ones_t = constp.tile([128, 1], F8)
            ones_row = constp.tile([1, 128], F16)
            nc.vector.memset(ones_row[:], 1.0)

            # persistent SBUF state
            persist_stack = ExitStack()
            persist = persist_stack.enter_context(
                tc.tile_pool(name="persist", bufs=1))
            # kT8: per head (rope_padded, nope) planes, fp8 stationary
            kT8_t = persist.tile([128, 4, 2, S], F8)
            v_t = persist.tile([128, n_st, 512], F16)
            wo_t = persist.tile([128, 4, D], F8)
            wor_t = persist.tile([128, 4, D], F8)
            qlat_t = persist.tile([128, NLQ, S], F8)
            qlatr_t = persist.tile([128, NLQ, S], F8)
            kvlat_t = persist.tile([128, NLKV, S], F8)
            kvlatr_t = persist.tile([128, NLKV, S], F8)

            # ---------------- P0: down-projections + rmsnorm ----------------
            PHASE_MARKS["P0"] = nc.next_id()
            p0_stack = ExitStack()
            p0w = p0_stack.enter_context(tc.tile_pool(name="p0w", bufs=1))
            wdq_t = p0w.tile([128, NKT, QL], F8)
            wdqr_t = p0w.tile([128, NKT, QL], F8)
            wdkv_t = p0w.tile([128, NKT, KVL], F8)
            wdkvr_t = p0w.tile([128, NKT, KVL], F8)
            with (
                tc.tile_pool(name="p0x", bufs=2) as p0x,
                tc.tile_pool(name="p0raw", bufs=2) as p0raw,
                tc.tile_pool(name="p0sq", bufs=2) as p0sq,
                tc.tile_pool(name="p0own", bufs=1) as p0own,
                tc.tile_pool(name="p0tmp", bufs=2) as p0tmp,
                tc.tile_pool(name="p0dram", bufs=1, space="DRAM") as p0dram,
                tc.tile_pool(name="ps_den", bufs=1, space="PSUM") as ps_denp,
                tc.tile_pool(name="ps_p0", bufs=2, space="PSUM") as ps_p0,
            ):
                def alt3_ps(i):
                    if i % 3 == 2:
                        return ps_p0.tile([128, 512], F32, tag="p0", name="ps")
                    return alt_ps(i % 3)

                # zero the pad halves of the rope planes once (rope data for
                # even heads lives at partitions 0:64, odd heads at 64:128)
                for h in range(4):
                    lo = 0 if h % 2 else 64
                    nc.vector.memset(kT8_t[lo:lo + 64, h, 0, :], 0.0)

                def down_proj(latname, w_t, wr_t, nl, xh, xrh, dest8,
                              dest8r, rawp=None, sqp=None, terms3=True):
                    raw = (rawp or p0raw).tile(
                        [128, nl, 512], F16, tag=f"raw{latname}",
                        name=f"raw{latname}")
                    sq = (sqp or p0sq).tile(
                        [128, nl, 512], F8, tag=f"sq{latname}",
                        name=f"sq{latname}")
                    ps_ss = ps_denp.tile([1, 512], F32, tag="den")
                    for lt in range(nl):
                        ps = alt3_ps(lt)
                        lsl = slice(128 * lt, 128 * (lt + 1))
                        terms = ((w_t, xh), (wr_t, xh), (w_t, xrh)) \
                            if terms3 else ((w_t, xh), (w_t, xrh))
                        for term_w, term_x in terms:
                            first = term_w is w_t and term_x is xh
                            last = term_x is xrh
                            for kp in range(NKP):
                                nc.tensor.matmul(
                                    ps[:],
                                    term_w[:, 2 * kp:2 * kp + 2, lsl],
                                    term_x[:, 2 * kp:2 * kp + 2, :],
                                    start=(first and kp == 0),
                                    stop=(last and kp == NKP - 1),
                                    perf_mode=DR_MODE,
                                )
                        nc.scalar.activation(
                            raw[:, lt, :], ps[:], AF.Copy, scale=PSUM_UNSCALE)
                        nc.scalar.activation(
                            sq[:, lt, :], ps[:], AF.Square, scale=PSUM_UNSCALE)
                        nc.tensor.matmul(
                            ps_ss[:], ones_t[:], sq[:, lt, :],
                            start=(lt == 0), stop=(lt == nl - 1),
                        )
                    # rsqrt of mean square: sqrt(1/m) via DVE reciprocal
                    # + ACT Sqrt (Copy/Square live in the sqrt act table,
                    # so P0 needs no act-table reloads)
                    mrow = p0tmp.tile([1, 512], F32, tag="mrow")
                    nc.vector.tensor_scalar(
                        mrow[:], ps_ss[:], 1.0 / (128 * nl), EPS,
                        mybir.AluOpType.mult, mybir.AluOpType.add)
                    rrec = p0tmp.tile([1, 512], F32, tag="rrec")
                    nc.vector.reciprocal(rrec[:], mrow[:])
                    rrow = p0tmp.tile([1, 512], F16, tag="rrow")
                    # scale=SL^2 folds the latent fp8 scale into the rsqrt
                    nc.scalar.activation(rrow[:], rrec[:], AF.Sqrt,
                                         scale=SL * SL)
                    # broadcast across partitions via a PE outer product so P0
                    # keeps the Pool queue empty (the AllGather blocks it)
                    ps_bc = ps_denp.tile([128, 512], F32, tag="bc")
                    nc.tensor.matmul(ps_bc[:], ones_row[:], rrow[:],
                                     start=True, stop=True)
                    rsb = p0tmp.tile([128, 512], F16, tag="rsb")
                    nc.scalar.copy(rsb[:], ps_bc[:])
                    for lt in range(nl):
                        tmp = p0tmp.tile([128, 512], F16, tag="ntmp")
                        nc.vector.tensor_mul(tmp[:], raw[:, lt, :], rsb[:])
                        nc.vector.tensor_copy(dest8(lt), tmp[:])
                        nc.vector.tensor_sub(dest8r(lt), tmp[:], dest8(lt))

                # --- kv down-proj for this core's own block only; the other
                # blocks arrive via an AllGather of the fp16 latents that
                # overlaps with the (replicated) q down-projection.
                xkvh = p0x.tile([128, NKT, 512], F8, tag="x8", name="xkv8")
                xkvrh = p0x.tile([128, NKT, 512], F8, tag="x8r", name="xkv8r")
                nc.sync.dma_start(xkvh[:], col3(xkv8_d))
                nc.sync.dma_start(ones_t[:], ones_d[:])
                nc.sync.dma_start(wdkv_t[:], col3(wdkv_d[:]))
                nc.sync.dma_start(wdkvr_t[:], col3(wdkvr_d[:]))
                nc.sync.dma_start(xkvrh[:], col3(xkv8r_d))
                kvlat_own = p0own.tile([128, NLKV, 512], F8, tag="kvown",
                                       name="kvlat_own")
                kvlatr_own = p0own.tile([128, NLKV, 512], F8, tag="kvownr",
                                        name="kvlatr_own")
                down_proj("kv", wdkv_t, wdkvr_t, NLKV, xkvh, xkvrh,
                          lambda lt: kvlat_own[:, lt, :],
                          lambda lt: kvlatr_own[:, lt, :],
                          rawp=p0own, sqp=p0own)
                # the whole collective path lives on the (otherwise idle)
                # Pool queue: its in-order waits must not block the SP/ACT
                # DMA queues or the ACT compute stream
                kv_own_d = p0dram.tile([2 * KVL, 512], F8, name="kv_own")
                kv_all_d = p0dram.tile([8 * KVL, 512], F8, name="kv_all")
                nc.gpsimd.dma_start(col3(kv_own_d[0:KVL, :]), kvlat_own[:])
                nc.gpsimd.dma_start(col3(kv_own_d[KVL:2 * KVL, :]),
                                    kvlatr_own[:])
                nc.gpsimd.collective_compute(
                    "AllGather",
                    mybir.AluOpType.bypass,
                    replica_groups=[[0, 1, 2, 3], [4, 5, 6, 7]],
                    ins=[kv_own_d[:]],
                    outs=[kv_all_d[:]],
                )
                for c in range(4):
                    base = c * 2 * KVL
                    nc.gpsimd.dma_start(
                        kvlat_t[:, :, 512 * c:512 * (c + 1)],
                        col3(kv_all_d[base:base + KVL, :]))
                    nc.gpsimd.dma_start(
                        kvlatr_t[:, :, 512 * c:512 * (c + 1)],
                        col3(kv_all_d[base + KVL:base + 2 * KVL, :]))

                # --- replicated q down-projection over all blocks
                for sb in range(n_sb):
                    cs = slice(512 * sb, 512 * (sb + 1))
                    xh = p0x.tile([128, NKT, 512], F8, tag="x8", name=f"x8_{sb}")
                    xrh = p0x.tile([128, NKT, 512], F8, tag="x8r",
                                   name=f"x8r_{sb}")
                    nc.sync.dma_start(xh[:], col3(x8_d[:, cs]))
                    if sb == 0:
                        nc.sync.dma_start(wdq_t[:], col3(wdq_d[:]))
                        nc.sync.dma_start(wdqr_t[:], col3(wdqr_d[:]))
                    nc.sync.dma_start(xrh[:], col3(x8r_d[:, cs]))
                    if sb == 0:
                        nc.sync.dma_start(mask_t[:], mask_d[:])
                        nc.sync.dma_start(id_t[:], id_d[:])
                    if sb == 3:
                        # W_o resident load (needed only from P2, and after
                        # the last x chunks so it never delays them)
                        nc.sync.dma_start(wo_t[:], col3(wo_d[:]))
                        nc.sync.dma_start(wor_t[:], col3(wor_d[:]))
                    down_proj("q", wdq_t, wdqr_t, NLQ, xh, xrh,
                              lambda lt, cs=cs: qlat_t[:, lt, cs],
                              lambda lt, cs=cs: qlatr_t[:, lt, cs])
            p0_stack.close()

            # ---------------- P1: k/v up-projections ----------------
            PHASE_MARKS["P1"] = nc.next_id()
            p2w_stack = ExitStack()
            p2w = p2w_stack.enter_context(tc.tile_pool(name="p2w", bufs=1))
            p2q = p2w_stack.enter_context(tc.tile_pool(name="p2q", bufs=2))
            p2tmp = p2w_stack.enter_context(tc.tile_pool(name="p2tmp", bufs=2))
            wuq_t = p2w.tile([128, NLQ, 512], F8)
            wuqr_t = p2w.tile([128, NLQ, 512], F8)
            wqr_t = p2w.tile([128, NLQ, 256], F8)
            wqrr_t = p2w.tile([128, NLQ, 256], F8)

            def compute_q8(qb):
                """q up-projection + rope for one q-block into a fp8 moving
                tile with slots (qr01, qn0, qn1, qr23, qn2, qn3)."""
                cs = slice(512 * qb, 512 * (qb + 1))
                c4s = p2tmp.tile([128, 512], F16, tag="c4")
                s4s = p2tmp.tile([128, 512], F16, tag="s4")
                nc.sync.dma_start(c4s[:], c4_d[:, cs])
                nc.sync.dma_start(s4s[:], s4_d[:, cs])
                q8 = p2q.tile([128, 6, 512], F8, tag="q8", name=f"q8_{qb}")

                def up_chain(ps, w_t, wr_t, ccols, np_, lat=qlat_t,
                             latr=qlatr_t):
                    terms = ((w_t, lat), (wr_t, lat), (w_t, latr))
                    for ti, (tw, tl) in enumerate(terms):
                        for qp in range(np_):
                            nc.tensor.matmul(
                                ps[:], tw[:, 2 * qp:2 * qp + 2, ccols],
                                tl[:, 2 * qp:2 * qp + 2, cs],
                                start=(ti == 0 and qp == 0),
                                stop=(ti == 2 and qp == np_ - 1),
                                perf_mode=DR_MODE,
                            )

                for pr in range(2):
                    ps = alt_ps(pr)
                    up_chain(ps, wqr_t, wqrr_t,
                             slice(128 * pr, 128 * (pr + 1)), NLQ // 2)
                    rope_pair(nc, p2tmp,
                              [(q8[:, 3 * pr, :], slice(0, 128))],
                              ps, c4s[:], s4s[:])
                for h in range(4):
                    ps = alt_ps(h)
                    up_chain(ps, wuq_t, wuqr_t,
                             slice(128 * h, 128 * (h + 1)), NLQ // 2)
                    slot = (1, 2, 4, 5)[h]
                    nc.scalar.activation(
                        q8[:, slot, :], ps[:], AF.Copy,
                        scale=SQ8 * UP_UNSCALE)
                return q8

            with (
                tc.tile_pool(name="p1w", bufs=1) as p1w,
                tc.tile_pool(name="p1tmp", bufs=2) as p1tmp,
            ):
                wuk_t = p1w.tile([128, NLKV, 512], F8)
                wukr_t = p1w.tile([128, NLKV, 512], F8)
                wkr_t = p1w.tile([128, NLKV, 256], F8)
                wkrr_t = p1w.tile([128, NLKV, 256], F8)
                wuv_t = p1w.tile([128, NLKV, 512], F8)
                wuvr_t = p1w.tile([128, NLKV, 512], F8)
                nc.sync.dma_start(wuk_t[:], col3(wuk_d[:]))
                nc.sync.dma_start(wukr_t[:], col3(wukr_d[:]))
                nc.sync.dma_start(wuq_t[:], col3(wuq_d[:]))
                nc.sync.dma_start(wuqr_t[:], col3(wuqr_d[:]))
                nc.sync.dma_start(wqr_t[:], col3(wqr_d[:]))
                nc.sync.dma_start(wqrr_t[:], col3(wqrr_d[:]))
                # q8 for block 0 first: its inputs are ready before the
                # AllGathered kv latents land, filling the P1 entry stall
                q8_0 = compute_q8(0)
                for sb in range(n_sb):
                    cs = slice(512 * sb, 512 * (sb + 1))
                    if sb == 0:
                        nc.sync.dma_start(wkr_t[:], col3(wkr_d[:]))
                        nc.sync.dma_start(wkrr_t[:], col3(wkrr_d[:]))
                        nc.sync.dma_start(wuv_t[:], col3(wuv_d[:]))
                        nc.sync.dma_start(wuvr_t[:], col3(wuvr_d[:]))
                    c4s = p1tmp.tile([128, 512], F16, tag="c4")
                    s4s = p1tmp.tile([128, 512], F16, tag="s4")
                    nc.sync.dma_start(c4s[:], c4_d[:, cs])
                    nc.sync.dma_start(s4s[:], s4_d[:, cs])
                    def kv_chain(ps, w_t, wr_t, ccols):
                        terms = ((w_t, kvlat_t), (wr_t, kvlat_t),
                                 (w_t, kvlatr_t))
                        for ti, (tw, tl) in enumerate(terms):
                            for kp in range(NLKV // 2):
                                nc.tensor.matmul(
                                    ps[:], tw[:, 2 * kp:2 * kp + 2, ccols],
                                    tl[:, 2 * kp:2 * kp + 2, cs],
                                    start=(ti == 0 and kp == 0),
                                    stop=(ti == 2 and kp == NLKV // 2 - 1),
                                    perf_mode=DR_MODE,
                                )

                    for h in range(4):
                        ps = alt_ps(h)
                        kv_chain(ps, wuk_t, wukr_t,
                                 slice(128 * h, 128 * (h + 1)))
                        nc.scalar.activation(
                            kT8_t[:, h, 1, cs], ps[:], AF.Copy,
                            scale=SQ8 * UP_UNSCALE)
                    for pr in range(2):
                        ps = alt_ps(pr)
                        kv_chain(ps, wkr_t, wkrr_t,
                                 slice(128 * pr, 128 * (pr + 1)))
                        he, ho = 2 * pr, 2 * pr + 1
                        rope_pair(
                            nc, p1tmp, [
                                (kT8_t[0:64, he, 0, cs], slice(0, 64)),
                                (kT8_t[64:128, ho, 0, cs], slice(64, 128)),
                            ], ps, c4s[:], s4s[:])
                    for stl in range(4):
                        st = 4 * sb + stl
                        stc = slice(512 * sb + 128 * stl,
                                    512 * sb + 128 * (stl + 1))
                        ps = alt_ps(stl)
                        terms = ((kvlat_t, wuv_t), (kvlatr_t, wuv_t),
                                 (kvlat_t, wuvr_t))
                        for ti, (tl, tw) in enumerate(terms):
                            for kp in range(NLKV // 2):
                                nc.tensor.matmul(
                                    ps[:], tl[:, 2 * kp:2 * kp + 2, stc],
                                    tw[:, 2 * kp:2 * kp + 2, :],
                                    start=(ti == 0 and kp == 0),
                                    stop=(ti == 2 and kp == NLKV // 2 - 1),
                                    perf_mode=DR_MODE,
                                )
                        # x SO so the fp8 split of attention outputs uses
                        # fp8 normal range (unscaled at the yT stage)
                        nc.scalar.activation(v_t[:, st, :], ps[:], AF.Copy,
                                             scale=SO * UP_UNSCALE)

            if debug:
                nc.sync.dma_start(dbg_qlat_d[:], qlat_t[:])
                nc.sync.dma_start(dbg_kvlat_d[:], kvlat_t[:])
                nc.sync.dma_start(dbg_v_d[:], v_t[:])
                nc.gpsimd.dma_start(dbg_kT8_d[:], kT8_t[:])

            # ---------------- P2: attention + W_o ----------------
            PHASE_MARKS["P2"] = nc.next_id()
            with (
                tc.tile_pool(name="p2exp", bufs=5) as p2exp,
                tc.tile_pool(name="ps_s", bufs=2, space="PSUM") as ps_sp,
                tc.tile_pool(name="p2acc", bufs=2) as p2acc,
                tc.tile_pool(name="p2acc1", bufs=2) as p2acc1,
                tc.tile_pool(name="p2out", bufs=2) as p2out,
                tc.tile_pool(name="p2y", bufs=4) as p2y,
            ):
                def emit_wo(outs, cs, dts=range(NDT), alt=False):
                    o8, o8r = outs
                    ystage = None
                    for dt in dts:
                        dsl = slice(128 * dt, 128 * (dt + 1))
                        # the final (non-interleaved) call alternates PSUM
                        # pools for 4-bank pipelining against the ystage drain
                        ps_y = alt_ps(dt if alt else 0)
                        for j in (0, 1):
                            hp = slice(2 * j, 2 * j + 2)
                            for ti, (w_s, o_s) in enumerate(
                                ((wo_t, o8), (wor_t, o8), (wo_t, o8r))
                            ):
                                nc.tensor.matmul(
                                    ps_y[:], w_s[:, hp, dsl], o_s[:, hp, :],
                                    start=(j == 0 and ti == 0),
                                    stop=(j == 1 and ti == 2),
                                    perf_mode=DR_MODE,
                                )
                        # pair two d-tiles per ystage buffer and yT store to
                        # halve the store count (the rows are DRAM-adjacent)
                        if ystage is None:
                            ystage = p2y.tile([128, 2, 512], F16, tag="y")
                        half = ystage[:, dt % 2, :]
                        if dt % 2 == 0:
                            nc.vector.tensor_scalar(
                                half, ps_y[:], Y_UNSCALE, None,
                                mybir.AluOpType.mult)
                        else:
                            nc.scalar.activation(
                                half, ps_y[:], AF.Copy, scale=Y_UNSCALE)
                            nc.sync.dma_start(
                                col3(yT_d[128 * (dt - 1):128 * (dt + 1), cs]),
                                ystage[:])
                            ystage = None

                prev_out = None
                prev_cs = None
                q8_next = q8_0
                for qb in range(n_sb):
                    cs = slice(512 * qb, 512 * (qb + 1))
                    q8 = q8_next

                    def q8_mov(h):
                        base = 3 * (h // 2)
                        if h % 2 == 0:
                            return q8[:, base:base + 2, :]
                        return q8[:, base:base + 3:2, :]

                    o8 = p2out.tile([128, 4, 512], F8, tag="o8",
                                    name=f"o8_{qb}")
                    o8r = p2out.tile([128, 4, 512], F8, tag="o8r",
                                     name=f"o8r_{qb}")
                    for h in range(4):
                        nkt = 4 * (qb + 1)
                        npair = nkt // 2
                        ps_o = ps_op.tile([128, 512], F32, tag="pv")
                        dacc = p2acc.tile([128, 1024], F16, tag="dacc")

                        def emit_pv(exp_pair, pk, npair, ps_o=ps_o, h=h):
                            for j in (0, 1):
                                kt = 2 * pk + j
                                nc.tensor.matmul(
                                    ps_o[:],
                                    v_t[:, kt, 128 * h:128 * (h + 1)],
                                    exp_pair[:, 512 * j:512 * (j + 1)],
                                    start=(kt == 0), stop=(kt == 2 * npair - 1),
                                )

                        pend = []   # (exp pair tile, pk) one pair behind
                        for pk in range(npair):
                            ps_s = ps_sp.tile([128, 1024], F32, tag="scores")
                            for j in (0, 1):
                                kt = 2 * pk + j
                                ks = slice(128 * kt, 128 * (kt + 1))
                                delta = 128 * kt - 512 * qb
                                diag = delta >= 0
                                half = ps_s[:, 512 * j:512 * (j + 1)]
                                nc.tensor.matmul(
                                    half, kT8_t[:, h, :, ks], q8_mov(h),
                                    start=True, stop=not diag,
                                    perf_mode=DR_MODE,
                                )
                                if diag:
                                    nc.tensor.matmul(
                                        half, id_t[:],
                                        mask_t[:, :, 384 - delta:896 - delta],
                                        start=False, stop=True,
                                        perf_mode=DR_MODE,
                                    )
                            exp_t = p2exp.tile([128, 1024], F16, tag="exp")
                            nc.scalar.activation(
                                exp_t[:], ps_s[:], AF.Exp, scale=EXP_SCALE)
                            # two alternating accumulators halve the serial
                            # add-chain latency on DVE
                            half = dacc[:, 512 * (pk % 2):512 * (pk % 2) + 512]
                            if pk < 2:
                                nc.vector.tensor_add(
                                    half, exp_t[:, 0:512], exp_t[:, 512:1024])
                            else:
                                nc.vector.tensor_add(
                                    half, half, exp_t[:, 0:512])
                                nc.vector.tensor_add(
                                    half, half, exp_t[:, 512:1024])
                            pend.append((exp_t, pk))
                            if len(pend) > 1:
                                emit_pv(*pend.pop(0), npair)
                        for e in pend:
                            emit_pv(*e, npair)
                        dfold = p2acc1.tile([128, 512], F16, tag="dfold")
                        if npair > 1:
                            nc.vector.tensor_add(
                                dfold[:], dacc[:, 0:512], dacc[:, 512:1024])
                        else:
                            nc.vector.tensor_copy(dfold[:], dacc[:, 0:512])
                        dred = p2acc1.tile([128, 512], F32, tag="dred")
                        nc.gpsimd.partition_all_reduce(
                            dred[:], dfold[:], 128, bass_isa.ReduceOp.add)
                        rsb = p2tmp.tile([128, 512], F32, tag="rsbd")
                        nc.vector.reciprocal(rsb[:], dred[:])
                        o16 = p2tmp.tile([128, 512], F16, tag="o16")
                        nc.vector.tensor_mul(o16[:], ps_o[:], rsb[:])
                        nc.vector.tensor_copy(o8[:, h, :], o16[:])
                        nc.vector.tensor_sub(o8r[:, h, :], o16[:], o8[:, h, :])
                        # interleave W_o d-tiles of the previous q-block so
                        # the in-order PE stream has fill work during this
                        # block's exp-latency stalls
                        if prev_out is not None:
                            emit_wo(prev_out, prev_cs,
                                    range(4 * h, 4 * (h + 1)))
                        # interleave the next block's q projections mid-stream
                        # instead of serializing them at the block boundary
                        if h == 1 and qb + 1 < n_sb:
                            q8_next = compute_q8(qb + 1)
                    prev_out, prev_cs = (o8, o8r), cs
                emit_wo(prev_out, prev_cs, alt=True)
            p2w_stack.close()
            persist_stack.close()

    nc.compile()
    return nc


def host_prep(inputs, S=S_FULL):
    """Build the 8 per-core input maps from the full problem inputs."""
    FP8 = ml_dtypes.float8_e4m3

    def to8(a):
        return np.ascontiguousarray(a).astype(FP8)

    def split8(a, scale):
        hi = (a * scale).astype(FP8)
        lo = (a * scale - hi.astype(np.float32)).astype(FP8)
        return hi, lo

    x = np.asarray(inputs["x"], np.float32)
    cosT = np.asarray(inputs["rope_cos"], np.float32).T
    sinT = np.asarray(inputs["rope_sin"], np.float32).T
    c4 = np.concatenate([cosT, cosT, cosT, cosT], 0) * SQ8
    s4 = np.concatenate([-sinT, sinT, -sinT, sinT], 0) * SQ8
    c4 = np.ascontiguousarray(c4).astype(np.float16)
    s4 = np.ascontiguousarray(s4).astype(np.float16)
    qw = np.asarray(inputs["q_norm_w"], np.float32)
    kvw = np.asarray(inputs["kv_norm_w"], np.float32)
    W_uq = np.asarray(inputs["W_uq"], np.float32) * qw[:, None]
    W_qr = np.asarray(inputs["W_qr"], np.float32) * qw[:, None]
    W_uk = np.asarray(inputs["W_uk"], np.float32) * kvw[:, None]
    W_kr = np.asarray(inputs["W_kr"], np.float32) * kvw[:, None]
    W_uv = np.asarray(inputs["W_uv"], np.float32) * kvw[:, None]
    W_o = np.asarray(inputs["W_o"], np.float32)
    W_dq = np.asarray(inputs["W_dq"], np.float32)
    W_dkv = np.asarray(inputs["W_dkv"], np.float32)

    wdq8, wdq8r = split8(W_dq, BW)
    wdkv8, wdkv8r = split8(W_dkv, BW)
    wo8_full, wo8r_full = split8(W_o, BWO)
    wuq8, wuq8r = split8(W_uq, BW)
    wqr8, wqr8r = split8(W_qr, BW)
    wuk8, wuk8r = split8(W_uk, BW)
    wkr8, wkr8r = split8(W_kr, BW)
    wuv8, wuv8r = split8(W_uv, BW)

    # mask table: plane 0 = {0, -240} causal pattern, plane 1 = 0
    cgrid = np.arange(896)[None, :] - 384
    igrid = np.arange(128)[:, None]
    mask8 = np.zeros((128, 2, 896), np.float32)
    mask8[:, 0, :] = np.where(cgrid >= igrid, 0.0, -240.0)
    mask8 = mask8.astype(FP8)
    id8 = np.zeros((128, 2, 128), np.float32)
    id8[:, 0, :] = MASK_ID * np.eye(128, dtype=np.float32)
    id8 = id8.astype(FP8)
    ones8 = np.ones((128, 1), np.float32).astype(FP8)

    in_maps = []
    for c in range(NCORES):
        b, g = c // 4, c % 4
        hs = slice(4 * g * DN, 4 * (g + 1) * DN)
        hr = slice(4 * g * DRR, 4 * (g + 1) * DRR)
        xT = np.ascontiguousarray(x[b].T)
        x8, x8r = split8(xT, AX)
        in_maps.append(dict(
            x8=x8, x8r=x8r,
            xkv8=np.ascontiguousarray(x8[:, 512 * g:512 * (g + 1)]),
            xkv8r=np.ascontiguousarray(x8r[:, 512 * g:512 * (g + 1)]),
            W_dq8=wdq8, W_dq8r=wdq8r,
            W_dkv8=wdkv8, W_dkv8r=wdkv8r,
            Wuq8=np.ascontiguousarray(wuq8[:, hs]),
            Wuq8r=np.ascontiguousarray(wuq8r[:, hs]),
            Wqr8=np.ascontiguousarray(wqr8[:, hr]),
            Wqr8r=np.ascontiguousarray(wqr8r[:, hr]),
            Wuk8=np.ascontiguousarray(wuk8[:, hs]),
            Wuk8r=np.ascontiguousarray(wuk8r[:, hs]),
            Wkr8=np.ascontiguousarray(wkr8[:, hr]),
            Wkr8r=np.ascontiguousarray(wkr8r[:, hr]),
            Wuv8=np.ascontiguousarray(wuv8[:, hs]),
            Wuv8r=np.ascontiguousarray(wuv8r[:, hs]),
            Wo8=np.ascontiguousarray(wo8_full[512 * g:512 * (g + 1), :]),
            Wo8r=np.ascontiguousarray(wo8r_full[512 * g:512 * (g + 1), :]),
            c4=c4, s4=s4, mask8=mask8, id8=id8, ones8=ones8,
        ))
    return in_maps


_NC_CACHE = {}


def kernel(**inputs) -> np.ndarray:
    S = np.asarray(inputs["x"]).shape[1]
    if S not in _NC_CACHE:
        _NC_CACHE[S] = build_nc(S)
    nc = _NC_CACHE[S]
    in_maps = host_prep(inputs, S)
    res = run_bass_kernel_spmd(nc, in_maps, core_ids=list(range(NCORES)))
    y = np.empty((B, S, D), np.float32)
    for b in range(B):
        acc = res.results[4 * b]["yT"].astype(np.float32)
        for g in range(1, 4):
            acc = acc + res.results[4 * b + g]["yT"].astype(np.float32)
        y[b] = acc.T
    return y


# revision 72
# speedup vs baseline: 2.0105x; 1.0051x over previous
"""MultiHeadLatentAttention (MLA) Trainium2 Bass kernel, v2.

Problem: B=2, S=2048, D=2048, H=16 heads, d_nope=128, d_rope=64, d_head=128,
q_latent=768, kv_latent=512. Causal attention, rmsnorm'd latents, half-dim RoPE.

Sharding (8 cores): core c handles batch b=c//4 and head group g=c%4 (4 heads).
The small latent down-projections are replicated within each batch group;
W_uq/W_qr/W_uk/W_kr/W_uv are column-sharded by head; W_o row-sharded; the
4 partial outputs per batch are summed on the host.

Precision/engine plan (metric = InstructionCostModel timeline; baseline
559461 ns -> 279693 ns, rel err 1.32e-2 < 2e-2):
  - scores matmul in fp8e4m3 with MatmulPerfMode.DoubleRow: the nope(128) and
    zero-padded rope(64) contractions are packed as the two DoubleRow k-tiles,
    so each 128x512 score tile costs 256 PE cycles instead of 1024.
    End-to-end error from quantizing qn/kn/qr/kr to fp8 measured 1.33e-2;
    every other fp8 stage below is a lossless-ish residual split adding <1e-3.
  - causal mask added in the same PSUM group by a fp8 DoubleRow matmul of
    60*I against a {0,-240} mask table (-14400 pre-scale -> exp()=2e-9).
  - all projections (down, up, W_o) run as 3-term fp8 DoubleRow residual
    splits (W8@x8 + W8r@x8 + W8@x8r, dropping the second-order W8r@x8r):
    fp8 PE speed (0.5 cycles/row) at fp16-like accuracy. Weights are split
    on the host; x arrives pre-split; latents are split once at the rmsnorm
    multiply; attention outputs are split on DVE before W_o.
  - kv down-projection is S-sharded: each core computes only its own 512-col
    block and the fp8 hi+lo latent pair is AllGathered (2.1MB) on the
    collective cores, overlapped with the replicated q down-projection.
    The collective's SWDGE upload/downloads live on the otherwise-empty Pool
    queue so its in-order waits block nothing.
  - PV in fp16 (exp quantization to fp8 would cost ~3% output error).
  - softmax: exp on ACT (fp16 out), denominator via two alternating fp16
    accumulators on DVE + partition_all_reduce on Pool + DVE reciprocal --
    no Ln anywhere, so a single act-table load (was 49 reloads/63us).
  - rmsnorm rsqrt = ACT Sqrt(DVE reciprocal(mean sq)); the partition
    broadcast of the per-token scale is a PE outer product (ones x row) so
    P0 keeps the Pool queue empty for the collective.
  - W_o, latents, k^T, v stay resident in SBUF; W_o(prev block) d-tiles and
    q8(next block) are interleaved into the in-order PE stream as fill work
    during exp-latency stalls; yT stores pair two d-tiles per DMA.
"""
import math
import os
from contextlib import ExitStack

import numpy as np
import ml_dtypes

import concourse.bass as bass
import concourse.bass_isa as bass_isa
import concourse.bacc as bacc
import concourse.mybir as mybir
import concourse.tile as tile
from concourse.bass_utils import run_bass_kernel_spmd

F32 = mybir.dt.float32
F32R = mybir.dt.float32r
F16 = mybir.dt.float16
F8 = mybir.dt.float8e4
AF = mybir.ActivationFunctionType
DR_MODE = mybir.MatmulPerfMode.DoubleRow

B, S_FULL, D = 2, 2048, 2048
H, DN, DRR, DH = 16, 128, 64, 128
QL, KVL = 768, 512
EPS = 1e-6
SCALE = 1.0 / math.sqrt(DH)
NCORES = 8
NKT = D // 128          # 16 contraction tiles over D
NKP = NKT // 2          # 8 DoubleRow pairs
NLQ = QL // 128         # 6
NLKV = KVL // 128       # 4
NDT = D // 128          # 16 output D tiles

# fp8 scaling for the residual-split down-projection: x' = x*AX, W' = W*BW
# so both the quantized tensors and their residuals stay in fp8 normal range.
AX = 32.0
BW = 256.0
PSUM_UNSCALE = 1.0 / (AX * BW)
# residual-split up-projections: latents x SL (folded into the rsqrt), and
# the up-projection weights x BW
SL = 16.0
UP_UNSCALE = 1.0 / (SL * BW)
# score operand quantization scale (qn8 = 8*qn etc.)
SQ8 = 8.0
EXP_SCALE = SCALE / (SQ8 * SQ8)
MASK_ID = 60.0          # mask matmul: 60 * (-240) * 1 plane = -14400 pre-scale
# W_o fp8 residual split: out tiles scaled x32 (folded into v), W_o x1024
SO = 32.0
BWO = 1024.0
Y_UNSCALE = 1.0 / (SO * BWO)

PHASE_MARKS = {}


def build_nc(S=S_FULL):
    assert S % 512 == 0
    n_sb = S // 512
    n_st = S // 128
    PHASE_MARKS.clear()

    nc = bacc.Bacc("TRN2", target_bir_lowering=False, debug=False,
                   num_devices=NCORES)

    x8_d = nc.dram_tensor("x8", [D, S], F8, kind="ExternalInput")
    x8r_d = nc.dram_tensor("x8r", [D, S], F8, kind="ExternalInput")
    # per-core own-block column slice of x, for the S-sharded kv down-proj
    xkv8_d = nc.dram_tensor("xkv8", [D, 512], F8, kind="ExternalInput")
    xkv8r_d = nc.dram_tensor("xkv8r", [D, 512], F8, kind="ExternalInput")
    wdq_d = nc.dram_tensor("W_dq8", [D, QL], F8, kind="ExternalInput")
    wdqr_d = nc.dram_tensor("W_dq8r", [D, QL], F8, kind="ExternalInput")
    wdkv_d = nc.dram_tensor("W_dkv8", [D, KVL], F8, kind="ExternalInput")
    wdkvr_d = nc.dram_tensor("W_dkv8r", [D, KVL], F8, kind="ExternalInput")
    wuq_d = nc.dram_tensor("Wuq8", [QL, 512], F8, kind="ExternalInput")
    wuqr_d = nc.dram_tensor("Wuq8r", [QL, 512], F8, kind="ExternalInput")
    wqr_d = nc.dram_tensor("Wqr8", [QL, 256], F8, kind="ExternalInput")
    wqrr_d = nc.dram_tensor("Wqr8r", [QL, 256], F8, kind="ExternalInput")
    wuk_d = nc.dram_tensor("Wuk8", [KVL, 512], F8, kind="ExternalInput")
    wukr_d = nc.dram_tensor("Wuk8r", [KVL, 512], F8, kind="ExternalInput")
    wkr_d = nc.dram_tensor("Wkr8", [KVL, 256], F8, kind="ExternalInput")
    wkrr_d = nc.dram_tensor("Wkr8r", [KVL, 256], F8, kind="ExternalInput")
    wuv_d = nc.dram_tensor("Wuv8", [KVL, 512], F8, kind="ExternalInput")
    wuvr_d = nc.dram_tensor("Wuv8r", [KVL, 512], F8, kind="ExternalInput")
    wo_d = nc.dram_tensor("Wo8", [512, D], F8, kind="ExternalInput")
    wor_d = nc.dram_tensor("Wo8r", [512, D], F8, kind="ExternalInput")
    c4_d = nc.dram_tensor("c4", [128, S], F16, kind="ExternalInput")
    s4_d = nc.dram_tensor("s4", [128, S], F16, kind="ExternalInput")
    mask_d = nc.dram_tensor("mask8", [128, 2, 896], F8, kind="ExternalInput")
    id_d = nc.dram_tensor("id8", [128, 2, 128], F8, kind="ExternalInput")
    ones_d = nc.dram_tensor("ones8", [128, 1], F8, kind="ExternalInput")
    yT_d = nc.dram_tensor("yT", [D, S], F16, kind="ExternalOutput")
    debug = bool(int(os.environ.get("MLA_DEBUG", "0")))
    if debug:
        dbg_qlat_d = nc.dram_tensor("dbg_qlat", [128, NLQ, S], F16,
                                    kind="ExternalOutput")
        dbg_kvlat_d = nc.dram_tensor("dbg_kvlat", [128, NLKV, S], F16,
                                     kind="ExternalOutput")
        dbg_kT8_d = nc.dram_tensor("dbg_kT8", [128, 4, 2, S], F16,
                                   kind="ExternalOutput")
        dbg_v_d = nc.dram_tensor("dbg_v", [128, n_st, 512], F16,
                                 kind="ExternalOutput")

    def col3(dram_ap, p=128):
        # [R, C] dram slice -> [128, R//128, C] tiled AP
        return dram_ap.rearrange("(t p) c -> p t c", p=p)

    def rope_pair(nc, pool, outs, ps, c4s, s4s):
        """Half-dim rope on a 2-head pair tile [128, 512] in PSUM.

        out = ps * c4 + shuf(ps) * s4, shuf swaps 32-blocks within each 64.
        Stages through fp16 SBUF so the DVE muls run in 2x mode. `outs` is a
        list of (out_ap, pslice) fp8 destinations.
        """
        rs = pool.tile([128, 512], F16, tag="rope_rs")
        nc.scalar.activation(rs[:], ps[:], AF.Copy, scale=UP_UNSCALE)
        shuf = pool.tile([128, 512], F16, tag="rope_shuf")
        nc.vector.tensor_copy(shuf[0:32, :], rs[32:64, :])
        nc.vector.tensor_copy(shuf[32:64, :], rs[0:32, :])
        nc.vector.tensor_copy(shuf[64:96, :], rs[96:128, :])
        nc.vector.tensor_copy(shuf[96:128, :], rs[64:96, :])
        t1 = pool.tile([128, 512], F16, tag="rope_t1")
        nc.vector.tensor_mul(t1[:], rs[:], c4s)
        nc.vector.tensor_mul(shuf[:], shuf[:], s4s)
        for out_ap, psl in outs:
            nc.vector.tensor_add(out_ap, t1[psl, :], shuf[psl, :])

    with tile.TileContext(nc) as tc:
        with (
            tc.tile_pool(name="const", bufs=1) as constp,
            tc.tile_pool(name="ps_mm", bufs=2, space="PSUM") as ps_mm,
            tc.tile_pool(name="ps_o", bufs=2, space="PSUM") as ps_op,
        ):
            def alt_ps(i):
                if i % 2 == 0:
                    return ps_mm.tile([128, 512], F32, tag="mm", name="ps")
                return ps_op.tile([128, 512], F32, tag="pv", name="ps")

            mask_t = constp.tile([128, 2, 896], F8)
            id_t = constp.tile([128, 2, 128], F8)
            ones_t = constp.tile([128, 1], F8)
            ones_row = constp.tile([1, 128], F16)
            nc.vector.memset(ones_row[:], 1.0)

            # persistent SBUF state
            persist_stack = ExitStack()
            persist = persist_stack.enter_context(
                tc.tile_pool(name="persist", bufs=1))
            # kT8: per head (rope_padded, nope) planes, fp8 stationary
            kT8_t = persist.tile([128, 4, 2, S], F8)
            v_t = persist.tile([128, n_st, 512], F16)
            wo_t = persist.tile([128, 4, D], F8)
            wor_t = persist.tile([128, 4, D], F8)
            qlat_t = persist.tile([128, NLQ, S], F8)
            qlatr_t = persist.tile([128, NLQ, S], F8)
            kvlat_t = persist.tile([128, NLKV, S], F8)
            kvlatr_t = persist.tile([128, NLKV, S], F8)

            # ---------------- P0: down-projections + rmsnorm ----------------
            PHASE_MARKS["P0"] = nc.next_id()
            p0_stack = ExitStack()
            p0w = p0_stack.enter_context(tc.tile_pool(name="p0w", bufs=1))
            wdq_t = p0w.tile([128, NKT, QL], F8)
            wdqr_t = p0w.tile([128, NKT, QL], F8)
            wdkv_t = p0w.tile([128, NKT, KVL], F8)
            wdkvr_t = p0w.tile([128, NKT, KVL], F8)
            with (
                tc.tile_pool(name="p0x", bufs=2) as p0x,
                tc.tile_pool(name="p0raw", bufs=2) as p0raw,
                tc.tile_pool(name="p0sq", bufs=2) as p0sq,
                tc.tile_pool(name="p0own", bufs=1) as p0own,
                tc.tile_pool(name="p0tmp", bufs=2) as p0tmp,
                tc.tile_pool(name="p0dram", bufs=1, space="DRAM") as p0dram,
                tc.tile_pool(name="ps_den", bufs=1, space="PSUM") as ps_denp,
                tc.tile_pool(name="ps_p0", bufs=2, space="PSUM") as ps_p0,
            ):
                def alt3_ps(i):
                    if i % 3 == 2:
                        return ps_p0.tile([128, 512], F32, tag="p0", name="ps")
                    return alt_ps(i % 3)

                # zero the pad halves of the rope planes once (rope data for
                # even heads lives at partitions 0:64, odd heads at 64:128)
                for h in range(4):
                    lo = 0 if h % 2 else 64
                    nc.gpsimd.memset(kT8_t[lo:lo + 64, h, 0, :], 0.0)

                def down_proj(latname, w_t, wr_t, nl, xh, xrh, dest8,
                              dest8r, rawp=None, sqp=None, terms3=True):
                    raw = (rawp or p0raw).tile(
                        [128, nl, 512], F16, tag=f"raw{latname}",
                        name=f"raw{latname}")
                    sq = (sqp or p0sq).tile(
                        [128, nl, 512], F8, tag=f"sq{latname}",
                        name=f"sq{latname}")
                    ps_ss = ps_denp.tile([1, 512], F32, tag="den")
                    for lt in range(nl):
                        ps = alt3_ps(lt)
                        lsl = slice(128 * lt, 128 * (lt + 1))
                        terms = ((w_t, xh), (wr_t, xh), (w_t, xrh)) \
                            if terms3 else ((w_t, xh), (w_t, xrh))
                        for term_w, term_x in terms:
                            first = term_w is w_t and term_x is xh
                            last = term_x is xrh
                            for kp in range(NKP):
                                nc.tensor.matmul(
                                    ps[:],
                                    term_w[:, 2 * kp:2 * kp + 2, lsl],
                                    term_x[:, 2 * kp:2 * kp + 2, :],
                                    start=(first and kp == 0),
                                    stop=(last and kp == NKP - 1),
                                    perf_mode=DR_MODE,
                                )
                        nc.scalar.activation(
                            raw[:, lt, :], ps[:], AF.Copy, scale=PSUM_UNSCALE)
                        nc.scalar.activation(
                            sq[:, lt, :], ps[:], AF.Square, scale=PSUM_UNSCALE)
                        nc.tensor.matmul(
                            ps_ss[:], ones_t[:], sq[:, lt, :],
                            start=(lt == 0), stop=(lt == nl - 1),
                        )
                    # rsqrt of mean square: sqrt(1/m) via DVE reciprocal
                    # + ACT Sqrt (Copy/Square live in the sqrt act table,
                    # so P0 needs no act-table reloads)
                    mrow = p0tmp.tile([1, 512], F32, tag="mrow")
                    nc.vector.tensor_scalar(
                        mrow[:], ps_ss[:], 1.0 / (128 * nl), EPS,
                        mybir.AluOpType.mult, mybir.AluOpType.add)
                    rrec = p0tmp.tile([1, 512], F32, tag="rrec")
                    nc.vector.reciprocal(rrec[:], mrow[:])
                    rrow = p0tmp.tile([1, 512], F16, tag="rrow")
                    # scale=SL^2 folds the latent fp8 scale into the rsqrt
                    nc.scalar.activation(rrow[:], rrec[:], AF.Sqrt,
                                         scale=SL * SL)
                    # broadcast across partitions via a PE outer product so P0
                    # keeps the Pool queue empty (the AllGather blocks it)
                    ps_bc = ps_denp.tile([128, 512], F32, tag="bc")
                    nc.tensor.matmul(ps_bc[:], ones_row[:], rrow[:],
                                     start=True, stop=True)
                    rsb = p0tmp.tile([128, 512], F16, tag="rsb")
                    nc.scalar.copy(rsb[:], ps_bc[:])
                    for lt in range(nl):
                        tmp = p0tmp.tile([128, 512], F16, tag="ntmp")
                        nc.vector.tensor_mul(tmp[:], raw[:, lt, :], rsb[:])
                        nc.vector.tensor_copy(dest8(lt), tmp[:])
                        nc.vector.tensor_sub(dest8r(lt), tmp[:], dest8(lt))

                # --- kv down-proj for this core's own block only; the other
                # blocks arrive via an AllGather of the fp16 latents that
                # overlaps with the (replicated) q down-projection.
                xkvh = p0x.tile([128, NKT, 512], F8, tag="x8", name="xkv8")
                xkvrh = p0x.tile([128, NKT, 512], F8, tag="x8r", name="xkv8r")
                nc.sync.dma_start(xkvh[:], col3(xkv8_d))
                nc.sync.dma_start(ones_t[:], ones_d[:])
                nc.sync.dma_start(wdkv_t[:], col3(wdkv_d[:]))
                nc.sync.dma_start(wdkvr_t[:], col3(wdkvr_d[:]))
                nc.sync.dma_start(xkvrh[:], col3(xkv8r_d))
                kvlat_own = p0own.tile([128, NLKV, 512], F8, tag="kvown",
                                       name="kvlat_own")
                kvlatr_own = p0own.tile([128, NLKV, 512], F8, tag="kvownr",
                                        name="kvlatr_own")
                down_proj("kv", wdkv_t, wdkvr_t, NLKV, xkvh, xkvrh,
                          lambda lt: kvlat_own[:, lt, :],
                          lambda lt: kvlatr_own[:, lt, :],
                          rawp=p0own, sqp=p0own)
                # the whole collective path lives on the (otherwise idle)
                # Pool queue: its in-order waits must not block the SP/ACT
                # DMA queues or the ACT compute stream
                kv_own_d = p0dram.tile([2 * KVL, 512], F8, name="kv_own")
                kv_all_d = p0dram.tile([8 * KVL, 512], F8, name="kv_all")
                nc.gpsimd.dma_start(col3(kv_own_d[0:KVL, :]), kvlat_own[:])
                nc.gpsimd.dma_start(col3(kv_own_d[KVL:2 * KVL, :]),
                                    kvlatr_own[:])
                nc.gpsimd.collective_compute(
                    "AllGather",
                    mybir.AluOpType.bypass,
                    replica_groups=[[0, 1, 2, 3], [4, 5, 6, 7]],
                    ins=[kv_own_d[:]],
                    outs=[kv_all_d[:]],
                )
                for c in range(4):
                    base = c * 2 * KVL
                    nc.gpsimd.dma_start(
                        kvlat_t[:, :, 512 * c:512 * (c + 1)],
                        col3(kv_all_d[base:base + KVL, :]))
                    nc.gpsimd.dma_start(
                        kvlatr_t[:, :, 512 * c:512 * (c + 1)],
                        col3(kv_all_d[base + KVL:base + 2 * KVL, :]))

                # --- replicated q down-projection over all blocks
                for sb in range(n_sb):
                    cs = slice(512 * sb, 512 * (sb + 1))
                    xh = p0x.tile([128, NKT, 512], F8, tag="x8", name=f"x8_{sb}")
                    xrh = p0x.tile([128, NKT, 512], F8, tag="x8r",
                                   name=f"x8r_{sb}")
                    nc.sync.dma_start(xh[:], col3(x8_d[:, cs]))
                    if sb == 0:
                        nc.sync.dma_start(wdq_t[:], col3(wdq_d[:]))
                        nc.sync.dma_start(wdqr_t[:], col3(wdqr_d[:]))
                    nc.sync.dma_start(xrh[:], col3(x8r_d[:, cs]))
                    if sb == 0:
                        nc.sync.dma_start(mask_t[:], mask_d[:])
                        nc.sync.dma_start(id_t[:], id_d[:])
                    if sb == 3:
                        # W_o resident load (needed only from P2, and after
                        # the last x chunks so it never delays them)
                        nc.sync.dma_start(wo_t[:], col3(wo_d[:]))
                        nc.sync.dma_start(wor_t[:], col3(wor_d[:]))
                    down_proj("q", wdq_t, wdqr_t, NLQ, xh, xrh,
                              lambda lt, cs=cs: qlat_t[:, lt, cs],
                              lambda lt, cs=cs: qlatr_t[:, lt, cs])
            p0_stack.close()

            # ---------------- P1: k/v up-projections ----------------
            PHASE_MARKS["P1"] = nc.next_id()
            p2w_stack = ExitStack()
            p2w = p2w_stack.enter_context(tc.tile_pool(name="p2w", bufs=1))
            p2q = p2w_stack.enter_context(tc.tile_pool(name="p2q", bufs=2))
            p2tmp = p2w_stack.enter_context(tc.tile_pool(name="p2tmp", bufs=2))
            wuq_t = p2w.tile([128, NLQ, 512], F8)
            wuqr_t = p2w.tile([128, NLQ, 512], F8)
            wqr_t = p2w.tile([128, NLQ, 256], F8)
            wqrr_t = p2w.tile([128, NLQ, 256], F8)

            def compute_q8(qb):
                """q up-projection + rope for one q-block into a fp8 moving
                tile with slots (qr01, qn0, qn1, qr23, qn2, qn3)."""
                cs = slice(512 * qb, 512 * (qb + 1))
                c4s = p2tmp.tile([128, 512], F16, tag="c4")
                s4s = p2tmp.tile([128, 512], F16, tag="s4")
                nc.sync.dma_start(c4s[:], c4_d[:, cs])
                nc.sync.dma_start(s4s[:], s4_d[:, cs])
                q8 = p2q.tile([128, 6, 512], F8, tag="q8", name=f"q8_{qb}")

                def up_chain(ps, w_t, wr_t, ccols, np_, lat=qlat_t,
                             latr=qlatr_t):
                    terms = ((w_t, lat), (wr_t, lat), (w_t, latr))
                    for ti, (tw, tl) in enumerate(terms):
                        for qp in range(np_):
                            nc.tensor.matmul(
                                ps[:], tw[:, 2 * qp:2 * qp + 2, ccols],
                                tl[:, 2 * qp:2 * qp + 2, cs],
                                start=(ti == 0 and qp == 0),
                                stop=(ti == 2 and qp == np_ - 1),
                                perf_mode=DR_MODE,
                            )

                for pr in range(2):
                    ps = alt_ps(pr)
                    up_chain(ps, wqr_t, wqrr_t,
                             slice(128 * pr, 128 * (pr + 1)), NLQ // 2)
                    rope_pair(nc, p2tmp,
                              [(q8[:, 3 * pr, :], slice(0, 128))],
                              ps, c4s[:], s4s[:])
                for h in range(4):
                    ps = alt_ps(h)
                    up_chain(ps, wuq_t, wuqr_t,
                             slice(128 * h, 128 * (h + 1)), NLQ // 2)
                    slot = (1, 2, 4, 5)[h]
                    nc.scalar.activation(
                        q8[:, slot, :], ps[:], AF.Copy,
                        scale=SQ8 * UP_UNSCALE)
                return q8

            with (
                tc.tile_pool(name="p1w", bufs=1) as p1w,
                tc.tile_pool(name="p1tmp", bufs=2) as p1tmp,
            ):
                wuk_t = p1w.tile([128, NLKV, 512], F8)
                wukr_t = p1w.tile([128, NLKV, 512], F8)
                wkr_t = p1w.tile([128, NLKV, 256], F8)
                wkrr_t = p1w.tile([128, NLKV, 256], F8)
                wuv_t = p1w.tile([128, NLKV, 512], F8)
                wuvr_t = p1w.tile([128, NLKV, 512], F8)
                nc.sync.dma_start(wuk_t[:], col3(wuk_d[:]))
                nc.sync.dma_start(wukr_t[:], col3(wukr_d[:]))
                nc.sync.dma_start(wuq_t[:], col3(wuq_d[:]))
                nc.sync.dma_start(wuqr_t[:], col3(wuqr_d[:]))
                nc.sync.dma_start(wqr_t[:], col3(wqr_d[:]))
                nc.sync.dma_start(wqrr_t[:], col3(wqrr_d[:]))
                # q8 for block 0 first: its inputs are ready before the
                # AllGathered kv latents land, filling the P1 entry stall
                q8_0 = compute_q8(0)
                for sb in range(n_sb):
                    cs = slice(512 * sb, 512 * (sb + 1))
                    if sb == 0:
                        nc.sync.dma_start(wkr_t[:], col3(wkr_d[:]))
                        nc.sync.dma_start(wkrr_t[:], col3(wkrr_d[:]))
                        nc.sync.dma_start(wuv_t[:], col3(wuv_d[:]))
                        nc.sync.dma_start(wuvr_t[:], col3(wuvr_d[:]))
                    c4s = p1tmp.tile([128, 512], F16, tag="c4")
                    s4s = p1tmp.tile([128, 512], F16, tag="s4")
                    nc.sync.dma_start(c4s[:], c4_d[:, cs])
                    nc.sync.dma_start(s4s[:], s4_d[:, cs])
                    def kv_chain(ps, w_t, wr_t, ccols):
                        terms = ((w_t, kvlat_t), (wr_t, kvlat_t),
                                 (w_t, kvlatr_t))
                        for ti, (tw, tl) in enumerate(terms):
                            for kp in range(NLKV // 2):
                                nc.tensor.matmul(
                                    ps[:], tw[:, 2 * kp:2 * kp + 2, ccols],
                                    tl[:, 2 * kp:2 * kp + 2, cs],
                                    start=(ti == 0 and kp == 0),
                                    stop=(ti == 2 and kp == NLKV // 2 - 1),
                                    perf_mode=DR_MODE,
                                )

                    for h in range(4):
                        ps = alt_ps(h)
                        kv_chain(ps, wuk_t, wukr_t,
                                 slice(128 * h, 128 * (h + 1)))
                        nc.scalar.activation(
                            kT8_t[:, h, 1, cs], ps[:], AF.Copy,
                            scale=SQ8 * UP_UNSCALE)
                    for pr in range(2):
                        ps = alt_ps(pr)
                        kv_chain(ps, wkr_t, wkrr_t,
                                 slice(128 * pr, 128 * (pr + 1)))
                        he, ho = 2 * pr, 2 * pr + 1
                        rope_pair(
                            nc, p1tmp, [
                                (kT8_t[0:64, he, 0, cs], slice(0, 64)),
                                (kT8_t[64:128, ho, 0, cs], slice(64, 128)),
                            ], ps, c4s[:], s4s[:])
                    for stl in range(4):
                        st = 4 * sb + stl
                        stc = slice(512 * sb + 128 * stl,
                                    512 * sb + 128 * (stl + 1))
                        ps = alt_ps(stl)
                        terms = ((kvlat_t, wuv_t), (kvlatr_t, wuv_t),
                                 (kvlat_t, wuvr_t))
                        for ti, (tl, tw) in enumerate(terms):
                            for kp in range(NLKV // 2):
                                nc.tensor.matmul(
                                    ps[:], tl[:, 2 * kp:2 * kp + 2, stc],
                                    tw[:, 2 * kp:2 * kp + 2, :],
                                    start=(ti == 0 and kp == 0),
                                    stop=(ti == 2 and kp == NLKV // 2 - 1),
                                    perf_mode=DR_MODE,
                                )
                        # x SO so the fp8 split of attention outputs uses
                        # fp8 normal range (unscaled at the yT stage)
                        nc.scalar.activation(v_t[:, st, :], ps[:], AF.Copy,
                                             scale=SO * UP_UNSCALE)

            if debug:
                nc.sync.dma_start(dbg_qlat_d[:], qlat_t[:])
                nc.sync.dma_start(dbg_kvlat_d[:], kvlat_t[:])
                nc.sync.dma_start(dbg_v_d[:], v_t[:])
                nc.gpsimd.dma_start(dbg_kT8_d[:], kT8_t[:])

            # ---------------- P2: attention + W_o ----------------
            PHASE_MARKS["P2"] = nc.next_id()
            with (
                tc.tile_pool(name="p2exp", bufs=5) as p2exp,
                tc.tile_pool(name="ps_s", bufs=2, space="PSUM") as ps_sp,
                tc.tile_pool(name="p2acc", bufs=2) as p2acc,
                tc.tile_pool(name="p2acc1", bufs=2) as p2acc1,
                tc.tile_pool(name="p2out", bufs=2) as p2out,
                tc.tile_pool(name="p2y", bufs=4) as p2y,
            ):
                def emit_wo(outs, cs, dts=range(NDT), alt=False):
                    o8, o8r = outs
                    ystage = None
                    for dt in dts:
                        dsl = slice(128 * dt, 128 * (dt + 1))
                        # the final (non-interleaved) call alternates PSUM
                        # pools for 4-bank pipelining against the ystage drain
                        ps_y = alt_ps(dt if alt else 0)
                        for j in (0, 1):
                            hp = slice(2 * j, 2 * j + 2)
                            for ti, (w_s, o_s) in enumerate(
                                ((wo_t, o8), (wor_t, o8), (wo_t, o8r))
                            ):
                                nc.tensor.matmul(
                                    ps_y[:], w_s[:, hp, dsl], o_s[:, hp, :],
                                    start=(j == 0 and ti == 0),
                                    stop=(j == 1 and ti == 2),
                                    perf_mode=DR_MODE,
                                )
                        # pair two d-tiles per ystage buffer and yT store to
                        # halve the store count (the rows are DRAM-adjacent)
                        if ystage is None:
                            ystage = p2y.tile([128, 2, 512], F16, tag="y")
                        half = ystage[:, dt % 2, :]
                        if dt % 2 == 0:
                            nc.vector.tensor_scalar(
                                half, ps_y[:], Y_UNSCALE, None,
                                mybir.AluOpType.mult)
                        else:
                            nc.scalar.activation(
                                half, ps_y[:], AF.Copy, scale=Y_UNSCALE)
                            nc.sync.dma_start(
                                col3(yT_d[128 * (dt - 1):128 * (dt + 1), cs]),
                                ystage[:])
                            ystage = None

                prev_out = None
                prev_cs = None
                q8_next = q8_0
                for qb in range(n_sb):
                    cs = slice(512 * qb, 512 * (qb + 1))
                    q8 = q8_next

                    def q8_mov(h):
                        base = 3 * (h // 2)
                        if h % 2 == 0:
                            return q8[:, base:base + 2, :]
                        return q8[:, base:base + 3:2, :]

                    o8 = p2out.tile([128, 4, 512], F8, tag="o8",
                                    name=f"o8_{qb}")
                    o8r = p2out.tile([128, 4, 512], F8, tag="o8r",
                                     name=f"o8r_{qb}")
                    for h in range(4):
                        nkt = 4 * (qb + 1)
                        npair = nkt // 2
                        ps_o = ps_op.tile([128, 512], F32, tag="pv")
                        dacc = p2acc.tile([128, 1024], F16, tag="dacc")

                        def emit_pv(exp_pair, pk, npair, ps_o=ps_o, h=h):
                            for j in (0, 1):
                                kt = 2 * pk + j
                                nc.tensor.matmul(
                                    ps_o[:],
                                    v_t[:, kt, 128 * h:128 * (h + 1)],
                                    exp_pair[:, 512 * j:512 * (j + 1)],
                                    start=(kt == 0), stop=(kt == 2 * npair - 1),
                                )

                        pend = []   # (exp pair tile, pk) one pair behind
                        for pk in range(npair):
                            ps_s = ps_sp.tile([128, 1024], F32, tag="scores")
                            for j in (0, 1):
                                kt = 2 * pk + j
                                ks = slice(128 * kt, 128 * (kt + 1))
                                delta = 128 * kt - 512 * qb
                                diag = delta >= 0
                                half = ps_s[:, 512 * j:512 * (j + 1)]
                                nc.tensor.matmul(
                                    half, kT8_t[:, h, :, ks], q8_mov(h),
                                    start=True, stop=not diag,
                                    perf_mode=DR_MODE,
                                )
                                if diag:
                                    nc.tensor.matmul(
                                        half, id_t[:],
                                        mask_t[:, :, 384 - delta:896 - delta],
                                        start=False, stop=True,
                                        perf_mode=DR_MODE,
                                    )
                            exp_t = p2exp.tile([128, 1024], F16, tag="exp")
                            nc.scalar.activation(
                                exp_t[:], ps_s[:], AF.Exp, scale=EXP_SCALE)
                            # two alternating accumulators halve the serial
                            # add-chain latency on DVE
                            half = dacc[:, 512 * (pk % 2):512 * (pk % 2) + 512]
                            if pk < 2:
                                nc.vector.tensor_add(
                                    half, exp_t[:, 0:512], exp_t[:, 512:1024])
                            else:
                                nc.vector.tensor_add(
                                    half, half, exp_t[:, 0:512])
                                nc.vector.tensor_add(
                                    half, half, exp_t[:, 512:1024])
                            pend.append((exp_t, pk))
                            if len(pend) > 2:
                                emit_pv(*pend.pop(0), npair)
                        for e in pend:
                            emit_pv(*e, npair)
                        dfold = p2acc1.tile([128, 512], F16, tag="dfold")
                        if npair > 1:
                            nc.vector.tensor_add(
                                dfold[:], dacc[:, 0:512], dacc[:, 512:1024])
                        else:
                            nc.vector.tensor_copy(dfold[:], dacc[:, 0:512])
                        dred = p2acc1.tile([128, 512], F32, tag="dred")
                        nc.gpsimd.partition_all_reduce(
                            dred[:], dfold[:], 128, bass_isa.ReduceOp.add)
                        rsb = p2tmp.tile([128, 512], F32, tag="rsbd")
                        nc.vector.reciprocal(rsb[:], dred[:])
                        o16 = p2tmp.tile([128, 512], F16, tag="o16")
                        nc.vector.tensor_mul(o16[:], ps_o[:], rsb[:])
                        nc.vector.tensor_copy(o8[:, h, :], o16[:])
                        nc.vector.tensor_sub(o8r[:, h, :], o16[:], o8[:, h, :])
                        # interleave W_o d-tiles of the previous q-block so
                        # the in-order PE stream has fill work during this
                        # block's exp-latency stalls
                        if prev_out is not None:
                            emit_wo(prev_out, prev_cs,
                                    range(4 * h, 4 * (h + 1)))
                        # interleave the next block's q projections mid-stream
                        # instead of serializing them at the block boundary
                        if h == 1 and qb + 1 < n_sb:
                            q8_next = compute_q8(qb + 1)
                    prev_out, prev_cs = (o8, o8r), cs
                emit_wo(prev_out, prev_cs, alt=True)
            p2w_stack.close()
            persist_stack.close()

    nc.compile()
    return nc


def host_prep(inputs, S=S_FULL):
    """Build the 8 per-core input maps from the full problem inputs."""
    FP8 = ml_dtypes.float8_e4m3

    def to8(a):
        return np.ascontiguousarray(a).astype(FP8)

    def split8(a, scale):
        hi = (a * scale).astype(FP8)
        lo = (a * scale - hi.astype(np.float32)).astype(FP8)
        return hi, lo

    x = np.asarray(inputs["x"], np.float32)
    cosT = np.asarray(inputs["rope_cos"], np.float32).T
    sinT = np.asarray(inputs["rope_sin"], np.float32).T
    c4 = np.concatenate([cosT, cosT, cosT, cosT], 0) * SQ8
    s4 = np.concatenate([-sinT, sinT, -sinT, sinT], 0) * SQ8
    c4 = np.ascontiguousarray(c4).astype(np.float16)
    s4 = np.ascontiguousarray(s4).astype(np.float16)
    qw = np.asarray(inputs["q_norm_w"], np.float32)
    kvw = np.asarray(inputs["kv_norm_w"], np.float32)
    W_uq = np.asarray(inputs["W_uq"], np.float32) * qw[:, None]
    W_qr = np.asarray(inputs["W_qr"], np.float32) * qw[:, None]
    W_uk = np.asarray(inputs["W_uk"], np.float32) * kvw[:, None]
    W_kr = np.asarray(inputs["W_kr"], np.float32) * kvw[:, None]
    W_uv = np.asarray(inputs["W_uv"], np.float32) * kvw[:, None]
    W_o = np.asarray(inputs["W_o"], np.float32)
    W_dq = np.asarray(inputs["W_dq"], np.float32)
    W_dkv = np.asarray(inputs["W_dkv"], np.float32)

    wdq8, wdq8r = split8(W_dq, BW)
    wdkv8, wdkv8r = split8(W_dkv, BW)
    wo8_full, wo8r_full = split8(W_o, BWO)
    wuq8, wuq8r = split8(W_uq, BW)
    wqr8, wqr8r = split8(W_qr, BW)
    wuk8, wuk8r = split8(W_uk, BW)
    wkr8, wkr8r = split8(W_kr, BW)
    wuv8, wuv8r = split8(W_uv, BW)

    # mask table: plane 0 = {0, -240} causal pattern, plane 1 = 0
    cgrid = np.arange(896)[None, :] - 384
    igrid = np.arange(128)[:, None]
    mask8 = np.zeros((128, 2, 896), np.float32)
    mask8[:, 0, :] = np.where(cgrid >= igrid, 0.0, -240.0)
    mask8 = mask8.astype(FP8)
    id8 = np.zeros((128, 2, 128), np.float32)
    id8[:, 0, :] = MASK_ID * np.eye(128, dtype=np.float32)
    id8 = id8.astype(FP8)
    ones8 = np.ones((128, 1), np.float32).astype(FP8)

    in_maps = []
    for c in range(NCORES):
        b, g = c // 4, c % 4
        hs = slice(4 * g * DN, 4 * (g + 1) * DN)
        hr = slice(4 * g * DRR, 4 * (g + 1) * DRR)
        xT = np.ascontiguousarray(x[b].T)
        x8, x8r = split8(xT, AX)
        in_maps.append(dict(
            x8=x8, x8r=x8r,
            xkv8=np.ascontiguousarray(x8[:, 512 * g:512 * (g + 1)]),
            xkv8r=np.ascontiguousarray(x8r[:, 512 * g:512 * (g + 1)]),
            W_dq8=wdq8, W_dq8r=wdq8r,
            W_dkv8=wdkv8, W_dkv8r=wdkv8r,
            Wuq8=np.ascontiguousarray(wuq8[:, hs]),
            Wuq8r=np.ascontiguousarray(wuq8r[:, hs]),
            Wqr8=np.ascontiguousarray(wqr8[:, hr]),
            Wqr8r=np.ascontiguousarray(wqr8r[:, hr]),
            Wuk8=np.ascontiguousarray(wuk8[:, hs]),
            Wuk8r=np.ascontiguousarray(wuk8r[:, hs]),
            Wkr8=np.ascontiguousarray(wkr8[:, hr]),
            Wkr8r=np.ascontiguousarray(wkr8r[:, hr]),
            Wuv8=np.ascontiguousarray(wuv8[:, hs]),
            Wuv8r=np.ascontiguousarray(wuv8r[:, hs]),
            Wo8=np.ascontiguousarray(wo8_full[512 * g:512 * (g + 1), :]),
            Wo8r=np.ascontiguousarray(wo8r_full[512 * g:512 * (g + 1), :]),
            c4=c4, s4=s4, mask8=mask8, id8=id8, ones8=ones8,
        ))
    return in_maps


_NC_CACHE = {}


def kernel(**inputs) -> np.ndarray:
    S = np.asarray(inputs["x"]).shape[1]
    if S not in _NC_CACHE:
        _NC_CACHE[S] = build_nc(S)
    nc = _NC_CACHE[S]
    in_maps = host_prep(inputs, S)
    res = run_bass_kernel_spmd(nc, in_maps, core_ids=list(range(NCORES)))
    y = np.empty((B, S, D), np.float32)
    for b in range(B):
        acc = res.results[4 * b]["yT"].astype(np.float32)
        for g in range(1, 4):
            acc = acc + res.results[4 * b + g]["yT"].astype(np.float32)
        y[b] = acc.T
    return y


# revision 74
# speedup vs baseline: 2.0120x; 1.0007x over previous
"""MultiHeadLatentAttention (MLA) Trainium2 Bass kernel, v2.

Problem: B=2, S=2048, D=2048, H=16 heads, d_nope=128, d_rope=64, d_head=128,
q_latent=768, kv_latent=512. Causal attention, rmsnorm'd latents, half-dim RoPE.

Sharding (8 cores): core c handles batch b=c//4 and head group g=c%4 (4 heads).
The small latent down-projections are replicated within each batch group;
W_uq/W_qr/W_uk/W_kr/W_uv are column-sharded by head; W_o row-sharded; the
4 partial outputs per batch are summed on the host.

Precision/engine plan (metric = InstructionCostModel timeline; baseline
559461 ns -> 278263 ns, rel err 1.32e-2 < 2e-2):
  - scores matmul in fp8e4m3 with MatmulPerfMode.DoubleRow: the nope(128) and
    zero-padded rope(64) contractions are packed as the two DoubleRow k-tiles,
    so each 128x512 score tile costs 256 PE cycles instead of 1024.
    End-to-end error from quantizing qn/kn/qr/kr to fp8 measured 1.33e-2;
    every other fp8 stage below is a lossless-ish residual split adding <1e-3.
  - causal mask added in the same PSUM group by a fp8 DoubleRow matmul of
    60*I against a {0,-240} mask table (-14400 pre-scale -> exp()=2e-9).
  - all projections (down, up, W_o) run as 3-term fp8 DoubleRow residual
    splits (W8@x8 + W8r@x8 + W8@x8r, dropping the second-order W8r@x8r):
    fp8 PE speed (0.5 cycles/row) at fp16-like accuracy. Weights are split
    on the host; x arrives pre-split; latents are split once at the rmsnorm
    multiply; attention outputs are split on DVE before W_o.
  - kv down-projection is S-sharded: each core computes only its own 512-col
    block and the fp8 hi+lo latent pair is AllGathered (2.1MB) on the
    collective cores, overlapped with the replicated q down-projection.
    The collective's SWDGE upload/downloads live on the otherwise-empty Pool
    queue so its in-order waits block nothing.
  - PV in fp16 (exp quantization to fp8 would cost ~3% output error).
  - softmax: exp on ACT (fp16 out), denominator via two alternating fp16
    accumulators on DVE + partition_all_reduce on Pool + DVE reciprocal --
    no Ln anywhere, so a single act-table load (was 49 reloads/63us).
  - rmsnorm rsqrt = ACT Sqrt(DVE reciprocal(mean sq)); the partition
    broadcast of the per-token scale is a PE outer product (ones x row) so
    P0 keeps the Pool queue empty for the collective.
  - W_o, latents, k^T, v stay resident in SBUF; W_o(prev block) d-tiles and
    q8(next block) are interleaved into the in-order PE stream as fill work
    during exp-latency stalls; yT stores pair two d-tiles per DMA.
"""
import math
import os
from contextlib import ExitStack

import numpy as np
import ml_dtypes

import concourse.bass as bass
import concourse.bass_isa as bass_isa
import concourse.bacc as bacc
import concourse.mybir as mybir
import concourse.tile as tile
from concourse.bass_utils import run_bass_kernel_spmd

F32 = mybir.dt.float32
F32R = mybir.dt.float32r
F16 = mybir.dt.float16
F8 = mybir.dt.float8e4
AF = mybir.ActivationFunctionType
DR_MODE = mybir.MatmulPerfMode.DoubleRow

B, S_FULL, D = 2, 2048, 2048
H, DN, DRR, DH = 16, 128, 64, 128
QL, KVL = 768, 512
EPS = 1e-6
SCALE = 1.0 / math.sqrt(DH)
NCORES = 8
NKT = D // 128          # 16 contraction tiles over D
NKP = NKT // 2          # 8 DoubleRow pairs
NLQ = QL // 128         # 6
NLKV = KVL // 128       # 4
NDT = D // 128          # 16 output D tiles

# fp8 scaling for the residual-split down-projection: x' = x*AX, W' = W*BW
# so both the quantized tensors and their residuals stay in fp8 normal range.
AX = 32.0
BW = 256.0
PSUM_UNSCALE = 1.0 / (AX * BW)
# residual-split up-projections: latents x SL (folded into the rsqrt), and
# the up-projection weights x BW
SL = 16.0
UP_UNSCALE = 1.0 / (SL * BW)
# score operand quantization scale (qn8 = 8*qn etc.)
SQ8 = 8.0
EXP_SCALE = SCALE / (SQ8 * SQ8)
MASK_ID = 60.0          # mask matmul: 60 * (-240) * 1 plane = -14400 pre-scale
# W_o fp8 residual split: out tiles scaled x32 (folded into v), W_o x1024
SO = 32.0
BWO = 1024.0
Y_UNSCALE = 1.0 / (SO * BWO)

PHASE_MARKS = {}


def build_nc(S=S_FULL):
    assert S % 512 == 0
    n_sb = S // 512
    n_st = S // 128
    PHASE_MARKS.clear()

    nc = bacc.Bacc("TRN2", target_bir_lowering=False, debug=False,
                   num_devices=NCORES)

    x8_d = nc.dram_tensor("x8", [D, S], F8, kind="ExternalInput")
    x8r_d = nc.dram_tensor("x8r", [D, S], F8, kind="ExternalInput")
    # per-core own-block column slice of x, for the S-sharded kv down-proj
    xkv8_d = nc.dram_tensor("xkv8", [D, 512], F8, kind="ExternalInput")
    xkv8r_d = nc.dram_tensor("xkv8r", [D, 512], F8, kind="ExternalInput")
    wdq_d = nc.dram_tensor("W_dq8", [D, QL], F8, kind="ExternalInput")
    wdqr_d = nc.dram_tensor("W_dq8r", [D, QL], F8, kind="ExternalInput")
    wdkv_d = nc.dram_tensor("W_dkv8", [D, KVL], F8, kind="ExternalInput")
    wdkvr_d = nc.dram_tensor("W_dkv8r", [D, KVL], F8, kind="ExternalInput")
    wuq_d = nc.dram_tensor("Wuq8", [QL, 512], F8, kind="ExternalInput")
    wuqr_d = nc.dram_tensor("Wuq8r", [QL, 512], F8, kind="ExternalInput")
    wqr_d = nc.dram_tensor("Wqr8", [QL, 256], F8, kind="ExternalInput")
    wqrr_d = nc.dram_tensor("Wqr8r", [QL, 256], F8, kind="ExternalInput")
    wuk_d = nc.dram_tensor("Wuk8", [KVL, 512], F8, kind="ExternalInput")
    wukr_d = nc.dram_tensor("Wuk8r", [KVL, 512], F8, kind="ExternalInput")
    wkr_d = nc.dram_tensor("Wkr8", [KVL, 256], F8, kind="ExternalInput")
    wkrr_d = nc.dram_tensor("Wkr8r", [KVL, 256], F8, kind="ExternalInput")
    wuv_d = nc.dram_tensor("Wuv8", [KVL, 512], F8, kind="ExternalInput")
    wuvr_d = nc.dram_tensor("Wuv8r", [KVL, 512], F8, kind="ExternalInput")
    wo_d = nc.dram_tensor("Wo8", [512, D], F8, kind="ExternalInput")
    wor_d = nc.dram_tensor("Wo8r", [512, D], F8, kind="ExternalInput")
    c4_d = nc.dram_tensor("c4", [128, S], F16, kind="ExternalInput")
    s4_d = nc.dram_tensor("s4", [128, S], F16, kind="ExternalInput")
    mask_d = nc.dram_tensor("mask8", [128, 2, 896], F8, kind="ExternalInput")
    id_d = nc.dram_tensor("id8", [128, 2, 128], F8, kind="ExternalInput")
    ones_d = nc.dram_tensor("ones8", [128, 1], F8, kind="ExternalInput")
    yT_d = nc.dram_tensor("yT", [D, S], F16, kind="ExternalOutput")
    debug = bool(int(os.environ.get("MLA_DEBUG", "0")))
    if debug:
        dbg_qlat_d = nc.dram_tensor("dbg_qlat", [128, NLQ, S], F16,
                                    kind="ExternalOutput")
        dbg_kvlat_d = nc.dram_tensor("dbg_kvlat", [128, NLKV, S], F16,
                                     kind="ExternalOutput")
        dbg_kT8_d = nc.dram_tensor("dbg_kT8", [128, 4, 2, S], F16,
                                   kind="ExternalOutput")
        dbg_v_d = nc.dram_tensor("dbg_v", [128, n_st, 512], F16,
                                 kind="ExternalOutput")

    def col3(dram_ap, p=128):
        # [R, C] dram slice -> [128, R//128, C] tiled AP
        return dram_ap.rearrange("(t p) c -> p t c", p=p)

    def rope_pair(nc, pool, outs, ps, c4s, s4s):
        """Half-dim rope on a 2-head pair tile [128, 512] in PSUM.

        out = ps * c4 + shuf(ps) * s4, shuf swaps 32-blocks within each 64.
        Stages through fp16 SBUF so the DVE muls run in 2x mode. `outs` is a
        list of (out_ap, pslice) fp8 destinations.
        """
        rs = pool.tile([128, 512], F16, tag="rope_rs")
        nc.scalar.activation(rs[:], ps[:], AF.Copy, scale=UP_UNSCALE)
        shuf = pool.tile([128, 512], F16, tag="rope_shuf")
        nc.vector.tensor_copy(shuf[0:32, :], rs[32:64, :])
        nc.vector.tensor_copy(shuf[32:64, :], rs[0:32, :])
        nc.vector.tensor_copy(shuf[64:96, :], rs[96:128, :])
        nc.vector.tensor_copy(shuf[96:128, :], rs[64:96, :])
        t1 = pool.tile([128, 512], F16, tag="rope_t1")
        nc.vector.tensor_mul(t1[:], rs[:], c4s)
        nc.vector.tensor_mul(shuf[:], shuf[:], s4s)
        for out_ap, psl in outs:
            nc.vector.tensor_add(out_ap, t1[psl, :], shuf[psl, :])

    with tile.TileContext(nc) as tc:
        with (
            tc.tile_pool(name="const", bufs=1) as constp,
            tc.tile_pool(name="ps_mm", bufs=2, space="PSUM") as ps_mm,
            tc.tile_pool(name="ps_o", bufs=2, space="PSUM") as ps_op,
        ):
            def alt_ps(i):
                if i % 2 == 0:
                    return ps_mm.tile([128, 512], F32, tag="mm", name="ps")
                return ps_op.tile([128, 512], F32, tag="pv", name="ps")

            mask_t = constp.tile([128, 2, 896], F8)
            id_t = constp.tile([128, 2, 128], F8)
            ones_t = constp.tile([128, 1], F8)
            ones_row = constp.tile([1, 128], F16)
            nc.vector.memset(ones_row[:], 1.0)

            # persistent SBUF state
            persist_stack = ExitStack()
            persist = persist_stack.enter_context(
                tc.tile_pool(name="persist", bufs=1))
            # kT8: per head (rope_padded, nope) planes, fp8 stationary
            kT8_t = persist.tile([128, 4, 2, S], F8)
            v_t = persist.tile([128, n_st, 512], F16)
            wo_t = persist.tile([128, 4, D], F8)
            wor_t = persist.tile([128, 4, D], F8)
            qlat_t = persist.tile([128, NLQ, S], F8)
            qlatr_t = persist.tile([128, NLQ, S], F8)
            kvlat_t = persist.tile([128, NLKV, S], F8)
            kvlatr_t = persist.tile([128, NLKV, S], F8)

            # ---------------- P0: down-projections + rmsnorm ----------------
            PHASE_MARKS["P0"] = nc.next_id()
            p0_stack = ExitStack()
            p0w = p0_stack.enter_context(tc.tile_pool(name="p0w", bufs=1))
            wdq_t = p0w.tile([128, NKT, QL], F8)
            wdqr_t = p0w.tile([128, NKT, QL], F8)
            wdkv_t = p0w.tile([128, NKT, KVL], F8)
            wdkvr_t = p0w.tile([128, NKT, KVL], F8)
            with (
                tc.tile_pool(name="p0x", bufs=2) as p0x,
                tc.tile_pool(name="p0raw", bufs=2) as p0raw,
                tc.tile_pool(name="p0sq", bufs=2) as p0sq,
                tc.tile_pool(name="p0own", bufs=1) as p0own,
                tc.tile_pool(name="p0tmp", bufs=2) as p0tmp,
                tc.tile_pool(name="p0dram", bufs=1, space="DRAM") as p0dram,
                tc.tile_pool(name="ps_den", bufs=1, space="PSUM") as ps_denp,
                tc.tile_pool(name="ps_p0", bufs=2, space="PSUM") as ps_p0,
            ):
                def alt3_ps(i):
                    if i % 3 == 2:
                        return ps_p0.tile([128, 512], F32, tag="p0", name="ps")
                    return alt_ps(i % 3)

                # zero the pad halves of the rope planes once (rope data for
                # even heads lives at partitions 0:64, odd heads at 64:128)
                for h in range(4):
                    lo = 0 if h % 2 else 64
                    nc.gpsimd.memset(kT8_t[lo:lo + 64, h, 0, :], 0.0)

                def down_proj(latname, w_t, wr_t, nl, xh, xrh, dest8,
                              dest8r, rawp=None, sqp=None, terms3=True):
                    raw = (rawp or p0raw).tile(
                        [128, nl, 512], F16, tag=f"raw{latname}",
                        name=f"raw{latname}")
                    sq = (sqp or p0sq).tile(
                        [128, nl, 512], F8, tag=f"sq{latname}",
                        name=f"sq{latname}")
                    ps_ss = ps_denp.tile([1, 512], F32, tag="den")
                    for lt in range(nl):
                        ps = alt3_ps(lt)
                        lsl = slice(128 * lt, 128 * (lt + 1))
                        terms = ((w_t, xh), (wr_t, xh), (w_t, xrh)) \
                            if terms3 else ((w_t, xh), (w_t, xrh))
                        for term_w, term_x in terms:
                            first = term_w is w_t and term_x is xh
                            last = term_x is xrh
                            for kp in range(NKP):
                                nc.tensor.matmul(
                                    ps[:],
                                    term_w[:, 2 * kp:2 * kp + 2, lsl],
                                    term_x[:, 2 * kp:2 * kp + 2, :],
                                    start=(first and kp == 0),
                                    stop=(last and kp == NKP - 1),
                                    perf_mode=DR_MODE,
                                )
                        nc.scalar.activation(
                            raw[:, lt, :], ps[:], AF.Copy, scale=PSUM_UNSCALE)
                        nc.scalar.activation(
                            sq[:, lt, :], ps[:], AF.Square, scale=PSUM_UNSCALE)
                        nc.tensor.matmul(
                            ps_ss[:], ones_t[:], sq[:, lt, :],
                            start=(lt == 0), stop=(lt == nl - 1),
                        )
                    # rsqrt of mean square: sqrt(1/m) via DVE reciprocal
                    # + ACT Sqrt (Copy/Square live in the sqrt act table,
                    # so P0 needs no act-table reloads)
                    mrow = p0tmp.tile([1, 512], F32, tag="mrow")
                    nc.vector.tensor_scalar(
                        mrow[:], ps_ss[:], 1.0 / (128 * nl), EPS,
                        mybir.AluOpType.mult, mybir.AluOpType.add)
                    rrec = p0tmp.tile([1, 512], F32, tag="rrec")
                    nc.vector.reciprocal(rrec[:], mrow[:])
                    rrow = p0tmp.tile([1, 512], F16, tag="rrow")
                    # scale=SL^2 folds the latent fp8 scale into the rsqrt
                    nc.scalar.activation(rrow[:], rrec[:], AF.Sqrt,
                                         scale=SL * SL)
                    # broadcast across partitions via a PE outer product so P0
                    # keeps the Pool queue empty (the AllGather blocks it)
                    ps_bc = ps_denp.tile([128, 512], F32, tag="bc")
                    nc.tensor.matmul(ps_bc[:], ones_row[:], rrow[:],
                                     start=True, stop=True)
                    rsb = p0tmp.tile([128, 512], F16, tag="rsb")
                    nc.scalar.copy(rsb[:], ps_bc[:])
                    for lt in range(nl):
                        tmp = p0tmp.tile([128, 512], F16, tag="ntmp")
                        nc.vector.tensor_mul(tmp[:], raw[:, lt, :], rsb[:])
                        nc.vector.tensor_copy(dest8(lt), tmp[:])
                        nc.vector.tensor_sub(dest8r(lt), tmp[:], dest8(lt))

                # --- kv down-proj for this core's own block only; the other
                # blocks arrive via an AllGather of the fp16 latents that
                # overlaps with the (replicated) q down-projection.
                xkvh = p0x.tile([128, NKT, 512], F8, tag="x8", name="xkv8")
                xkvrh = p0x.tile([128, NKT, 512], F8, tag="x8r", name="xkv8r")
                nc.sync.dma_start(xkvh[:], col3(xkv8_d))
                nc.sync.dma_start(ones_t[:], ones_d[:])
                nc.sync.dma_start(wdkv_t[:], col3(wdkv_d[:]))
                nc.sync.dma_start(wdkvr_t[:], col3(wdkvr_d[:]))
                nc.sync.dma_start(xkvrh[:], col3(xkv8r_d))
                kvlat_own = p0own.tile([128, NLKV, 512], F8, tag="kvown",
                                       name="kvlat_own")
                kvlatr_own = p0own.tile([128, NLKV, 512], F8, tag="kvownr",
                                        name="kvlatr_own")
                down_proj("kv", wdkv_t, wdkvr_t, NLKV, xkvh, xkvrh,
                          lambda lt: kvlat_own[:, lt, :],
                          lambda lt: kvlatr_own[:, lt, :],
                          rawp=p0own, sqp=p0own)
                # the whole collective path lives on the (otherwise idle)
                # Pool queue: its in-order waits must not block the SP/ACT
                # DMA queues or the ACT compute stream
                kv_own_d = p0dram.tile([2 * KVL, 512], F8, name="kv_own")
                kv_all_d = p0dram.tile([8 * KVL, 512], F8, name="kv_all")
                nc.gpsimd.dma_start(col3(kv_own_d[0:KVL, :]), kvlat_own[:])
                nc.gpsimd.dma_start(col3(kv_own_d[KVL:2 * KVL, :]),
                                    kvlatr_own[:])
                nc.gpsimd.collective_compute(
                    "AllGather",
                    mybir.AluOpType.bypass,
                    replica_groups=[[0, 1, 2, 3], [4, 5, 6, 7]],
                    ins=[kv_own_d[:]],
                    outs=[kv_all_d[:]],
                )
                for c in range(4):
                    base = c * 2 * KVL
                    nc.gpsimd.dma_start(
                        kvlat_t[:, :, 512 * c:512 * (c + 1)],
                        col3(kv_all_d[base:base + KVL, :]))
                    nc.gpsimd.dma_start(
                        kvlatr_t[:, :, 512 * c:512 * (c + 1)],
                        col3(kv_all_d[base + KVL:base + 2 * KVL, :]))

                # --- replicated q down-projection over all blocks
                for sb in range(n_sb):
                    cs = slice(512 * sb, 512 * (sb + 1))
                    xh = p0x.tile([128, NKT, 512], F8, tag="x8", name=f"x8_{sb}")
                    xrh = p0x.tile([128, NKT, 512], F8, tag="x8r",
                                   name=f"x8r_{sb}")
                    nc.sync.dma_start(xh[:], col3(x8_d[:, cs]))
                    if sb == 0:
                        nc.sync.dma_start(wdq_t[:], col3(wdq_d[:]))
                        nc.sync.dma_start(wdqr_t[:], col3(wdqr_d[:]))
                    nc.sync.dma_start(xrh[:], col3(x8r_d[:, cs]))
                    if sb == 0:
                        nc.sync.dma_start(mask_t[:], mask_d[:])
                        nc.sync.dma_start(id_t[:], id_d[:])
                    if sb == 3:
                        # W_o resident load (needed only from P2, and after
                        # the last x chunks so it never delays them)
                        nc.sync.dma_start(wo_t[:], col3(wo_d[:]))
                        nc.sync.dma_start(wor_t[:], col3(wor_d[:]))
                    down_proj("q", wdq_t, wdqr_t, NLQ, xh, xrh,
                              lambda lt, cs=cs: qlat_t[:, lt, cs],
                              lambda lt, cs=cs: qlatr_t[:, lt, cs])
            p0_stack.close()

            # ---------------- P1: k/v up-projections ----------------
            PHASE_MARKS["P1"] = nc.next_id()
            p2w_stack = ExitStack()
            p2w = p2w_stack.enter_context(tc.tile_pool(name="p2w", bufs=1))
            p2q = p2w_stack.enter_context(tc.tile_pool(name="p2q", bufs=2))
            p2tmp = p2w_stack.enter_context(tc.tile_pool(name="p2tmp", bufs=2))
            wuq_t = p2w.tile([128, NLQ, 512], F8)
            wuqr_t = p2w.tile([128, NLQ, 512], F8)
            wqr_t = p2w.tile([128, NLQ, 256], F8)
            wqrr_t = p2w.tile([128, NLQ, 256], F8)

            def compute_q8(qb):
                """q up-projection + rope for one q-block into a fp8 moving
                tile with slots (qr01, qn0, qn1, qr23, qn2, qn3)."""
                cs = slice(512 * qb, 512 * (qb + 1))
                c4s = p2tmp.tile([128, 512], F16, tag="c4")
                s4s = p2tmp.tile([128, 512], F16, tag="s4")
                nc.sync.dma_start(c4s[:], c4_d[:, cs])
                nc.sync.dma_start(s4s[:], s4_d[:, cs])
                q8 = p2q.tile([128, 6, 512], F8, tag="q8", name=f"q8_{qb}")

                def up_chain(ps, w_t, wr_t, ccols, np_, lat=qlat_t,
                             latr=qlatr_t):
                    terms = ((w_t, lat), (wr_t, lat), (w_t, latr))
                    for ti, (tw, tl) in enumerate(terms):
                        for qp in range(np_):
                            nc.tensor.matmul(
                                ps[:], tw[:, 2 * qp:2 * qp + 2, ccols],
                                tl[:, 2 * qp:2 * qp + 2, cs],
                                start=(ti == 0 and qp == 0),
                                stop=(ti == 2 and qp == np_ - 1),
                                perf_mode=DR_MODE,
                            )

                for pr in range(2):
                    ps = alt_ps(pr)
                    up_chain(ps, wqr_t, wqrr_t,
                             slice(128 * pr, 128 * (pr + 1)), NLQ // 2)
                    rope_pair(nc, p2tmp,
                              [(q8[:, 3 * pr, :], slice(0, 128))],
                              ps, c4s[:], s4s[:])
                for h in range(4):
                    ps = alt_ps(h)
                    up_chain(ps, wuq_t, wuqr_t,
                             slice(128 * h, 128 * (h + 1)), NLQ // 2)
                    slot = (1, 2, 4, 5)[h]
                    nc.scalar.activation(
                        q8[:, slot, :], ps[:], AF.Copy,
                        scale=SQ8 * UP_UNSCALE)
                return q8

            with (
                tc.tile_pool(name="p1w", bufs=1) as p1w,
                tc.tile_pool(name="p1tmp", bufs=2) as p1tmp,
            ):
                wuk_t = p1w.tile([128, NLKV, 512], F8)
                wukr_t = p1w.tile([128, NLKV, 512], F8)
                wkr_t = p1w.tile([128, NLKV, 256], F8)
                wkrr_t = p1w.tile([128, NLKV, 256], F8)
                wuv_t = p1w.tile([128, NLKV, 512], F8)
                wuvr_t = p1w.tile([128, NLKV, 512], F8)
                nc.sync.dma_start(wuk_t[:], col3(wuk_d[:]))
                nc.sync.dma_start(wukr_t[:], col3(wukr_d[:]))
                nc.sync.dma_start(wuq_t[:], col3(wuq_d[:]))
                nc.sync.dma_start(wuqr_t[:], col3(wuqr_d[:]))
                nc.sync.dma_start(wqr_t[:], col3(wqr_d[:]))
                nc.sync.dma_start(wqrr_t[:], col3(wqrr_d[:]))
                # q8 for blocks 0 and 1 first: their inputs are ready
                # before the AllGathered kv latents land, filling the P1
                # entry stall with ~15us of PE work
                q8_0 = compute_q8(0)
                q8_1 = compute_q8(1)
                for sb in range(n_sb):
                    cs = slice(512 * sb, 512 * (sb + 1))
                    if sb == 0:
                        nc.sync.dma_start(wkr_t[:], col3(wkr_d[:]))
                        nc.sync.dma_start(wkrr_t[:], col3(wkrr_d[:]))
                        nc.sync.dma_start(wuv_t[:], col3(wuv_d[:]))
                        nc.sync.dma_start(wuvr_t[:], col3(wuvr_d[:]))
                    c4s = p1tmp.tile([128, 512], F16, tag="c4")
                    s4s = p1tmp.tile([128, 512], F16, tag="s4")
                    nc.sync.dma_start(c4s[:], c4_d[:, cs])
                    nc.sync.dma_start(s4s[:], s4_d[:, cs])
                    def kv_chain(ps, w_t, wr_t, ccols):
                        terms = ((w_t, kvlat_t), (wr_t, kvlat_t),
                                 (w_t, kvlatr_t))
                        for ti, (tw, tl) in enumerate(terms):
                            for kp in range(NLKV // 2):
                                nc.tensor.matmul(
                                    ps[:], tw[:, 2 * kp:2 * kp + 2, ccols],
                                    tl[:, 2 * kp:2 * kp + 2, cs],
                                    start=(ti == 0 and kp == 0),
                                    stop=(ti == 2 and kp == NLKV // 2 - 1),
                                    perf_mode=DR_MODE,
                                )

                    for h in range(4):
                        ps = alt_ps(h)
                        kv_chain(ps, wuk_t, wukr_t,
                                 slice(128 * h, 128 * (h + 1)))
                        nc.scalar.activation(
                            kT8_t[:, h, 1, cs], ps[:], AF.Copy,
                            scale=SQ8 * UP_UNSCALE)
                    for pr in range(2):
                        ps = alt_ps(pr)
                        kv_chain(ps, wkr_t, wkrr_t,
                                 slice(128 * pr, 128 * (pr + 1)))
                        he, ho = 2 * pr, 2 * pr + 1
                        rope_pair(
                            nc, p1tmp, [
                                (kT8_t[0:64, he, 0, cs], slice(0, 64)),
                                (kT8_t[64:128, ho, 0, cs], slice(64, 128)),
                            ], ps, c4s[:], s4s[:])
                    for stl in range(4):
                        st = 4 * sb + stl
                        stc = slice(512 * sb + 128 * stl,
                                    512 * sb + 128 * (stl + 1))
                        ps = alt_ps(stl)
                        terms = ((kvlat_t, wuv_t), (kvlatr_t, wuv_t),
                                 (kvlat_t, wuvr_t))
                        for ti, (tl, tw) in enumerate(terms):
                            for kp in range(NLKV // 2):
                                nc.tensor.matmul(
                                    ps[:], tl[:, 2 * kp:2 * kp + 2, stc],
                                    tw[:, 2 * kp:2 * kp + 2, :],
                                    start=(ti == 0 and kp == 0),
                                    stop=(ti == 2 and kp == NLKV // 2 - 1),
                                    perf_mode=DR_MODE,
                                )
                        # x SO so the fp8 split of attention outputs uses
                        # fp8 normal range (unscaled at the yT stage)
                        nc.scalar.activation(v_t[:, st, :], ps[:], AF.Copy,
                                             scale=SO * UP_UNSCALE)

            if debug:
                nc.sync.dma_start(dbg_qlat_d[:], qlat_t[:])
                nc.sync.dma_start(dbg_kvlat_d[:], kvlat_t[:])
                nc.sync.dma_start(dbg_v_d[:], v_t[:])
                nc.gpsimd.dma_start(dbg_kT8_d[:], kT8_t[:])

            # ---------------- P2: attention + W_o ----------------
            PHASE_MARKS["P2"] = nc.next_id()
            with (
                tc.tile_pool(name="p2exp", bufs=5) as p2exp,
                tc.tile_pool(name="ps_s", bufs=2, space="PSUM") as ps_sp,
                tc.tile_pool(name="p2acc", bufs=2) as p2acc,
                tc.tile_pool(name="p2acc1", bufs=2) as p2acc1,
                tc.tile_pool(name="p2out", bufs=2) as p2out,
                tc.tile_pool(name="p2y", bufs=4) as p2y,
            ):
                def emit_wo(outs, cs, dts=range(NDT), alt=False):
                    o8, o8r = outs
                    ystage = None
                    for dt in dts:
                        dsl = slice(128 * dt, 128 * (dt + 1))
                        # the final (non-interleaved) call alternates PSUM
                        # pools for 4-bank pipelining against the ystage drain
                        ps_y = alt_ps(dt if alt else 0)
                        for j in (0, 1):
                            hp = slice(2 * j, 2 * j + 2)
                            for ti, (w_s, o_s) in enumerate(
                                ((wo_t, o8), (wor_t, o8), (wo_t, o8r))
                            ):
                                nc.tensor.matmul(
                                    ps_y[:], w_s[:, hp, dsl], o_s[:, hp, :],
                                    start=(j == 0 and ti == 0),
                                    stop=(j == 1 and ti == 2),
                                    perf_mode=DR_MODE,
                                )
                        # pair two d-tiles per ystage buffer and yT store to
                        # halve the store count (the rows are DRAM-adjacent)
                        if ystage is None:
                            ystage = p2y.tile([128, 2, 512], F16, tag="y")
                        half = ystage[:, dt % 2, :]
                        if dt % 2 == 0:
                            nc.vector.tensor_scalar(
                                half, ps_y[:], Y_UNSCALE, None,
                                mybir.AluOpType.mult)
                        else:
                            nc.scalar.activation(
                                half, ps_y[:], AF.Copy, scale=Y_UNSCALE)
                            nc.sync.dma_start(
                                col3(yT_d[128 * (dt - 1):128 * (dt + 1), cs]),
                                ystage[:])
                            ystage = None

                prev_out = None
                prev_cs = None
                q8_next = None
                for qb in range(n_sb):
                    cs = slice(512 * qb, 512 * (qb + 1))
                    q8 = (q8_0, q8_1)[qb] if qb < 2 else q8_next

                    def q8_mov(h):
                        base = 3 * (h // 2)
                        if h % 2 == 0:
                            return q8[:, base:base + 2, :]
                        return q8[:, base:base + 3:2, :]

                    o8 = p2out.tile([128, 4, 512], F8, tag="o8",
                                    name=f"o8_{qb}")
                    o8r = p2out.tile([128, 4, 512], F8, tag="o8r",
                                     name=f"o8r_{qb}")
                    for h in range(4):
                        nkt = 4 * (qb + 1)
                        npair = nkt // 2
                        ps_o = ps_op.tile([128, 512], F32, tag="pv")
                        dacc = p2acc.tile([128, 1024], F16, tag="dacc")

                        def emit_pv(exp_pair, pk, npair, ps_o=ps_o, h=h):
                            for j in (0, 1):
                                kt = 2 * pk + j
                                nc.tensor.matmul(
                                    ps_o[:],
                                    v_t[:, kt, 128 * h:128 * (h + 1)],
                                    exp_pair[:, 512 * j:512 * (j + 1)],
                                    start=(kt == 0), stop=(kt == 2 * npair - 1),
                                )

                        pend = []   # (exp pair tile, pk) one pair behind
                        for pk in range(npair):
                            ps_s = ps_sp.tile([128, 1024], F32, tag="scores")
                            for j in (0, 1):
                                kt = 2 * pk + j
                                ks = slice(128 * kt, 128 * (kt + 1))
                                delta = 128 * kt - 512 * qb
                                diag = delta >= 0
                                half = ps_s[:, 512 * j:512 * (j + 1)]
                                nc.tensor.matmul(
                                    half, kT8_t[:, h, :, ks], q8_mov(h),
                                    start=True, stop=not diag,
                                    perf_mode=DR_MODE,
                                )
                                if diag:
                                    nc.tensor.matmul(
                                        half, id_t[:],
                                        mask_t[:, :, 384 - delta:896 - delta],
                                        start=False, stop=True,
                                        perf_mode=DR_MODE,
                                    )
                            exp_t = p2exp.tile([128, 1024], F16, tag="exp")
                            nc.scalar.activation(
                                exp_t[:], ps_s[:], AF.Exp, scale=EXP_SCALE)
                            # two alternating accumulators halve the serial
                            # add-chain latency on DVE
                            half = dacc[:, 512 * (pk % 2):512 * (pk % 2) + 512]
                            if pk < 2:
                                nc.vector.tensor_add(
                                    half, exp_t[:, 0:512], exp_t[:, 512:1024])
                            else:
                                nc.vector.tensor_add(
                                    half, half, exp_t[:, 0:512])
                                nc.vector.tensor_add(
                                    half, half, exp_t[:, 512:1024])
                            pend.append((exp_t, pk))
                            if len(pend) > 2:
                                emit_pv(*pend.pop(0), npair)
                        for e in pend:
                            emit_pv(*e, npair)
                        dfold = p2acc1.tile([128, 512], F16, tag="dfold")
                        if npair > 1:
                            nc.vector.tensor_add(
                                dfold[:], dacc[:, 0:512], dacc[:, 512:1024])
                        else:
                            nc.vector.tensor_copy(dfold[:], dacc[:, 0:512])
                        dred = p2acc1.tile([128, 512], F32, tag="dred")
                        nc.gpsimd.partition_all_reduce(
                            dred[:], dfold[:], 128, bass_isa.ReduceOp.add)
                        rsb = p2tmp.tile([128, 512], F32, tag="rsbd")
                        nc.vector.reciprocal(rsb[:], dred[:])
                        o16 = p2tmp.tile([128, 512], F16, tag="o16")
                        nc.vector.tensor_mul(o16[:], ps_o[:], rsb[:])
                        nc.vector.tensor_copy(o8[:, h, :], o16[:])
                        nc.vector.tensor_sub(o8r[:, h, :], o16[:], o8[:, h, :])
                        # interleave W_o d-tiles of the previous q-block so
                        # the in-order PE stream has fill work during this
                        # block's exp-latency stalls
                        if prev_out is not None:
                            emit_wo(prev_out, prev_cs,
                                    range(4 * h, 4 * (h + 1)))
                        # interleave the next block's q projections mid-stream
                        # instead of serializing them at the block boundary
                        if h == 1 and 2 <= qb + 1 < n_sb:
                            q8_next = compute_q8(qb + 1)
                    prev_out, prev_cs = (o8, o8r), cs
                emit_wo(prev_out, prev_cs, alt=True)
            p2w_stack.close()
            persist_stack.close()

    nc.compile()
    return nc


def host_prep(inputs, S=S_FULL):
    """Build the 8 per-core input maps from the full problem inputs."""
    FP8 = ml_dtypes.float8_e4m3

    def to8(a):
        return np.ascontiguousarray(a).astype(FP8)

    def split8(a, scale):
        hi = (a * scale).astype(FP8)
        lo = (a * scale - hi.astype(np.float32)).astype(FP8)
        return hi, lo

    x = np.asarray(inputs["x"], np.float32)
    cosT = np.asarray(inputs["rope_cos"], np.float32).T
    sinT = np.asarray(inputs["rope_sin"], np.float32).T
    c4 = np.concatenate([cosT, cosT, cosT, cosT], 0) * SQ8
    s4 = np.concatenate([-sinT, sinT, -sinT, sinT], 0) * SQ8
    c4 = np.ascontiguousarray(c4).astype(np.float16)
    s4 = np.ascontiguousarray(s4).astype(np.float16)
    qw = np.asarray(inputs["q_norm_w"], np.float32)
    kvw = np.asarray(inputs["kv_norm_w"], np.float32)
    W_uq = np.asarray(inputs["W_uq"], np.float32) * qw[:, None]
    W_qr = np.asarray(inputs["W_qr"], np.float32) * qw[:, None]
    W_uk = np.asarray(inputs["W_uk"], np.float32) * kvw[:, None]
    W_kr = np.asarray(inputs["W_kr"], np.float32) * kvw[:, None]
    W_uv = np.asarray(inputs["W_uv"], np.float32) * kvw[:, None]
    W_o = np.asarray(inputs["W_o"], np.float32)
    W_dq = np.asarray(inputs["W_dq"], np.float32)
    W_dkv = np.asarray(inputs["W_dkv"], np.float32)

    wdq8, wdq8r = split8(W_dq, BW)
    wdkv8, wdkv8r = split8(W_dkv, BW)
    wo8_full, wo8r_full = split8(W_o, BWO)
    wuq8, wuq8r = split8(W_uq, BW)
    wqr8, wqr8r = split8(W_qr, BW)
    wuk8, wuk8r = split8(W_uk, BW)
    wkr8, wkr8r = split8(W_kr, BW)
    wuv8, wuv8r = split8(W_uv, BW)

    # mask table: plane 0 = {0, -240} causal pattern, plane 1 = 0
    cgrid = np.arange(896)[None, :] - 384
    igrid = np.arange(128)[:, None]
    mask8 = np.zeros((128, 2, 896), np.float32)
    mask8[:, 0, :] = np.where(cgrid >= igrid, 0.0, -240.0)
    mask8 = mask8.astype(FP8)
    id8 = np.zeros((128, 2, 128), np.float32)
    id8[:, 0, :] = MASK_ID * np.eye(128, dtype=np.float32)
    id8 = id8.astype(FP8)
    ones8 = np.ones((128, 1), np.float32).astype(FP8)

    in_maps = []
    for c in range(NCORES):
        b, g = c // 4, c % 4
        hs = slice(4 * g * DN, 4 * (g + 1) * DN)
        hr = slice(4 * g * DRR, 4 * (g + 1) * DRR)
        xT = np.ascontiguousarray(x[b].T)
        x8, x8r = split8(xT, AX)
        in_maps.append(dict(
            x8=x8, x8r=x8r,
            xkv8=np.ascontiguousarray(x8[:, 512 * g:512 * (g + 1)]),
            xkv8r=np.ascontiguousarray(x8r[:, 512 * g:512 * (g + 1)]),
            W_dq8=wdq8, W_dq8r=wdq8r,
            W_dkv8=wdkv8, W_dkv8r=wdkv8r,
            Wuq8=np.ascontiguousarray(wuq8[:, hs]),
            Wuq8r=np.ascontiguousarray(wuq8r[:, hs]),
            Wqr8=np.ascontiguousarray(wqr8[:, hr]),
            Wqr8r=np.ascontiguousarray(wqr8r[:, hr]),
            Wuk8=np.ascontiguousarray(wuk8[:, hs]),
            Wuk8r=np.ascontiguousarray(wuk8r[:, hs]),
            Wkr8=np.ascontiguousarray(wkr8[:, hr]),
            Wkr8r=np.ascontiguousarray(wkr8r[:, hr]),
            Wuv8=np.ascontiguousarray(wuv8[:, hs]),
            Wuv8r=np.ascontiguousarray(wuv8r[:, hs]),
            Wo8=np.ascontiguousarray(wo8_full[512 * g:512 * (g + 1), :]),
            Wo8r=np.ascontiguousarray(wo8r_full[512 * g:512 * (g + 1), :]),
            c4=c4, s4=s4, mask8=mask8, id8=id8, ones8=ones8,
        ))
    return in_maps


_NC_CACHE = {}


def kernel(**inputs) -> np.ndarray:
    S = np.asarray(inputs["x"]).shape[1]
    if S not in _NC_CACHE:
        _NC_CACHE[S] = build_nc(S)
    nc = _NC_CACHE[S]
    in_maps = host_prep(inputs, S)
    res = run_bass_kernel_spmd(nc, in_maps, core_ids=list(range(NCORES)))
    y = np.empty((B, S, D), np.float32)
    for b in range(B):
        acc = res.results[4 * b]["yT"].astype(np.float32)
        for g in range(1, 4):
            acc = acc + res.results[4 * b + g]["yT"].astype(np.float32)
        y[b] = acc.T
    return y


# revision 78
# speedup vs baseline: 2.0220x; 1.0050x over previous
"""MultiHeadLatentAttention (MLA) Trainium2 Bass kernel, v2.

Problem: B=2, S=2048, D=2048, H=16 heads, d_nope=128, d_rope=64, d_head=128,
q_latent=768, kv_latent=512. Causal attention, rmsnorm'd latents, half-dim RoPE.

Sharding (8 cores): core c handles batch b=c//4 and head group g=c%4 (4 heads).
The small latent down-projections are replicated within each batch group;
W_uq/W_qr/W_uk/W_kr/W_uv are column-sharded by head; W_o row-sharded; the
4 partial outputs per batch are summed on the host.

Precision/engine plan (metric = InstructionCostModel timeline; baseline
559461 ns -> 276691 ns, rel err 1.32e-2 < 2e-2):
  - scores matmul in fp8e4m3 with MatmulPerfMode.DoubleRow: the nope(128) and
    zero-padded rope(64) contractions are packed as the two DoubleRow k-tiles,
    so each 128x512 score tile costs 256 PE cycles instead of 1024.
    End-to-end error from quantizing qn/kn/qr/kr to fp8 measured 1.33e-2;
    every other fp8 stage below is a lossless-ish residual split adding <1e-3.
  - causal mask added in the same PSUM group by a fp8 DoubleRow matmul of
    60*I against a {0,-240} mask table (-14400 pre-scale -> exp()=2e-9).
  - all projections (down, up, W_o) run as 3-term fp8 DoubleRow residual
    splits (W8@x8 + W8r@x8 + W8@x8r, dropping the second-order W8r@x8r):
    fp8 PE speed (0.5 cycles/row) at fp16-like accuracy. Weights are split
    on the host; x arrives pre-split; latents are split once at the rmsnorm
    multiply; attention outputs are split on DVE before W_o.
  - kv down-projection is S-sharded: each core computes only its own 512-col
    block and the fp8 hi+lo latent pair is AllGathered (2.1MB) on the
    collective cores, overlapped with the replicated q down-projection.
    The collective's SWDGE upload/downloads live on the otherwise-empty Pool
    queue so its in-order waits block nothing.
  - PV in fp16 (exp quantization to fp8 would cost ~3% output error).
  - softmax: exp on ACT (fp16 out), denominator via two alternating fp16
    accumulators on DVE + partition_all_reduce on Pool + DVE reciprocal --
    no Ln anywhere, so a single act-table load (was 49 reloads/63us).
  - rmsnorm rsqrt = ACT Sqrt(DVE reciprocal(mean sq)); the partition
    broadcast of the per-token scale is a PE outer product (ones x row) so
    P0 keeps the Pool queue empty for the collective.
  - W_o, latents, k^T, v stay resident in SBUF; W_o(prev block) d-tiles and
    q8(next block) are interleaved into the in-order PE stream as fill work
    during exp-latency stalls; yT stores pair two d-tiles per DMA.
"""
import math
import os
from contextlib import ExitStack

import numpy as np
import ml_dtypes

import concourse.bass as bass
import concourse.bass_isa as bass_isa
import concourse.bacc as bacc
import concourse.mybir as mybir
import concourse.tile as tile
from concourse.bass_utils import run_bass_kernel_spmd

F32 = mybir.dt.float32
F32R = mybir.dt.float32r
F16 = mybir.dt.float16
F8 = mybir.dt.float8e4
AF = mybir.ActivationFunctionType
DR_MODE = mybir.MatmulPerfMode.DoubleRow

B, S_FULL, D = 2, 2048, 2048
H, DN, DRR, DH = 16, 128, 64, 128
QL, KVL = 768, 512
EPS = 1e-6
SCALE = 1.0 / math.sqrt(DH)
NCORES = 8
NKT = D // 128          # 16 contraction tiles over D
NKP = NKT // 2          # 8 DoubleRow pairs
NLQ = QL // 128         # 6
NLKV = KVL // 128       # 4
NDT = D // 128          # 16 output D tiles

# fp8 scaling for the residual-split down-projection: x' = x*AX, W' = W*BW
# so both the quantized tensors and their residuals stay in fp8 normal range.
AX = 32.0
BW = 256.0
PSUM_UNSCALE = 1.0 / (AX * BW)
# residual-split up-projections: latents x SL (folded into the rsqrt), and
# the up-projection weights x BW
SL = 16.0
UP_UNSCALE = 1.0 / (SL * BW)
# score operand quantization scale (qn8 = 8*qn etc.)
SQ8 = 8.0
EXP_SCALE = SCALE / (SQ8 * SQ8)
MASK_ID = 60.0          # mask matmul: 60 * (-240) * 1 plane = -14400 pre-scale
# W_o fp8 residual split: out tiles scaled x32 (folded into v), W_o x1024
SO = 32.0
BWO = 1024.0
Y_UNSCALE = 1.0 / (SO * BWO)

PHASE_MARKS = {}


def build_nc(S=S_FULL):
    assert S % 512 == 0
    n_sb = S // 512
    n_st = S // 128
    PHASE_MARKS.clear()

    nc = bacc.Bacc("TRN2", target_bir_lowering=False, debug=False,
                   num_devices=NCORES)

    x8_d = nc.dram_tensor("x8", [D, S], F8, kind="ExternalInput")
    x8r_d = nc.dram_tensor("x8r", [D, S], F8, kind="ExternalInput")
    # per-core own-block column slice of x, for the S-sharded kv down-proj
    xkv8_d = nc.dram_tensor("xkv8", [D, 512], F8, kind="ExternalInput")
    xkv8r_d = nc.dram_tensor("xkv8r", [D, 512], F8, kind="ExternalInput")
    wdq_d = nc.dram_tensor("W_dq8", [D, QL], F8, kind="ExternalInput")
    wdqr_d = nc.dram_tensor("W_dq8r", [D, QL], F8, kind="ExternalInput")
    wdkv_d = nc.dram_tensor("W_dkv8", [D, KVL], F8, kind="ExternalInput")
    wdkvr_d = nc.dram_tensor("W_dkv8r", [D, KVL], F8, kind="ExternalInput")
    wuq_d = nc.dram_tensor("Wuq8", [QL, 512], F8, kind="ExternalInput")
    wuqr_d = nc.dram_tensor("Wuq8r", [QL, 512], F8, kind="ExternalInput")
    wqr_d = nc.dram_tensor("Wqr8", [QL, 256], F8, kind="ExternalInput")
    wqrr_d = nc.dram_tensor("Wqr8r", [QL, 256], F8, kind="ExternalInput")
    wuk_d = nc.dram_tensor("Wuk8", [KVL, 512], F8, kind="ExternalInput")
    wukr_d = nc.dram_tensor("Wuk8r", [KVL, 512], F8, kind="ExternalInput")
    wkr_d = nc.dram_tensor("Wkr8", [KVL, 256], F8, kind="ExternalInput")
    wkrr_d = nc.dram_tensor("Wkr8r", [KVL, 256], F8, kind="ExternalInput")
    wuv_d = nc.dram_tensor("Wuv8", [KVL, 512], F8, kind="ExternalInput")
    wuvr_d = nc.dram_tensor("Wuv8r", [KVL, 512], F8, kind="ExternalInput")
    wo_d = nc.dram_tensor("Wo8", [512, D], F8, kind="ExternalInput")
    wor_d = nc.dram_tensor("Wo8r", [512, D], F8, kind="ExternalInput")
    c4_d = nc.dram_tensor("c4", [128, S], F16, kind="ExternalInput")
    s4_d = nc.dram_tensor("s4", [128, S], F16, kind="ExternalInput")
    mask_d = nc.dram_tensor("mask8", [128, 2, 896], F8, kind="ExternalInput")
    id_d = nc.dram_tensor("id8", [128, 2, 128], F8, kind="ExternalInput")
    ones_d = nc.dram_tensor("ones8", [128, 1], F8, kind="ExternalInput")
    yT_d = nc.dram_tensor("yT", [D, S], F16, kind="ExternalOutput")
    debug = bool(int(os.environ.get("MLA_DEBUG", "0")))
    if debug:
        dbg_qlat_d = nc.dram_tensor("dbg_qlat", [128, NLQ, S], F16,
                                    kind="ExternalOutput")
        dbg_kvlat_d = nc.dram_tensor("dbg_kvlat", [128, NLKV, S], F16,
                                     kind="ExternalOutput")
        dbg_kT8_d = nc.dram_tensor("dbg_kT8", [128, 4, 2, S], F16,
                                   kind="ExternalOutput")
        dbg_v_d = nc.dram_tensor("dbg_v", [128, n_st, 512], F16,
                                 kind="ExternalOutput")

    def col3(dram_ap, p=128):
        # [R, C] dram slice -> [128, R//128, C] tiled AP
        return dram_ap.rearrange("(t p) c -> p t c", p=p)

    def rope_pair(nc, pool, outs, ps, c4s, s4s):
        """Half-dim rope on a 2-head pair tile [128, 512] in PSUM.

        out = ps * c4 + shuf(ps) * s4, shuf swaps 32-blocks within each 64.
        Stages through fp16 SBUF so the DVE muls run in 2x mode. `outs` is a
        list of (out_ap, pslice) fp8 destinations.
        """
        rs = pool.tile([128, 512], F16, tag="rope_rs")
        nc.scalar.activation(rs[:], ps[:], AF.Copy, scale=UP_UNSCALE)
        shuf = pool.tile([128, 512], F16, tag="rope_shuf")
        nc.vector.tensor_copy(shuf[0:32, :], rs[32:64, :])
        nc.vector.tensor_copy(shuf[32:64, :], rs[0:32, :])
        nc.vector.tensor_copy(shuf[64:96, :], rs[96:128, :])
        nc.vector.tensor_copy(shuf[96:128, :], rs[64:96, :])
        t1 = pool.tile([128, 512], F16, tag="rope_t1")
        nc.vector.tensor_mul(t1[:], rs[:], c4s)
        nc.vector.tensor_mul(shuf[:], shuf[:], s4s)
        for out_ap, psl in outs:
            nc.vector.tensor_add(out_ap, t1[psl, :], shuf[psl, :])

    with tile.TileContext(nc) as tc:
        with (
            tc.tile_pool(name="const", bufs=1) as constp,
            tc.tile_pool(name="ps_mm", bufs=2, space="PSUM") as ps_mm,
            tc.tile_pool(name="ps_o", bufs=2, space="PSUM") as ps_op,
        ):
            def alt_ps(i):
                if i % 2 == 0:
                    return ps_mm.tile([128, 512], F32, tag="mm", name="ps")
                return ps_op.tile([128, 512], F32, tag="pv", name="ps")

            mask_t = constp.tile([128, 2, 896], F8)
            id_t = constp.tile([128, 2, 128], F8)
            ones_t = constp.tile([128, 1], F8)
            ones_row = constp.tile([1, 128], F16)
            nc.vector.memset(ones_row[:], 1.0)

            # persistent SBUF state
            persist_stack = ExitStack()
            persist = persist_stack.enter_context(
                tc.tile_pool(name="persist", bufs=1))
            # kT8: per head (rope_padded, nope) planes, fp8 stationary
            kT8_t = persist.tile([128, 4, 2, S], F8)
            v_t = persist.tile([128, n_st, 512], F16)
            wo_t = persist.tile([128, 4, D], F8)
            wor_t = persist.tile([128, 4, D], F8)
            qlat_t = persist.tile([128, NLQ, S], F8)
            qlatr_t = persist.tile([128, NLQ, S], F8)
            kvlat_t = persist.tile([128, NLKV, S], F8)
            kvlatr_t = persist.tile([128, NLKV, S], F8)

            # ---------------- P0: down-projections + rmsnorm ----------------
            PHASE_MARKS["P0"] = nc.next_id()
            p0_stack = ExitStack()
            p0w = p0_stack.enter_context(tc.tile_pool(name="p0w", bufs=1))
            wdq_t = p0w.tile([128, NKT, QL], F8)
            wdqr_t = p0w.tile([128, NKT, QL], F8)
            wdkv_t = p0w.tile([128, NKT, KVL], F8)
            wdkvr_t = p0w.tile([128, NKT, KVL], F8)
            with (
                tc.tile_pool(name="p0x", bufs=2) as p0x,
                tc.tile_pool(name="p0raw", bufs=2) as p0raw,
                tc.tile_pool(name="p0sq", bufs=2) as p0sq,
                tc.tile_pool(name="p0own", bufs=1) as p0own,
                tc.tile_pool(name="p0tmp", bufs=2) as p0tmp,
                tc.tile_pool(name="p0dram", bufs=1, space="DRAM") as p0dram,
                tc.tile_pool(name="ps_den", bufs=1, space="PSUM") as ps_denp,
                tc.tile_pool(name="ps_p0", bufs=2, space="PSUM") as ps_p0,
            ):
                def alt3_ps(i):
                    if i % 3 == 2:
                        return ps_p0.tile([128, 512], F32, tag="p0", name="ps")
                    return alt_ps(i % 3)

                # zero the pad halves of the rope planes once (rope data for
                # even heads lives at partitions 0:64, odd heads at 64:128)
                for h in range(4):
                    lo = 0 if h % 2 else 64
                    nc.gpsimd.memset(kT8_t[lo:lo + 64, h, 0, :], 0.0)

                def down_proj(latname, w_t, wr_t, nl, xh, xrh, dest8,
                              dest8r, rawp=None, sqp=None, terms3=True):
                    raw = (rawp or p0raw).tile(
                        [128, nl, 512], F16, tag=f"raw{latname}",
                        name=f"raw{latname}")
                    sq = (sqp or p0sq).tile(
                        [128, nl, 512], F8, tag=f"sq{latname}",
                        name=f"sq{latname}")
                    ps_ss = ps_denp.tile([1, 512], F32, tag="den")
                    for lt in range(nl):
                        ps = alt3_ps(lt)
                        lsl = slice(128 * lt, 128 * (lt + 1))
                        terms = ((w_t, xh), (wr_t, xh), (w_t, xrh)) \
                            if terms3 else ((w_t, xh), (w_t, xrh))
                        for term_w, term_x in terms:
                            first = term_w is w_t and term_x is xh
                            last = term_x is xrh
                            for kp in range(NKP):
                                nc.tensor.matmul(
                                    ps[:],
                                    term_w[:, 2 * kp:2 * kp + 2, lsl],
                                    term_x[:, 2 * kp:2 * kp + 2, :],
                                    start=(first and kp == 0),
                                    stop=(last and kp == NKP - 1),
                                    perf_mode=DR_MODE,
                                )
                        nc.scalar.activation(
                            raw[:, lt, :], ps[:], AF.Copy, scale=PSUM_UNSCALE)
                        nc.scalar.activation(
                            sq[:, lt, :], ps[:], AF.Square, scale=PSUM_UNSCALE)
                        nc.tensor.matmul(
                            ps_ss[:], ones_t[:], sq[:, lt, :],
                            start=(lt == 0), stop=(lt == nl - 1),
                        )
                    # rsqrt of mean square: sqrt(1/m) via DVE reciprocal
                    # + ACT Sqrt (Copy/Square live in the sqrt act table,
                    # so P0 needs no act-table reloads)
                    mrow = p0tmp.tile([1, 512], F32, tag="mrow")
                    nc.vector.tensor_scalar(
                        mrow[:], ps_ss[:], 1.0 / (128 * nl), EPS,
                        mybir.AluOpType.mult, mybir.AluOpType.add)
                    rrec = p0tmp.tile([1, 512], F32, tag="rrec")
                    nc.vector.reciprocal(rrec[:], mrow[:])
                    rrow = p0tmp.tile([1, 512], F16, tag="rrow")
                    # scale=SL^2 folds the latent fp8 scale into the rsqrt
                    nc.scalar.activation(rrow[:], rrec[:], AF.Sqrt,
                                         scale=SL * SL)
                    # broadcast across partitions via a PE outer product so P0
                    # keeps the Pool queue empty (the AllGather blocks it)
                    ps_bc = ps_denp.tile([128, 512], F32, tag="bc")
                    nc.tensor.matmul(ps_bc[:], ones_row[:], rrow[:],
                                     start=True, stop=True)
                    rsb = p0tmp.tile([128, 512], F16, tag="rsb")
                    nc.scalar.copy(rsb[:], ps_bc[:])
                    for lt in range(nl):
                        tmp = p0tmp.tile([128, 512], F16, tag="ntmp")
                        nc.vector.tensor_mul(tmp[:], raw[:, lt, :], rsb[:])
                        nc.vector.tensor_copy(dest8(lt), tmp[:])
                        nc.vector.tensor_sub(dest8r(lt), tmp[:], dest8(lt))

                # --- kv down-proj for this core's own block only; the other
                # blocks arrive via an AllGather of the fp16 latents that
                # overlaps with the (replicated) q down-projection.
                xkvh = p0x.tile([128, NKT, 512], F8, tag="x8", name="xkv8")
                xkvrh = p0x.tile([128, NKT, 512], F8, tag="x8r", name="xkv8r")
                nc.sync.dma_start(xkvh[:], col3(xkv8_d))
                nc.sync.dma_start(ones_t[:], ones_d[:])
                nc.sync.dma_start(wdkv_t[:], col3(wdkv_d[:]))
                nc.sync.dma_start(wdkvr_t[:], col3(wdkvr_d[:]))
                nc.sync.dma_start(xkvrh[:], col3(xkv8r_d))
                kvlat_own = p0own.tile([128, NLKV, 512], F8, tag="kvown",
                                       name="kvlat_own")
                kvlatr_own = p0own.tile([128, NLKV, 512], F8, tag="kvownr",
                                        name="kvlatr_own")
                down_proj("kv", wdkv_t, wdkvr_t, NLKV, xkvh, xkvrh,
                          lambda lt: kvlat_own[:, lt, :],
                          lambda lt: kvlatr_own[:, lt, :],
                          rawp=p0own, sqp=p0own)
                # the whole collective path lives on the (otherwise idle)
                # Pool queue: its in-order waits must not block the SP/ACT
                # DMA queues or the ACT compute stream
                kv_own_d = p0dram.tile([2 * KVL, 512], F8, name="kv_own")
                kv_all_d = p0dram.tile([8 * KVL, 512], F8, name="kv_all")
                nc.gpsimd.dma_start(col3(kv_own_d[0:KVL, :]), kvlat_own[:])
                nc.gpsimd.dma_start(col3(kv_own_d[KVL:2 * KVL, :]),
                                    kvlatr_own[:])
                nc.gpsimd.collective_compute(
                    "AllGather",
                    mybir.AluOpType.bypass,
                    replica_groups=[[0, 1, 2, 3], [4, 5, 6, 7]],
                    ins=[kv_own_d[:]],
                    outs=[kv_all_d[:]],
                )
                for c in range(4):
                    base = c * 2 * KVL
                    nc.gpsimd.dma_start(
                        kvlat_t[:, :, 512 * c:512 * (c + 1)],
                        col3(kv_all_d[base:base + KVL, :]))
                    nc.gpsimd.dma_start(
                        kvlatr_t[:, :, 512 * c:512 * (c + 1)],
                        col3(kv_all_d[base + KVL:base + 2 * KVL, :]))

                # --- replicated q down-projection over all blocks
                for sb in range(n_sb):
                    cs = slice(512 * sb, 512 * (sb + 1))
                    xh = p0x.tile([128, NKT, 512], F8, tag="x8", name=f"x8_{sb}")
                    xrh = p0x.tile([128, NKT, 512], F8, tag="x8r",
                                   name=f"x8r_{sb}")
                    nc.sync.dma_start(xh[:], col3(x8_d[:, cs]))
                    if sb == 0:
                        nc.sync.dma_start(wdq_t[:], col3(wdq_d[:]))
                        nc.sync.dma_start(wdqr_t[:], col3(wdqr_d[:]))
                    nc.sync.dma_start(xrh[:], col3(x8r_d[:, cs]))
                    if sb == 0:
                        nc.sync.dma_start(mask_t[:], mask_d[:])
                        nc.sync.dma_start(id_t[:], id_d[:])
                    if sb == 3:
                        # W_o resident load (needed only from P2, and after
                        # the last x chunks so it never delays them)
                        nc.sync.dma_start(wo_t[:], col3(wo_d[:]))
                        nc.sync.dma_start(wor_t[:], col3(wor_d[:]))
                    down_proj("q", wdq_t, wdqr_t, NLQ, xh, xrh,
                              lambda lt, cs=cs: qlat_t[:, lt, cs],
                              lambda lt, cs=cs: qlatr_t[:, lt, cs])
            p0_stack.close()

            # ---------------- P1: k/v up-projections ----------------
            PHASE_MARKS["P1"] = nc.next_id()
            p2w_stack = ExitStack()
            p2w = p2w_stack.enter_context(tc.tile_pool(name="p2w", bufs=1))
            p2q = p2w_stack.enter_context(tc.tile_pool(name="p2q", bufs=2))
            p2tmp = p2w_stack.enter_context(tc.tile_pool(name="p2tmp", bufs=2))
            wuq_t = p2w.tile([128, NLQ, 512], F8)
            wuqr_t = p2w.tile([128, NLQ, 512], F8)
            wqr_t = p2w.tile([128, NLQ, 256], F8)
            wqrr_t = p2w.tile([128, NLQ, 256], F8)

            def compute_q8(qb):
                """q up-projection + rope for one q-block into a fp8 moving
                tile with slots (qr01, qn0, qn1, qr23, qn2, qn3)."""
                cs = slice(512 * qb, 512 * (qb + 1))
                c4s = p2tmp.tile([128, 512], F16, tag="c4")
                s4s = p2tmp.tile([128, 512], F16, tag="s4")
                nc.sync.dma_start(c4s[:], c4_d[:, cs])
                nc.sync.dma_start(s4s[:], s4_d[:, cs])
                q8 = p2q.tile([128, 6, 512], F8, tag="q8", name=f"q8_{qb}")

                def up_chain(ps, w_t, wr_t, ccols, np_, lat=qlat_t,
                             latr=qlatr_t):
                    terms = ((w_t, lat), (wr_t, lat), (w_t, latr))
                    for ti, (tw, tl) in enumerate(terms):
                        for qp in range(np_):
                            nc.tensor.matmul(
                                ps[:], tw[:, 2 * qp:2 * qp + 2, ccols],
                                tl[:, 2 * qp:2 * qp + 2, cs],
                                start=(ti == 0 and qp == 0),
                                stop=(ti == 2 and qp == np_ - 1),
                                perf_mode=DR_MODE,
                            )

                for h in range(4):
                    ps = alt_ps(h)
                    up_chain(ps, wuq_t, wuqr_t,
                             slice(128 * h, 128 * (h + 1)), NLQ // 2)
                    slot = (1, 2, 4, 5)[h]
                    nc.scalar.activation(
                        q8[:, slot, :], ps[:], AF.Copy,
                        scale=SQ8 * UP_UNSCALE)
                for pr in range(2):
                    ps = alt_ps(pr)
                    up_chain(ps, wqr_t, wqrr_t,
                             slice(128 * pr, 128 * (pr + 1)), NLQ // 2)
                    rope_pair(nc, p2tmp,
                              [(q8[:, 3 * pr, :], slice(0, 128))],
                              ps, c4s[:], s4s[:])
                return q8

            with (
                tc.tile_pool(name="p1w", bufs=1) as p1w,
                tc.tile_pool(name="p1tmp", bufs=2) as p1tmp,
            ):
                wuk_t = p1w.tile([128, NLKV, 512], F8)
                wukr_t = p1w.tile([128, NLKV, 512], F8)
                wkr_t = p1w.tile([128, NLKV, 256], F8)
                wkrr_t = p1w.tile([128, NLKV, 256], F8)
                wuv_t = p1w.tile([128, NLKV, 512], F8)
                wuvr_t = p1w.tile([128, NLKV, 512], F8)
                nc.sync.dma_start(wuk_t[:], col3(wuk_d[:]))
                nc.sync.dma_start(wukr_t[:], col3(wukr_d[:]))
                nc.sync.dma_start(wuq_t[:], col3(wuq_d[:]))
                nc.sync.dma_start(wuqr_t[:], col3(wuqr_d[:]))
                nc.sync.dma_start(wqr_t[:], col3(wqr_d[:]))
                nc.sync.dma_start(wqrr_t[:], col3(wqrr_d[:]))
                # q8 for blocks 0 and 1 first: their inputs are ready
                # before the AllGathered kv latents land, filling the P1
                # entry stall with ~15us of PE work
                q8_0 = compute_q8(0)
                q8_1 = compute_q8(1)
                for sb in range(n_sb):
                    cs = slice(512 * sb, 512 * (sb + 1))
                    if sb == 0:
                        nc.sync.dma_start(wkr_t[:], col3(wkr_d[:]))
                        nc.sync.dma_start(wkrr_t[:], col3(wkrr_d[:]))
                        nc.sync.dma_start(wuv_t[:], col3(wuv_d[:]))
                        nc.sync.dma_start(wuvr_t[:], col3(wuvr_d[:]))
                    c4s = p1tmp.tile([128, 512], F16, tag="c4")
                    s4s = p1tmp.tile([128, 512], F16, tag="s4")
                    nc.sync.dma_start(c4s[:], c4_d[:, cs])
                    nc.sync.dma_start(s4s[:], s4_d[:, cs])
                    def kv_chain(ps, w_t, wr_t, ccols):
                        terms = ((w_t, kvlat_t), (wr_t, kvlat_t),
                                 (w_t, kvlatr_t))
                        for ti, (tw, tl) in enumerate(terms):
                            for kp in range(NLKV // 2):
                                nc.tensor.matmul(
                                    ps[:], tw[:, 2 * kp:2 * kp + 2, ccols],
                                    tl[:, 2 * kp:2 * kp + 2, cs],
                                    start=(ti == 0 and kp == 0),
                                    stop=(ti == 2 and kp == NLKV // 2 - 1),
                                    perf_mode=DR_MODE,
                                )

                    for h in range(4):
                        ps = alt_ps(h)
                        kv_chain(ps, wuk_t, wukr_t,
                                 slice(128 * h, 128 * (h + 1)))
                        nc.scalar.activation(
                            kT8_t[:, h, 1, cs], ps[:], AF.Copy,
                            scale=SQ8 * UP_UNSCALE)
                    for pr in range(2):
                        ps = alt_ps(pr)
                        kv_chain(ps, wkr_t, wkrr_t,
                                 slice(128 * pr, 128 * (pr + 1)))
                        he, ho = 2 * pr, 2 * pr + 1
                        rope_pair(
                            nc, p1tmp, [
                                (kT8_t[0:64, he, 0, cs], slice(0, 64)),
                                (kT8_t[64:128, ho, 0, cs], slice(64, 128)),
                            ], ps, c4s[:], s4s[:])
                    for stl in range(4):
                        st = 4 * sb + stl
                        stc = slice(512 * sb + 128 * stl,
                                    512 * sb + 128 * (stl + 1))
                        ps = alt_ps(stl)
                        terms = ((kvlat_t, wuv_t), (kvlatr_t, wuv_t),
                                 (kvlat_t, wuvr_t))
                        for ti, (tl, tw) in enumerate(terms):
                            for kp in range(NLKV // 2):
                                nc.tensor.matmul(
                                    ps[:], tl[:, 2 * kp:2 * kp + 2, stc],
                                    tw[:, 2 * kp:2 * kp + 2, :],
                                    start=(ti == 0 and kp == 0),
                                    stop=(ti == 2 and kp == NLKV // 2 - 1),
                                    perf_mode=DR_MODE,
                                )
                        # x SO so the fp8 split of attention outputs uses
                        # fp8 normal range (unscaled at the yT stage)
                        nc.scalar.activation(v_t[:, st, :], ps[:], AF.Copy,
                                             scale=SO * UP_UNSCALE)

            if debug:
                nc.sync.dma_start(dbg_qlat_d[:], qlat_t[:])
                nc.sync.dma_start(dbg_kvlat_d[:], kvlat_t[:])
                nc.sync.dma_start(dbg_v_d[:], v_t[:])
                nc.gpsimd.dma_start(dbg_kT8_d[:], kT8_t[:])

            # ---------------- P2: attention + W_o ----------------
            PHASE_MARKS["P2"] = nc.next_id()
            with (
                tc.tile_pool(name="p2exp", bufs=5) as p2exp,
                tc.tile_pool(name="ps_s", bufs=2, space="PSUM") as ps_sp,
                tc.tile_pool(name="p2acc", bufs=2) as p2acc,
                tc.tile_pool(name="p2acc1", bufs=2) as p2acc1,
                tc.tile_pool(name="p2out", bufs=2) as p2out,
                tc.tile_pool(name="p2y", bufs=4) as p2y,
            ):
                def emit_wo(outs, cs, dts=range(NDT), alt=False):
                    o8, o8r = outs
                    ystage = None
                    for dt in dts:
                        dsl = slice(128 * dt, 128 * (dt + 1))
                        # the final (non-interleaved) call alternates PSUM
                        # pools for 4-bank pipelining against the ystage drain
                        ps_y = alt_ps(dt if alt else 0)
                        for j in (0, 1):
                            hp = slice(2 * j, 2 * j + 2)
                            for ti, (w_s, o_s) in enumerate(
                                ((wo_t, o8), (wor_t, o8), (wo_t, o8r))
                            ):
                                nc.tensor.matmul(
                                    ps_y[:], w_s[:, hp, dsl], o_s[:, hp, :],
                                    start=(j == 0 and ti == 0),
                                    stop=(j == 1 and ti == 2),
                                    perf_mode=DR_MODE,
                                )
                        # pair two d-tiles per ystage buffer and yT store to
                        # halve the store count (the rows are DRAM-adjacent)
                        if ystage is None:
                            ystage = p2y.tile([128, 2, 512], F16, tag="y")
                        half = ystage[:, dt % 2, :]
                        if dt % 2 == 0:
                            nc.vector.tensor_scalar(
                                half, ps_y[:], Y_UNSCALE, None,
                                mybir.AluOpType.mult)
                        else:
                            nc.scalar.activation(
                                half, ps_y[:], AF.Copy, scale=Y_UNSCALE)
                            nc.sync.dma_start(
                                col3(yT_d[128 * (dt - 1):128 * (dt + 1), cs]),
                                ystage[:])
                            ystage = None

                prev_out = None
                prev_cs = None
                q8_next = None
                for qb in range(n_sb):
                    cs = slice(512 * qb, 512 * (qb + 1))
                    q8 = (q8_0, q8_1)[qb] if qb < 2 else q8_next

                    def q8_mov(h):
                        base = 3 * (h // 2)
                        if h % 2 == 0:
                            return q8[:, base:base + 2, :]
                        return q8[:, base:base + 3:2, :]

                    o8 = p2out.tile([128, 4, 512], F8, tag="o8",
                                    name=f"o8_{qb}")
                    o8r = p2out.tile([128, 4, 512], F8, tag="o8r",
                                     name=f"o8r_{qb}")
                    for h in range(4):
                        nkt = 4 * (qb + 1)
                        npair = nkt // 2
                        ps_o = ps_op.tile([128, 512], F32, tag="pv")
                        dacc = p2acc.tile([128, 1024], F16, tag="dacc")

                        def emit_pv(exp_pair, pk, npair, ps_o=ps_o, h=h):
                            for j in (0, 1):
                                kt = 2 * pk + j
                                nc.tensor.matmul(
                                    ps_o[:],
                                    v_t[:, kt, 128 * h:128 * (h + 1)],
                                    exp_pair[:, 512 * j:512 * (j + 1)],
                                    start=(kt == 0), stop=(kt == 2 * npair - 1),
                                )

                        pend = []   # (exp pair tile, pk) one pair behind
                        for pk in range(npair):
                            ps_s = ps_sp.tile([128, 1024], F32, tag="scores")
                            for j in (0, 1):
                                kt = 2 * pk + j
                                ks = slice(128 * kt, 128 * (kt + 1))
                                delta = 128 * kt - 512 * qb
                                diag = delta >= 0
                                half = ps_s[:, 512 * j:512 * (j + 1)]
                                nc.tensor.matmul(
                                    half, kT8_t[:, h, :, ks], q8_mov(h),
                                    start=True, stop=not diag,
                                    perf_mode=DR_MODE,
                                )
                                if diag:
                                    nc.tensor.matmul(
                                        half, id_t[:],
                                        mask_t[:, :, 384 - delta:896 - delta],
                                        start=False, stop=True,
                                        perf_mode=DR_MODE,
                                    )
                            exp_t = p2exp.tile([128, 1024], F16, tag="exp")
                            nc.scalar.activation(
                                exp_t[:], ps_s[:], AF.Exp, scale=EXP_SCALE)
                            # two alternating accumulators halve the serial
                            # add-chain latency on DVE
                            half = dacc[:, 512 * (pk % 2):512 * (pk % 2) + 512]
                            if pk < 2:
                                nc.vector.tensor_add(
                                    half, exp_t[:, 0:512], exp_t[:, 512:1024])
                            else:
                                nc.vector.tensor_add(
                                    half, half, exp_t[:, 0:512])
                                nc.vector.tensor_add(
                                    half, half, exp_t[:, 512:1024])
                            pend.append((exp_t, pk))
                            if len(pend) > 2:
                                emit_pv(*pend.pop(0), npair)
                        for e in pend:
                            emit_pv(*e, npair)
                        dfold = p2acc1.tile([128, 512], F16, tag="dfold")
                        if npair > 1:
                            nc.vector.tensor_add(
                                dfold[:], dacc[:, 0:512], dacc[:, 512:1024])
                        else:
                            nc.vector.tensor_copy(dfold[:], dacc[:, 0:512])
                        dred = p2acc1.tile([128, 512], F32, tag="dred")
                        nc.gpsimd.partition_all_reduce(
                            dred[:], dfold[:], 128, bass_isa.ReduceOp.add)
                        rsb = p2tmp.tile([128, 512], F32, tag="rsbd")
                        nc.vector.reciprocal(rsb[:], dred[:])
                        o16 = p2tmp.tile([128, 512], F16, tag="o16")
                        nc.vector.tensor_mul(o16[:], ps_o[:], rsb[:])
                        nc.vector.tensor_copy(o8[:, h, :], o16[:])
                        nc.vector.tensor_sub(o8r[:, h, :], o16[:], o8[:, h, :])
                        # interleave W_o d-tiles of the previous q-block so
                        # the in-order PE stream has fill work during this
                        # block's exp-latency stalls
                        if prev_out is not None:
                            emit_wo(prev_out, prev_cs,
                                    range(4 * h, 4 * (h + 1)))
                        # interleave the next block's q projections mid-stream
                        # instead of serializing them at the block boundary
                        if h == 1 and 2 <= qb + 1 < n_sb:
                            q8_next = compute_q8(qb + 1)
                    prev_out, prev_cs = (o8, o8r), cs
                emit_wo(prev_out, prev_cs, alt=True)
            p2w_stack.close()
            persist_stack.close()

    nc.compile()
    return nc


def host_prep(inputs, S=S_FULL):
    """Build the 8 per-core input maps from the full problem inputs."""
    FP8 = ml_dtypes.float8_e4m3

    def to8(a):
        return np.ascontiguousarray(a).astype(FP8)

    def split8(a, scale):
        hi = (a * scale).astype(FP8)
        lo = (a * scale - hi.astype(np.float32)).astype(FP8)
        return hi, lo

    x = np.asarray(inputs["x"], np.float32)
    cosT = np.asarray(inputs["rope_cos"], np.float32).T
    sinT = np.asarray(inputs["rope_sin"], np.float32).T
    c4 = np.concatenate([cosT, cosT, cosT, cosT], 0) * SQ8
    s4 = np.concatenate([-sinT, sinT, -sinT, sinT], 0) * SQ8
    c4 = np.ascontiguousarray(c4).astype(np.float16)
    s4 = np.ascontiguousarray(s4).astype(np.float16)
    qw = np.asarray(inputs["q_norm_w"], np.float32)
    kvw = np.asarray(inputs["kv_norm_w"], np.float32)
    W_uq = np.asarray(inputs["W_uq"], np.float32) * qw[:, None]
    W_qr = np.asarray(inputs["W_qr"], np.float32) * qw[:, None]
    W_uk = np.asarray(inputs["W_uk"], np.float32) * kvw[:, None]
    W_kr = np.asarray(inputs["W_kr"], np.float32) * kvw[:, None]
    W_uv = np.asarray(inputs["W_uv"], np.float32) * kvw[:, None]
    W_o = np.asarray(inputs["W_o"], np.float32)
    W_dq = np.asarray(inputs["W_dq"], np.float32)
    W_dkv = np.asarray(inputs["W_dkv"], np.float32)

    wdq8, wdq8r = split8(W_dq, BW)
    wdkv8, wdkv8r = split8(W_dkv, BW)
    wo8_full, wo8r_full = split8(W_o, BWO)
    wuq8, wuq8r = split8(W_uq, BW)
    wqr8, wqr8r = split8(W_qr, BW)
    wuk8, wuk8r = split8(W_uk, BW)
    wkr8, wkr8r = split8(W_kr, BW)
    wuv8, wuv8r = split8(W_uv, BW)

    # mask table: plane 0 = {0, -240} causal pattern, plane 1 = 0
    cgrid = np.arange(896)[None, :] - 384
    igrid = np.arange(128)[:, None]
    mask8 = np.zeros((128, 2, 896), np.float32)
    mask8[:, 0, :] = np.where(cgrid >= igrid, 0.0, -240.0)
    mask8 = mask8.astype(FP8)
    id8 = np.zeros((128, 2, 128), np.float32)
    id8[:, 0, :] = MASK_ID * np.eye(128, dtype=np.float32)
    id8 = id8.astype(FP8)
    ones8 = np.ones((128, 1), np.float32).astype(FP8)

    in_maps = []
    for c in range(NCORES):
        b, g = c // 4, c % 4
        hs = slice(4 * g * DN, 4 * (g + 1) * DN)
        hr = slice(4 * g * DRR, 4 * (g + 1) * DRR)
        xT = np.ascontiguousarray(x[b].T)
        x8, x8r = split8(xT, AX)
        in_maps.append(dict(
            x8=x8, x8r=x8r,
            xkv8=np.ascontiguousarray(x8[:, 512 * g:512 * (g + 1)]),
            xkv8r=np.ascontiguousarray(x8r[:, 512 * g:512 * (g + 1)]),
            W_dq8=wdq8, W_dq8r=wdq8r,
            W_dkv8=wdkv8, W_dkv8r=wdkv8r,
            Wuq8=np.ascontiguousarray(wuq8[:, hs]),
            Wuq8r=np.ascontiguousarray(wuq8r[:, hs]),
            Wqr8=np.ascontiguousarray(wqr8[:, hr]),
            Wqr8r=np.ascontiguousarray(wqr8r[:, hr]),
            Wuk8=np.ascontiguousarray(wuk8[:, hs]),
            Wuk8r=np.ascontiguousarray(wuk8r[:, hs]),
            Wkr8=np.ascontiguousarray(wkr8[:, hr]),
            Wkr8r=np.ascontiguousarray(wkr8r[:, hr]),
            Wuv8=np.ascontiguousarray(wuv8[:, hs]),
            Wuv8r=np.ascontiguousarray(wuv8r[:, hs]),
            Wo8=np.ascontiguousarray(wo8_full[512 * g:512 * (g + 1), :]),
            Wo8r=np.ascontiguousarray(wo8r_full[512 * g:512 * (g + 1), :]),
            c4=c4, s4=s4, mask8=mask8, id8=id8, ones8=ones8,
        ))
    return in_maps


_NC_CACHE = {}


def kernel(**inputs) -> np.ndarray:
    S = np.asarray(inputs["x"]).shape[1]
    if S not in _NC_CACHE:
        _NC_CACHE[S] = build_nc(S)
    nc = _NC_CACHE[S]
    in_maps = host_prep(inputs, S)
    res = run_bass_kernel_spmd(nc, in_maps, core_ids=list(range(NCORES)))
    y = np.empty((B, S, D), np.float32)
    for b in range(B):
        acc = res.results[4 * b]["yT"].astype(np.float32)
        for g in range(1, 4):
            acc = acc + res.results[4 * b + g]["yT"].astype(np.float32)
        y[b] = acc.T
    return y
